# revision 1
# baseline (speedup 1.0000x reference)
"""Trainium2 Bass kernel for nn_Cross_Attention — fp8 DoubleRow rewrite.

L1: convs + gram partials with fp8 DoubleRow matmuls (phases A-D), bf16 E/F.
L2: attn-apply with chunked v DMA. Host glue between (softmax etc.).
Sharding: 4 samples x 2 row-halves across 8 cores.
"""
import sys
sys.path.insert(0, "/opt/trn_rl_repo")
import numpy as np
import ml_dtypes

import concourse.bass as bass
import concourse.tile as tile
from concourse import bacc, mybir
from contextlib import ExitStack

BF16 = mybir.dt.bfloat16
F8 = mybir.dt.float8e4
F32 = mybir.dt.float32
bf16 = ml_dtypes.bfloat16
f8 = ml_dtypes.float8_e4m3
DR = mybir.MatmulPerfMode.DoubleRow

WS = 16.0          # fp8 weight pre-scale
WP = 130           # padded row width

TAPS = [(dy, dx) for dy in (-1, 0, 1) for dx in (-1, 0, 1)]
# taps sorted by flat offset dy*WP+dx (they already are, given WP>2)
TAP_OFF = [dy * WP + dx for dy, dx in TAPS]
# DoubleRow tap pairs for B/D: (idx_a, idx_b). The odd tap rides first with a
# zero-weight second tile at stride +1 (always in-bounds since tap 0 has the
# smallest offset).
BD_PAIRS = [(0, None), (1, 2), (3, 4), (5, 6), (7, 8)]


def qkv_halves():
    """Per (pb, half): (x1_base, qkv_base, ch0, nch).  ch0 = qkv-global channel."""
    out = []
    for pb in range(6):
        P, odd = pb // 2, pb % 2
        for h in (0, 1):
            nch = 3 if (odd and h == 1) else 63
            ch0 = 3 * (64 * P + 42 * odd + 21 * h)
            x1b = 64 * h
            qb = 64 * h if not odd else 64 * (1 - h)
            out.append((pb, h, x1b, qb, ch0, nch))
    return out


def qkv_channel_at(pb, p):
    """qkv-global channel stored at partition p of qkv pblock pb, or None."""
    for (pb2, h, x1b, qb, ch0, nch) in qkv_halves():
        if pb2 == pb and qb <= p < qb + nch:
            return ch0 + (p - qb)
    return None


# newk/newv input chunks with PLAIN dw layout (dw pb_i = kv ch 128i..128i+127):
# k = qkv ch 192-383 (pb 2,3); k_mask = kv ch 0-191 = dw pb0 + dw pb1[0:64]
# v = qkv ch 384-575 (pb 4,5); v_mask = kv ch 192-383 = dw pb1[64:128] + dw pb2
KCC = [("qkv", 2, 0, 128), ("qkv", 3, 0, 128), ("dw", 0, 0, 128), ("dw", 1, 0, 64)]
VCC = [("qkv", 4, 0, 128), ("qkv", 5, 0, 128), ("dw", 1, 64, 64), ("dw", 2, 0, 128)]


def _bcast(ap, p):
    return bass.AP(tensor=ap.tensor, offset=ap.offset, ap=[[0, p]] + list(ap.ap[1:]))


def _pair_ap(t, off, delta, dims):
    """AP over tile t at flat free-offset `off`: [partitions, 2 (stride delta), *dims]."""
    return bass.AP(tensor=t.tensor, offset=t.offset + off,
                   ap=[list(t.ap[0]), [delta, 2]] + [list(d) for d in dims])


def build_l1(R=64, W=128, S=16):
    assert W == 128 and R % S == 0 and S % 4 == 0
    NSLAB = R // S
    NS = S * W
    XCOLS = (S + 2) * WP
    MCOLS = (S + 4) * WP
    MC2 = MCOLS + 2
    N128 = NS // 128

    nc = bacc.Bacc("TRN2", target_bir_lowering=False, debug=False, num_devices=8)

    def din(name, shape, dt=F8):
        return nc.dram_tensor(name, shape, dt, kind="ExternalInput").ap()

    def dout(name, shape, dt=F32):
        return nc.dram_tensor(name, shape, dt, kind="ExternalOutput").ap()

    x8 = din("x8", [128, 2, (R + 2) * WP])
    xm_lo = din("xm_lo", [128, (R + 4) * WP])
    xm_d1 = din("xm_d1", [128, (R + 4) * WP])
    xm_d2 = din("xm_d2", [128, (R + 4) * WP])
    qw8 = din("qw8", [128, 2, 768])
    qdw8 = din("qdw8", [128, 6, 5, 2, 128])
    kv8 = din("kv8", [128, 7, 2, 384])
    kvdw8 = din("kvdw8", [128, 3, 5, 2, 128])
    newk_w_m = din("newk_w_m", [128, 4, 192], BF16)
    newv_wT = din("newv_wT", [128, 4, 192], BF16)
    ident = din("ident", [128, 128], BF16)
    ones_col = din("ones_col", [128, 1], BF16)
    x1_bias = din("x1_bias", [128, 6], F32)      # x16
    qkv_bias = din("qkv_bias", [128, 6], F32)
    kv_bias = din("kv_bias", [128, 3], F32)      # x16, plain layout
    kvdw_bias = din("kvdw_bias", [128, 3], F32)  # plain layout
    newk_b_row = din("newk_b_row", [1, 192], F32)
    newv_bias = din("newv_bias", [128, 2], F32)
    mask_rc = din("mask_rc", [1, (R + 2) * WP], BF16)  # 1/16 at valid, 0 at pad

    v_out = dout("v_out", [192, R * W], BF16)
    gram_out = dout("gram_out", [192, 256])
    kss_out = dout("kss_out", [1, 192])
    qstats_out = dout("qstats_out", [128, 2, 2])
    vstats_out = dout("vstats_out", [128, 2, 2])

    # C-phase DoubleRow k-tile pair coordinates in the [128, 3, MC2] msl tile
    # flat free space: region*MC2 + off.  Region 0 = lo channels, 1 = d1
    # (hi | hi<<WP), 2 = d2 (hi | hi<<2).
    def lo(dy, dx):
        return (1 + dy) * WP + dx + 1

    def d1(off):
        return MC2 + off

    def d2(off):
        return 2 * MC2 + off

    # 14 k-tiles -> 7 pairs; weight slot kv8[:, pair, j, :] must match.
    CPAIRS = [
        (lo(-1, -1), lo(-1, 0)), (lo(-1, 1), lo(0, -1)), (lo(0, 0), lo(0, 1)),
        (lo(1, -1), lo(1, 0)), (lo(1, 1), d1(0)), (d1(1), d1(2)),
        (d1(2 * WP + 1), d2(2 * WP)),
    ]

    with tile.TileContext(nc) as tc, ExitStack() as ctx:
        wpool = ctx.enter_context(tc.tile_pool(name="weights", bufs=1))
        xpool = ctx.enter_context(tc.tile_pool(name="xslab", bufs=2))
        bigpool = ctx.enter_context(tc.tile_pool(name="big", bufs=1))
        midpool = ctx.enter_context(tc.tile_pool(name="mid", bufs=2))
        smpool = ctx.enter_context(tc.tile_pool(name="small", bufs=4))
        statpool = ctx.enter_context(tc.tile_pool(name="stats", bufs=1))
        pspool = ctx.enter_context(tc.tile_pool(name="ps", bufs=5, space="PSUM"))
        pspers = ctx.enter_context(tc.tile_pool(name="pspers", bufs=1, space="PSUM"))

        def load1(ap_in, shape, dt=F8, eng=None):
            t = wpool.tile(shape, dt, tag=ap_in.tensor.name)
            (eng or nc.sync).dma_start(out=t[:ap_in.shape[0]], in_=ap_in[:])
            return t

        g = nc.gpsimd
        qw8_s = load1(qw8, [128, 2, 768])
        x1b_s = load1(x1_bias, [128, 6], F32)
        qdw8_s = load1(qdw8, [128, 6, 5, 2, 128], eng=g)
        kv8_s = load1(kv8, [128, 7, 2, 384], eng=g)
        kvdw8_s = load1(kvdw8, [128, 3, 5, 2, 128], eng=g)
        nkw_s = load1(newk_w_m, [128, 4, 192], BF16, eng=g)
        nvw_s = load1(newv_wT, [128, 4, 192], BF16, eng=g)
        id_s = load1(ident, [128, 128], BF16, eng=g)
        ones_s = load1(ones_col, [128, 1], BF16, eng=g)
        qkvb_s = load1(qkv_bias, [128, 6], F32, eng=g)
        kvb_s = load1(kv_bias, [128, 3], F32, eng=g)
        dwb_s = load1(kvdw_bias, [128, 3], F32, eng=g)
        nvb_s = load1(newv_bias, [128, 2], F32, eng=g)
        nkb_bc = wpool.tile([128, 192], F32, tag="nkb_bc")
        nc.gpsimd.dma_start(out=nkb_bc[:], in_=_bcast(newk_b_row[0:1, :], 128))

        # persistent accumulators: one PSUM bank each (zero-region granularity)
        gramA = pspers.tile([128, 256], F32)
        gramB = pspers.tile([64, 256], F32)
        kss_ps = pspers.tile([1, 192], F32)

        qstats = statpool.tile([128, 2, NSLAB * (NS // 512), 6], F32)
        vstats = statpool.tile([128, 2, NSLAB * (NS // 512), 6], F32)

        n128_total = NSLAB * N128
        CT = [(c0, min(512, XCOLS - c0)) for c0 in range(0, XCOLS, 512)]

        for s in range(NSLAB):
            xsl = xpool.tile([128, 2, XCOLS], F8, tag="xsl")
            msl = xpool.tile([128, 3, MC2], F8, tag="msl")
            off = s * S * WP
            nc.sync.dma_start(out=xsl[:], in_=x8[:, :, off:off + XCOLS])
            nc.sync.dma_start(out=msl[:, 0, 1:1 + MCOLS], in_=xm_lo[:, off:off + MCOLS])
            nc.sync.dma_start(out=msl[:, 1, 1:1 + MCOLS], in_=xm_d1[:, off:off + MCOLS])
            nc.sync.dma_start(out=msl[:, 2, 1:1 + MCOLS], in_=xm_d2[:, off:off + MCOLS])
            for r in range(3):
                nc.vector.memset(msl[:, r, 0:1], 0.0)
                nc.vector.memset(msl[:, r, MC2 - 1:MC2], 0.0)
            mtile = xpool.tile([128, XCOLS], BF16, tag="mtile")
            nc.sync.dma_start(out=mtile[:], in_=_bcast(mask_rc[0:1, off:off + XCOLS], 128))

            # ---- Phase A: x1 = 1x1(x); one DR matmul per (pb, col-tile)
            x1 = bigpool.tile([128, 6, XCOLS], F8, tag="x1")
            for pb in range(6):
                for c0, cs in CT:
                    ps = pspool.tile([128, 512], F32, tag="ps", name=f"psA{s}_{pb}_{c0}")
                    nc.tensor.matmul(ps[:, :cs], qw8_s[:, :, 128 * pb:128 * pb + 128],
                                     xsl[:, :, c0:c0 + cs], start=True, stop=True,
                                     perf_mode=DR)
                    nc.vector.scalar_tensor_tensor(
                        out=x1[:, pb, c0:c0 + cs], in0=ps[:, :cs],
                        scalar=x1b_s[:, pb:pb + 1], in1=mtile[:, c0:c0 + cs],
                        op0=mybir.AluOpType.add, op1=mybir.AluOpType.mult)

            # ---- Phase C: kv1 = 3x3(xm); 7 DR pairs per (pb, col-tile)
            kv1 = bigpool.tile([128, 3, XCOLS], F8, tag="kv1")
            for pb in range(3):
                for cset in (CT[:3], CT[3:]):
                    pss = [pspool.tile([128, 512], F32, tag="ps",
                                       name=f"psC{s}_{pb}_{c0}")
                           for (c0, cs) in cset]
                    for pj, (ca, cb) in enumerate(CPAIRS):
                        lhsT = kv8_s[:, pj, :, 128 * pb:128 * pb + 128]
                        for ti, (c0, cs) in enumerate(cset):
                            rhs = _pair_ap(msl, ca + c0, cb - ca, [[1, cs]])
                            nc.tensor.matmul(pss[ti][:, :cs], lhsT, rhs,
                                             start=(pj == 0), stop=(pj == 6),
                                             perf_mode=DR)
                    for ti, (c0, cs) in enumerate(cset):
                        nc.vector.scalar_tensor_tensor(
                            out=kv1[:, pb, c0:c0 + cs], in0=pss[ti][:, :cs],
                            scalar=kvb_s[:, pb:pb + 1], in1=mtile[:, c0:c0 + cs],
                            op0=mybir.AluOpType.add, op1=mybir.AluOpType.mult)

            # ---- Phase B: qkv = qdw(x1); 2-row groups (N=258), 5 DR tap-pairs
            # per group chained in PSUM; 4-group sets amortize weight loads.
            def dwconv(src, wsrc, dst, npb, bias_s, tag):
                for pb in range(npb):
                    for st in range(2):
                        gset = list(range(st * 4, st * 4 + 4))
                        pss = {gi: pspool.tile([128, 258], F32, tag="ps",
                                               name=f"ps{tag}{s}_{pb}_{gi}")
                               for gi in gset}
                        for pj, (ta, tb) in enumerate(BD_PAIRS):
                            oa = TAP_OFF[ta]
                            delta = (TAP_OFF[tb] - oa) if tb is not None else 1
                            lhsT = wsrc[:, pb, pj, :, :]
                            for gi in gset:
                                base = pb * XCOLS + (2 * gi + 1) * WP + 1 + oa
                                rhs = _pair_ap(src, base, delta, [[1, 258]])
                                nc.tensor.matmul(pss[gi][:, :], lhsT, rhs,
                                                 start=(pj == 0), stop=(pj == 4),
                                                 perf_mode=DR)
                        for gi in gset:
                            ps = pss[gi]
                            in_ap = bass.AP(tensor=ps.tensor, offset=ps.offset,
                                            ap=[list(ps.ap[0]), [WP, 2], [1, 128]])
                            nc.scalar.activation(
                                out=dst[:, pb, 2 * gi * 128:(2 * gi + 2) * 128],
                                in_=in_ap,
                                func=mybir.ActivationFunctionType.Identity,
                                bias=bias_s[:, pb:pb + 1], scale=1.0 / WS)

            qkv = bigpool.tile([128, 6, NS], BF16, tag="qkv")
            dwconv(x1, qdw8_s, qkv, 6, qkvb_s, "B")

            # ---- Phase D: dw = kvdw(kv1); same structure, 3 pblocks
            dw = bigpool.tile([128, 3, NS], BF16, tag="dw")
            dwconv(kv1, kvdw8_s, dw, 3, dwb_s, "D")

            # ---- Phase E: v = newv(v_cc) + stats + dma out (bf16)
            vt = midpool.tile([128, 2, NS], BF16, tag="vt")
            for mb in range(2):
                msz = 128 if mb == 0 else 64
                pss = [pspool.tile([128, 512], F32, tag="ps", name=f"psE{s}_{mb}_{ic}")
                       for ic in range(NS // 512)]
                for j, (src, pb, base, sz) in enumerate(VCC):
                    data = (qkv if src == "qkv" else dw)
                    lhsT = nvw_s[base:base + sz, j, mb * 128:mb * 128 + msz]
                    for ic in range(NS // 512):
                        rhs = data[base:base + sz, pb, ic * 512:(ic + 1) * 512]
                        nc.tensor.matmul(pss[ic][:msz, :], lhsT, rhs, start=(j == 0),
                                         stop=(j == 3),
                                         tile_position=(base, 0) if base else None)
                for ic in range(NS // 512):
                    nc.scalar.activation(out=vt[:msz, mb, ic * 512:(ic + 1) * 512],
                                         in_=pss[ic][:msz, :],
                                         func=mybir.ActivationFunctionType.Identity,
                                         bias=nvb_s[:msz, mb:mb + 1], scale=1.0)
            nc.sync.dma_start(out=v_out[0:128, s * NS:(s + 1) * NS], in_=vt[:, 0, :])
            nc.sync.dma_start(out=v_out[128:192, s * NS:(s + 1) * NS], in_=vt[:64, 1, :])
            for sub in range(NS // 512):
                si = s * (NS // 512) + sub
                sl = slice(sub * 512, (sub + 1) * 512)
                nc.vector.bn_stats(out=vstats[:, 0, si, :], in_=vt[:, 0, sl])
                nc.vector.bn_stats(out=vstats[:64, 1, si, :], in_=vt[:64, 1, sl])
                nc.vector.bn_stats(out=qstats[:, 0, si, :], in_=qkv[:, 0, sl])
                nc.vector.bn_stats(out=qstats[:, 1, si, :], in_=qkv[:, 1, sl])

            # ---- Phase F: per 128-n chunk: k_T, q_T, gram, kss
            for ic in range(N128):
                c0 = ic * 128
                gidx = s * N128 + ic
                kps = pspool.tile([128, 192], F32, tag="ps", name=f"kps{s}_{ic}")
                for j, (src, pb, base, sz) in enumerate(KCC):
                    data = (qkv if src == "qkv" else dw)
                    lhsT = data[base:base + sz, pb, c0:c0 + 128]
                    rhs = nkw_s[base:base + sz, j, :]
                    nc.tensor.matmul(kps[:, :], lhsT, rhs, start=(j == 0), stop=(j == 3),
                                     tile_position=(base, 0) if base else None)
                kT = smpool.tile([128, 192], BF16, tag="kT")
                nc.vector.scalar_tensor_tensor(
                    out=kT[:], in0=kps[:], scalar=1.0, in1=nkb_bc[:],
                    op0=mybir.AluOpType.mult, op1=mybir.AluOpType.add)
                qps = pspool.tile([128, 256], BF16, tag="ps", name=f"qps{s}_{ic}")
                nc.tensor.transpose(qps[:, 0:128], qkv[:, 0, c0:c0 + 128], id_s[:, :])
                nc.tensor.transpose(qps[:, 128:256], qkv[:, 1, c0:c0 + 128], id_s[:, :])
                qT = smpool.tile([128, 256], BF16, tag="qT")
                nc.scalar.copy(out=qT[:], in_=qps[:])
                nc.tensor.matmul(gramA[:, :], kT[:, 0:128], qT[:],
                                 start=(gidx == 0), stop=(gidx == n128_total - 1))
                nc.tensor.matmul(gramB[:, :], kT[:, 128:192], qT[:],
                                 start=(gidx == 0), stop=(gidx == n128_total - 1))
                ksq = smpool.tile([128, 192], BF16, tag="ksq")
                nc.gpsimd.tensor_mul(ksq[:], kT[:], kT[:])
                nc.tensor.matmul(kss_ps[:, :], ones_s[:, :], ksq[:],
                                 start=(gidx == 0), stop=(gidx == n128_total - 1))

        qmv = statpool.tile([128, 2, 2], F32)
        vmv = statpool.tile([128, 2, 2], F32)
        nc.vector.memset(qmv[:], 0.0)
        nc.vector.memset(vmv[:], 0.0)
        nc.vector.bn_aggr(out=qmv[:, 0, :], in_=qstats[:, 0, :, :])
        nc.vector.bn_aggr(out=qmv[:, 1, :], in_=qstats[:, 1, :, :])
        nc.vector.bn_aggr(out=vmv[:, 0, :], in_=vstats[:, 0, :, :])
        nc.vector.bn_aggr(out=vmv[:64, 1, :], in_=vstats[:64, 1, :, :])
        nc.sync.dma_start(out=qstats_out[:], in_=qmv[:])
        nc.sync.dma_start(out=vstats_out[:], in_=vmv[:])
        gA = statpool.tile([128, 256], F32)
        gB = statpool.tile([64, 256], F32)
        kssb = statpool.tile([1, 192], F32)
        nc.scalar.copy(out=gA[:], in_=gramA[:])
        nc.scalar.copy(out=gB[:], in_=gramB[:])
        nc.scalar.copy(out=kssb[:], in_=kss_ps[:])
        nc.sync.dma_start(out=gram_out[0:128, :], in_=gA[:])
        nc.sync.dma_start(out=gram_out[128:192, :], in_=gB[:])
        nc.sync.dma_start(out=kss_out[:], in_=kssb[:])

    nc.compile()
    return nc


def build_l2(R=64, W=128):
    NS = R * W
    nc = bacc.Bacc("TRN2", target_bir_lowering=False, debug=False, num_devices=8)
    v_in = nc.dram_tensor("v_in", [192, NS], BF16, kind="ExternalInput").ap()
    awT = nc.dram_tensor("awT", [128, 2, 192], BF16, kind="ExternalInput").ap()
    pbias = nc.dram_tensor("pbias", [128, 2], F32, kind="ExternalInput").ap()
    out = nc.dram_tensor("out", [192, NS], BF16, kind="ExternalOutput").ap()

    with tile.TileContext(nc) as tc, ExitStack() as ctx:
        wpool = ctx.enter_context(tc.tile_pool(name="w", bufs=1))
        vpool = ctx.enter_context(tc.tile_pool(name="v", bufs=4))
        opool = ctx.enter_context(tc.tile_pool(name="o", bufs=4))
        pspool = ctx.enter_context(tc.tile_pool(name="ps", bufs=4, space="PSUM"))

        aw = wpool.tile([128, 2, 192], BF16)
        nc.sync.dma_start(out=aw[:], in_=awT[:])
        pb = wpool.tile([128, 2], F32)
        nc.sync.dma_start(out=pb[:], in_=pbias[:])

        for c0 in range(0, NS, 512):
            vt = vpool.tile([128, 2, 512], BF16, tag="vt")
            nc.sync.dma_start(out=vt[:, 0, :], in_=v_in[0:128, c0:c0 + 512])
            nc.sync.dma_start(out=vt[:64, 1, :], in_=v_in[128:192, c0:c0 + 512])
            for mb in range(2):
                msz = 128 if mb == 0 else 64
                ps = pspool.tile([128, 512], F32, tag="ps")
                nc.tensor.matmul(ps[:msz, :], aw[:, 0, mb * 128:mb * 128 + msz],
                                 vt[:, 0, :], start=True, stop=False)
                nc.tensor.matmul(ps[:msz, :], aw[:64, 1, mb * 128:mb * 128 + msz],
                                 vt[:64, 1, :], start=False, stop=True)
                ot = opool.tile([128, 512], BF16, tag="ot")
                nc.scalar.activation(out=ot[:msz, :], in_=ps[:msz, :],
                                     func=mybir.ActivationFunctionType.Identity,
                                     bias=pb[:msz, mb:mb + 1], scale=1.0)
                nc.sync.dma_start(out=out[mb * 128:mb * 128 + msz, c0:c0 + 512],
                                  in_=ot[:msz, :])
    nc.compile()
    return nc


# ---------------- host-side prep ----------------

def prep_weights(w):
    """w: dict of reference weights (numpy f32). Returns dict of L1 input arrays."""
    out = {}
    qw = w["q_w"][:, :, 0, 0]          # (576, 192)
    qwT = np.zeros((128, 2, 768), np.float32)
    for (pb, h, x1b, qb, ch0, nch) in qkv_halves():
        win = 128 * pb + 64 * h
        qwT[0:128, 0, win:win + nch] = qw.T[0:128, ch0:ch0 + nch]
        qwT[0:64, 1, win:win + nch] = qw.T[128:192, ch0:ch0 + nch]
    out["qw8"] = (qwT * WS).astype(f8)

    # qdw: grouped 3x3, (tap, pb) 128x128 block; repack into DR tap pairs
    qdw = w["qdw_w"]                   # (576, 3, 3, 3)
    qdwT = np.zeros((9, 6, 128, 128), np.float32)   # (tap_sorted, pb, row, col)
    for ti, (dy, dx) in enumerate(TAPS):
        for (pb, h, x1b, qb, ch0, nch) in qkv_halves():
            for gl in range(nch // 3):
                for i in range(3):
                    for j in range(3):
                        qdwT[ti, pb, x1b + 3 * gl + i, qb + 3 * gl + j] = \
                            qdw[ch0 + 3 * gl + j, i, dy + 1, dx + 1]
    qdw8 = np.zeros((128, 6, 5, 2, 128), np.float32)
    for pj, (ta, tb) in enumerate(BD_PAIRS):
        for pb in range(6):
            qdw8[:, pb, pj, 0, :] = qdwT[ta, pb]
            if tb is not None:
                qdw8[:, pb, pj, 1, :] = qdwT[tb, pb]
    out["qdw8"] = (qdw8 * WS).astype(f8)

    # kv conv: 14 k-tiles -> 7 DR pairs (order must match CPAIRS in build_l1)
    kvw = w["kv_w"]                    # (384, 192, 3, 3)
    tl = lambda dy, dx: kvw[:, 0:128, dy + 1, dx + 1].T       # (128, 384) lo
    th = lambda dy, dx: kvw[:, 128:192, dy + 1, dx + 1].T     # (64, 384) hi
    ktiles = []
    for dy, dx in TAPS:   # 9 lo tiles (sorted tap order == TAPS order)
        ktiles.append(("lo", (dy, dx)))
    ktiles.append(("d1", -1)); ktiles.append(("d1", 0)); ktiles.append(("d1", 1))
    ktiles.append(("single", None)); ktiles.append(("d2", None))
    kv8 = np.zeros((128, 7, 2, 384), np.float32)
    for pj in range(7):
        for j in range(2):
            kind, arg = ktiles[2 * pj + j]
            blk = np.zeros((128, 384), np.float32)
            if kind == "lo":
                blk[0:128] = tl(*arg)
            elif kind == "d1":
                blk[0:64] = th(-1, arg)
                blk[64:128] = th(0, arg)
            elif kind == "d2":
                blk[0:64] = th(1, -1)
                blk[64:128] = th(1, 1)
            elif kind == "single":
                blk[0:64] = th(1, 0)
            kv8[:, pj, j, :] = blk
    out["kv8"] = (kv8 * WS).astype(f8)

    # kvdw depthwise: plain layout, diag blocks per pblock, DR tap pairs
    kvdw = w["kvdw_w"][:, 0]           # (384, 3, 3)
    kvdw8 = np.zeros((128, 3, 5, 2, 128), np.float32)
    for pj, (ta, tb) in enumerate(BD_PAIRS):
        for pb in range(3):
            dya, dxa = TAPS[ta]
            kvdw8[:, pb, pj, 0, :] = np.diag(kvdw[128 * pb:128 * pb + 128, dya + 1, dxa + 1])
            if tb is not None:
                dyb, dxb = TAPS[tb]
                kvdw8[:, pb, pj, 1, :] = np.diag(kvdw[128 * pb:128 * pb + 128, dyb + 1, dxb + 1])
    out["kvdw8"] = (kvdw8 * WS).astype(f8)

    def dw_channel(pb, p):
        return 128 * pb + p   # plain layout

    nk = w["newk_w"][:, :, 0, 0]       # (192, 384): in = [k(192) | k_mask(192)]
    nkm = np.zeros((128, 4, 192), np.float32)
    for j, (src, pb, base, sz) in enumerate(KCC):
        for p in range(base, base + sz):
            if src == "qkv":
                ch = qkv_channel_at(pb, p)
                if ch is not None:
                    nkm[p, j, :] = nk[:, ch - 192]      # k part: qkv ch 192-383
            else:
                ch = dw_channel(pb, p)
                if ch < 192:
                    nkm[p, j, :] = nk[:, 192 + ch]      # k_mask: dw ch 0-191
    out["newk_w_m"] = nkm.astype(bf16)

    nv = w["newv_w"][:, :, 0, 0]       # (192, 384): in = [v(192) | v_mask(192)]
    nvT = np.zeros((128, 4, 192), np.float32)
    for j, (src, pb, base, sz) in enumerate(VCC):
        for p in range(base, base + sz):
            if src == "qkv":
                ch = qkv_channel_at(pb, p)
                if ch is not None:
                    nvT[p, j, :] = nv[:, ch - 384]      # v part: qkv ch 384-575
            else:
                ch = dw_channel(pb, p)
                if ch >= 192:
                    nvT[p, j, :] = nv[:, ch]            # v_mask: dw ch 192-383
    out["newv_wT"] = nvT.astype(bf16)

    out["ident"] = np.eye(128, dtype=bf16)
    out["ones_col"] = np.ones((128, 1), dtype=bf16)

    x1b = np.zeros((128, 6), np.float32)
    qkvb = np.zeros((128, 6), np.float32)
    for (pb, h, x1b_base, qb, ch0, nch) in qkv_halves():
        x1b[x1b_base:x1b_base + nch, pb] = w["q_b"][ch0:ch0 + nch]
        qkvb[qb:qb + nch, pb] = w["qdw_b"][ch0:ch0 + nch]
    out["x1_bias"] = x1b * WS
    out["qkv_bias"] = qkvb

    kvb = np.zeros((128, 3), np.float32)
    dwb = np.zeros((128, 3), np.float32)
    for pb in range(3):
        kvb[:, pb] = w["kv_b"][128 * pb:128 * pb + 128]
        dwb[:, pb] = w["kvdw_b"][128 * pb:128 * pb + 128]
    out["kv_bias"] = kvb * WS
    out["kvdw_bias"] = dwb
    out["newk_b_row"] = w["newk_b"][None, :].astype(np.float32)
    nvb = np.zeros((128, 2), np.float32)
    nvb[:, 0] = w["newv_b"][0:128]
    nvb[0:64, 1] = w["newv_b"][128:192]
    out["newv_bias"] = nvb
    return out


def prep_masks(R, H, half):
    m = np.zeros((R + 2, WP), np.float32)
    for r in range(R + 2):
        g = half * R + (r - 1)
        if 0 <= g < H:
            m[r, 1:129] = 1.0 / WS
    return m.reshape(1, -1)


def prep_core(x, xm, b, half, R, H):
    xp = np.zeros((192, R + 2, WP), np.float32)
    mp = np.zeros((192, R + 4, WP), np.float32)
    for r in range(R + 2):
        g = half * R + (r - 1)
        if 0 <= g < H:
            xp[:, r, 1:129] = x[b, :, g, :]
    for r in range(R + 4):
        g = half * R + (r - 2)
        if 0 <= g < H:
            mp[:, r, 1:129] = xm[b, :, g, :]
    xp = xp.reshape(192, -1)
    x8 = np.zeros((128, 2, xp.shape[1]), np.float32)
    x8[:, 0, :] = xp[0:128]
    x8[0:64, 1, :] = xp[128:192]
    x8[64:128, 1, :] = xp[128:192]   # dup (weights zero) to avoid NaN garbage
    mp = mp.reshape(192, -1)
    L = mp.shape[1]
    hi = mp[128:192]
    d1 = np.zeros((128, L), np.float32)
    d2 = np.zeros((128, L), np.float32)
    d1[0:64] = hi
    d1[64:128, :L - WP] = hi[:, WP:]
    d2[0:64] = hi
    d2[64:128, :L - 2] = hi[:, 2:]
    return {
        "x8": x8.astype(f8),
        "xm_lo": mp[0:128].astype(f8),
        "xm_d1": d1.astype(f8), "xm_d2": d2.astype(f8),
        "mask_rc": prep_masks(R, H, half).astype(bf16),
    }


# ---------------- host glue (unchanged semantics) ----------------

def _q_maps():
    part = np.zeros(192, np.int64)
    pblk = np.zeros(192, np.int64)
    for (pb, h, x1b, qb, ch0, nch) in qkv_halves():
        if pb >= 2:
            continue
        for i in range(nch):
            pblk[ch0 + i] = pb
            part[ch0 + i] = qb + i
    return pblk, part


def _ss_from_qstats(stats, n_half):
    pblk, part = _q_maps()
    mv = stats.astype(np.float64)
    return (mv[part, pblk, 1] + mv[part, pblk, 0] ** 2) * n_half


def _ss_from_vstats(stats, n_half):
    ss = np.zeros(192, np.float64)
    mv = stats.astype(np.float64)
    ss[0:128] = (mv[0:128, 0, 1] + mv[0:128, 0, 0] ** 2) * n_half
    ss[128:192] = (mv[0:64, 1, 1] + mv[0:64, 1, 0] ** 2) * n_half
    return ss


def glue(res0, res1, temperature, proj_w, proj_b, n_half):
    """Combine two half-core L1 results -> L2 inputs (awT, pbias)."""
    G = res0["gram_out"].astype(np.float64) + res1["gram_out"].astype(np.float64)
    pblk, part = _q_maps()
    qcol = pblk * 128 + part
    G = G[:, qcol]                              # (d, c): sum_n k[d,n] q[c,n]
    qss = _ss_from_qstats(res0["qstats_out"], n_half) + _ss_from_qstats(res1["qstats_out"], n_half)
    vss = _ss_from_vstats(res0["vstats_out"], n_half) + _ss_from_vstats(res1["vstats_out"], n_half)
    kss = (res0["kss_out"].astype(np.float64) + res1["kss_out"].astype(np.float64))[0]
    qn = np.maximum(np.sqrt(qss), 1e-12)
    kn = np.maximum(np.sqrt(kss), 1e-12)
    vn = np.maximum(np.sqrt(vss), 1e-12)
    A = G.T / (qn[:, None] * kn[None, :])      # (c, d)
    M = np.zeros((192, 192), np.float64)
    t = np.asarray(temperature).reshape(-1)
    for h in range(8):
        sl = slice(24 * h, 24 * h + 24)
        a = A[sl, sl] * t[h]
        a = a - a.max(axis=-1, keepdims=True)
        e = np.exp(a)
        sm = e / e.sum(axis=-1, keepdims=True)
        M[sl, sl] = sm / vn[None, sl]
    At = proj_w[:, :, 0, 0].astype(np.float64) @ M   # (out-ch o, d)
    awT = np.zeros((128, 2, 192), np.float32)
    awT[:, 0, :] = At.T[0:128]
    awT[0:64, 1, :] = At.T[128:192]
    pbias = np.zeros((128, 2), np.float32)
    pbias[:, 0] = proj_b[0:128]
    pbias[0:64, 1] = proj_b[128:192]
    return {"awT": awT.astype(bf16), "pbias": pbias}


# ---------------- driver: kernel(**inputs) ----------------
from concourse.bass_utils import run_bass_kernel_spmd

R_FULL, H_FULL, B_FULL = 64, 128, 4
_NC1 = None
_NC2 = None


def _get_progs():
    global _NC1, _NC2
    if _NC1 is None:
        _NC1 = build_l1(R=R_FULL, S=16)
        _NC2 = build_l2(R=R_FULL)
    return _NC1, _NC2


def kernel(**inputs):
    inputs = {k: np.asarray(v) for k, v in inputs.items()}
    x, xm = inputs["x"], inputs["x_mask"]
    nc1, nc2 = _get_progs()
    wprep = prep_weights(inputs)
    in_maps = []
    for core in range(8):
        b, half = core // 2, core % 2
        m = dict(wprep)
        m.update(prep_core(x, xm, b, half, R_FULL, H_FULL))
        in_maps.append(m)
    res1 = run_bass_kernel_spmd(nc1, in_maps, list(range(8))).results

    n_half = R_FULL * 128
    in_maps2 = []
    for core in range(8):
        b, half = core // 2, core % 2
        if half == 0:
            l2c = glue(res1[2 * b], res1[2 * b + 1], inputs["temperature"],
                       inputs["proj_w"], inputs["proj_b"], n_half)
        m = dict(l2c)
        m["v_in"] = res1[core]["v_out"]
        in_maps2.append(m)
    res2 = run_bass_kernel_spmd(nc2, in_maps2, list(range(8))).results

    out = np.empty((B_FULL, 192, H_FULL, 128), np.float32)
    for core in range(8):
        b, half = core // 2, core % 2
        out[b, :, half * R_FULL:(half + 1) * R_FULL, :] = \
            res2[core]["out"].reshape(192, R_FULL, 128).astype(np.float32)
    return out



# revision 18
# speedup vs baseline: 1.0058x; 1.0058x over previous
"""Trainium2 Bass kernel for nn_Cross_Attention — fp8 DoubleRow rewrite.

L1: convs + gram partials with fp8 DoubleRow matmuls (phases A-D), fp8 F-phase
(newk/gram/kss via DR), bf16 E. L2: attn-apply with chunked v DMA. Host glue
between (softmax etc.). Sharding: 4 samples x 2 row-halves across 8 cores.
"""
import sys
sys.path.insert(0, "/opt/trn_rl_repo")
import numpy as np
import ml_dtypes

import concourse.bass as bass
import concourse.tile as tile
from concourse import bacc, mybir
from contextlib import ExitStack

BF16 = mybir.dt.bfloat16
F8 = mybir.dt.float8e4
F32 = mybir.dt.float32
bf16 = ml_dtypes.bfloat16
f8 = ml_dtypes.float8_e4m3
DR = mybir.MatmulPerfMode.DoubleRow

WS = 16.0          # fp8 weight pre-scale
WP = 130           # padded row width
NKS = 32.0         # newk weight pre-scale (kT stored as NKS*k0)
QTS = 16.0         # qT pre-scale (qT stored as QTS*q)
KSQS = 64.0        # ksq stored as (KSQS*k0)**2
ONE_ROWS = (32, 33)  # junk q pb1 partitions hijacked as 1.0 cols in qT8 (ksum/kss);
                     # must be 32-aligned (engine partition-base alignment)

TAPS = [(dy, dx) for dy in (-1, 0, 1) for dx in (-1, 0, 1)]
# taps sorted by flat offset dy*WP+dx (they already are, given WP>2)
TAP_OFF = [dy * WP + dx for dy, dx in TAPS]
# DoubleRow tap pairs for B/D: (idx_a, idx_b). The odd tap rides first with a
# zero-weight second tile at stride +1 (always in-bounds since tap 0 has the
# smallest offset).
BD_PAIRS = [(0, None), (1, 2), (3, 4), (5, 6), (7, 8)]


def qkv_halves():
    """Per (pb, half): (x1_base, qkv_base, ch0, nch).  ch0 = qkv-global channel."""
    out = []
    for pb in range(6):
        P, odd = pb // 2, pb % 2
        for h in (0, 1):
            nch = 3 if (odd and h == 1) else 63
            ch0 = 3 * (64 * P + 42 * odd + 21 * h)
            x1b = 64 * h
            qb = 64 * h if not odd else 64 * (1 - h)
            out.append((pb, h, x1b, qb, ch0, nch))
    return out


def qkv_channel_at(pb, p):
    """qkv-global channel stored at partition p of qkv pblock pb, or None."""
    for (pb2, h, x1b, qb, ch0, nch) in qkv_halves():
        if pb2 == pb and qb <= p < qb + nch:
            return ch0 + (p - qb)
    return None


# newk/newv input chunks with PLAIN dw layout (dw pb_i = kv ch 128i..128i+127):
# k = qkv ch 192-383 (pb 2,3); k_mask = kv ch 0-191 = dw pb0 + dw pb1[0:64]
# v = qkv ch 384-575 (pb 4,5); v_mask = kv ch 192-383 = dw pb1[64:128] + dw pb2
# New tile routing: q_t = qkv pb0,1 (bf16); k8 = qkv pb2,3 (f8);
# vq = qkv pb4,5 (bf16); dwk8 = dw pb0 + dw pb1[0:64] (f8, slot1 parts 64-127
# zeroed); dwv = dw pb1[64:128] + dw pb2 (bf16, slot0 parts 64-127 used).
VCC = [("vq", 0, 0, 128), ("vq", 1, 0, 128), ("dwv", 0, 64, 64), ("dwv", 1, 0, 128)]


def _bcast(ap, p):
    return bass.AP(tensor=ap.tensor, offset=ap.offset, ap=[[0, p]] + list(ap.ap[1:]))


def _pair_ap(t, off, delta, dims):
    """AP over tile t at flat free-offset `off`: [partitions, 2 (stride delta), *dims]."""
    return bass.AP(tensor=t.tensor, offset=t.offset + off,
                   ap=[list(t.ap[0]), [delta, 2]] + [list(d) for d in dims])


def build_l1(R=64, W=128, S=16):
    assert W == 128 and R % S == 0 and S % 4 == 0
    NSLAB = R // S
    NS = S * W
    XCOLS = (S + 2) * WP
    MCOLS = (S + 4) * WP
    MC2 = MCOLS + 2
    N128 = NS // 128

    nc = bacc.Bacc("TRN2", target_bir_lowering=False, debug=False, num_devices=8)

    def din(name, shape, dt=F8):
        return nc.dram_tensor(name, shape, dt, kind="ExternalInput").ap()

    def dout(name, shape, dt=F32):
        return nc.dram_tensor(name, shape, dt, kind="ExternalOutput").ap()

    x8 = din("x8", [128, 2, (R + 2) * WP])
    xm_lo = din("xm_lo", [128, (R + 4) * WP])
    xm_d1 = din("xm_d1", [128, (R + 4) * WP])
    xm_d2 = din("xm_d2", [128, (R + 4) * WP])
    qw8 = din("qw8", [128, 2, 768])
    qdw8 = din("qdw8", [128, 6, 5, 2, 128])
    kv8 = din("kv8", [128, 7, 2, 384])
    kvdw8 = din("kvdw8", [128, 3, 5, 2, 128])
    nkw8 = din("nkw8", [128, 2, 2, 192], F8)
    newv_wT = din("newv_wT", [128, 4, 192], BF16)
    ident = din("ident", [128, 128], BF16)
    x1_bias = din("x1_bias", [128, 6], F32)      # x16
    qkv_bias = din("qkv_bias", [128, 6], F32)
    kv_bias = din("kv_bias", [128, 3], F32)      # x16, plain layout
    kvdw_bias = din("kvdw_bias", [128, 3], F32)  # plain layout
    newv_bias = din("newv_bias", [128, 2], F32)
    mask_rc = din("mask_rc", [1, (R + 2) * WP], BF16)  # 1/16 at valid, 0 at pad

    v_out = dout("v_out", [192, R * W], BF16)
    gramT_out = dout("gramT_out", [256, 192])    # rows = qcol space (pb*128+part)
    kss_out = dout("kss_out", [1, 384])          # [sum (KSQS*k0)^2 | sum NKS*k0]
    qstats_out = dout("qstats_out", [128, 2, 2])
    vstats_out = dout("vstats_out", [128, 2, 2])

    # C-phase DoubleRow k-tile pair coordinates in the [128, 3, MC2] msl tile
    # flat free space: region*MC2 + off.  Region 0 = lo channels, 1 = d1
    # (hi | hi<<WP), 2 = d2 (hi | hi<<2).
    def lo(dy, dx):
        return (1 + dy) * WP + dx + 1

    def d1(off):
        return MC2 + off

    def d2(off):
        return 2 * MC2 + off

    # 14 k-tiles -> 7 pairs; weight slot kv8[:, pair, j, :] must match.
    CPAIRS = [
        (lo(-1, -1), lo(-1, 0)), (lo(-1, 1), lo(0, -1)), (lo(0, 0), lo(0, 1)),
        (lo(1, -1), lo(1, 0)), (lo(1, 1), d1(0)), (d1(1), d1(2)),
        (d1(2 * WP + 1), d2(2 * WP)),
    ]

    with tile.TileContext(nc) as tc, ExitStack() as ctx:
        wpool = ctx.enter_context(tc.tile_pool(name="weights", bufs=1))
        xpool = ctx.enter_context(tc.tile_pool(name="xslab", bufs=2))
        bigpool = ctx.enter_context(tc.tile_pool(name="big", bufs=1))
        midpool = ctx.enter_context(tc.tile_pool(name="mid", bufs=2))
        smpool = ctx.enter_context(tc.tile_pool(name="small", bufs=4))
        statpool = ctx.enter_context(tc.tile_pool(name="stats", bufs=1))
        pspool = ctx.enter_context(tc.tile_pool(name="ps", bufs=5, space="PSUM"))
        pspers = ctx.enter_context(tc.tile_pool(name="pspers", bufs=1, space="PSUM"))

        def load1(ap_in, shape, dt=F8, eng=None):
            t = wpool.tile(shape, dt, tag=ap_in.tensor.name)
            (eng or nc.sync).dma_start(out=t[:ap_in.shape[0]], in_=ap_in[:])
            return t

        g = nc.gpsimd
        qw8_s = load1(qw8, [128, 2, 768])
        x1b_s = load1(x1_bias, [128, 6], F32)
        qdw8_s = load1(qdw8, [128, 6, 5, 2, 128], eng=g)
        kv8_s = load1(kv8, [128, 7, 2, 384], eng=g)
        kvdw8_s = load1(kvdw8, [128, 3, 5, 2, 128], eng=g)
        nkw8_s = load1(nkw8, [128, 2, 2, 192], F8, eng=g)
        nvw_s = load1(newv_wT, [128, 4, 192], BF16, eng=g)
        id_s = load1(ident, [128, 128], BF16, eng=g)
        qkvb_s = load1(qkv_bias, [128, 6], F32, eng=g)
        kvb_s = load1(kv_bias, [128, 3], F32, eng=g)
        dwb_s = load1(kvdw_bias, [128, 3], F32, eng=g)
        nvb_s = load1(newv_bias, [128, 2], F32, eng=g)

        # persistent accumulators: one PSUM bank each (zero-region granularity)
        # gramB is 384 wide: cols 192:384 see the ksq half of the moving data;
        # its ONE_ROWS rows (ones in the stationary) yield ksum/kss.
        gramA = pspers.tile([128, 192], F32)
        gramB = pspers.tile([128, 384], F32)

        qstats = statpool.tile([128, 2, NSLAB * (NS // 512), 6], F32)
        vstats = statpool.tile([128, 2, NSLAB * (NS // 512), 6], F32)
        # double-buffered chunk-pair staging for gram/kss (dims: pairbuf, slot).
        # inner dim padded to 400 so the DR pair stride is NOT contiguous with
        # the column dim (contiguous dims get re-flattened in lowering, which
        # breaks the pair interpretation of the moving AP).
        kq8 = statpool.tile([128, 2, 2, 400], F8)    # [0:192]=NKS*k0, [192:384]=ksq
        qT8 = statpool.tile([128, 2, 2, 256], F8)

        n128_total = NSLAB * N128
        CT = [(c0, min(512, XCOLS - c0)) for c0 in range(0, XCOLS, 512)]

        for s in range(NSLAB):
            xsl = xpool.tile([128, 2, XCOLS], F8, tag="xsl")
            msl = xpool.tile([128, 3, MC2], F8, tag="msl")
            off = s * S * WP
            nc.sync.dma_start(out=xsl[:], in_=x8[:, :, off:off + XCOLS])
            nc.sync.dma_start(out=msl[:, 0, 1:1 + MCOLS], in_=xm_lo[:, off:off + MCOLS])
            nc.sync.dma_start(out=msl[:, 1, 1:1 + MCOLS], in_=xm_d1[:, off:off + MCOLS])
            nc.sync.dma_start(out=msl[:, 2, 1:1 + MCOLS], in_=xm_d2[:, off:off + MCOLS])
            for r in range(3):
                nc.vector.memset(msl[:, r, 0:1], 0.0)
                nc.vector.memset(msl[:, r, MC2 - 1:MC2], 0.0)
            mtile = xpool.tile([128, XCOLS], BF16, tag="mtile")
            nc.sync.dma_start(out=mtile[:], in_=_bcast(mask_rc[0:1, off:off + XCOLS], 128))

            # ---- Phase A: x1 = 1x1(x); one DR matmul per (pb, col-tile)
            x1 = bigpool.tile([128, 6, XCOLS], F8, tag="x1")
            for pb in range(6):
                for c0, cs in CT:
                    ps = pspool.tile([128, 512], F32, tag="ps", name=f"psA{s}_{pb}_{c0}")
                    nc.tensor.matmul(ps[:, :cs], qw8_s[:, :, 128 * pb:128 * pb + 128],
                                     xsl[:, :, c0:c0 + cs], start=True, stop=True,
                                     perf_mode=DR)
                    nc.vector.scalar_tensor_tensor(
                        out=x1[:, pb, c0:c0 + cs], in0=ps[:, :cs],
                        scalar=x1b_s[:, pb:pb + 1], in1=mtile[:, c0:c0 + cs],
                        op0=mybir.AluOpType.add, op1=mybir.AluOpType.mult)

            # ---- Phase C: kv1 = 3x3(xm); 7 DR pairs per (pb, col-tile)
            kv1 = bigpool.tile([128, 3, XCOLS], F8, tag="kv1")
            for pb in range(3):
                for cset in (CT[:3], CT[3:]):
                    pss = [pspool.tile([128, 512], F32, tag="ps",
                                       name=f"psC{s}_{pb}_{c0}")
                           for (c0, cs) in cset]
                    for pj, (ca, cb) in enumerate(CPAIRS):
                        lhsT = kv8_s[:, pj, :, 128 * pb:128 * pb + 128]
                        for ti, (c0, cs) in enumerate(cset):
                            rhs = _pair_ap(msl, ca + c0, cb - ca, [[1, cs]])
                            nc.tensor.matmul(pss[ti][:, :cs], lhsT, rhs,
                                             start=(pj == 0), stop=(pj == 6),
                                             perf_mode=DR)
                    for ti, (c0, cs) in enumerate(cset):
                        nc.vector.scalar_tensor_tensor(
                            out=kv1[:, pb, c0:c0 + cs], in0=pss[ti][:, :cs],
                            scalar=kvb_s[:, pb:pb + 1], in1=mtile[:, c0:c0 + cs],
                            op0=mybir.AluOpType.add, op1=mybir.AluOpType.mult)

            # ---- Phase B: qkv = qdw(x1); 2-row groups (N=258), 5 DR tap-pairs
            # per group chained in PSUM; 4-group sets amortize weight loads.
            # route: pb -> list of (dst_tile, slot, part_lo, part_hi)
            def dwconv(src, wsrc, route, npb, bias_s, tag):
                for pb in range(npb):
                    for st in range(2):
                        gset = list(range(st * 4, st * 4 + 4))
                        pss = {gi: pspool.tile([128, 258], F32, tag="ps",
                                               name=f"ps{tag}{s}_{pb}_{gi}")
                               for gi in gset}
                        for pj, (ta, tb) in enumerate(BD_PAIRS):
                            oa = TAP_OFF[ta]
                            delta = (TAP_OFF[tb] - oa) if tb is not None else 1
                            lhsT = wsrc[:, pb, pj, :, :]
                            for gi in gset:
                                base = pb * XCOLS + (2 * gi + 1) * WP + 1 + oa
                                rhs = _pair_ap(src, base, delta, [[1, 258]])
                                nc.tensor.matmul(pss[gi][:, :], lhsT, rhs,
                                                 start=(pj == 0), stop=(pj == 4),
                                                 perf_mode=DR)
                        for gi in gset:
                            ps = pss[gi]
                            in_ap = bass.AP(tensor=ps.tensor, offset=ps.offset,
                                            ap=[list(ps.ap[0]), [WP, 2], [1, 128]])
                            for (dst, slot, plo, phi, chunked) in route(pb):
                                if chunked:
                                    out_ap = dst[plo:phi, 2 * gi:2 * gi + 2, slot, :]
                                else:
                                    out_ap = dst[plo:phi, slot,
                                                 2 * gi * 128:(2 * gi + 2) * 128]
                                nc.scalar.activation(
                                    out=out_ap,
                                    in_=in_ap[plo:phi],
                                    func=mybir.ActivationFunctionType.Identity,
                                    bias=bias_s[plo:phi, pb:pb + 1], scale=1.0 / WS)

            # k8/dwk8 are chunk-major [128, N128, 2, 128] so the kps DR
            # stationary pair is contiguous (pair stride 128 — ISA requires
            # small pair strides for dual-fp8 ldweights).
            q_t = bigpool.tile([128, 2, NS], BF16, tag="q_t")
            k8 = bigpool.tile([128, N128, 2, 128], F8, tag="k8")
            vq = bigpool.tile([128, 2, NS], BF16, tag="vq")

            def qkv_route(pb):
                dst = (q_t, k8, vq)[pb // 2]
                return [(dst, pb % 2, 0, 128, pb // 2 == 1)]

            dwconv(x1, qdw8_s, qkv_route, 6, qkvb_s, "B")

            # ---- Phase D: dw = kvdw(kv1); same structure, 3 pblocks
            dwk8 = bigpool.tile([128, N128, 2, 128], F8, tag="dwk8")
            dwv = bigpool.tile([128, 2, NS], BF16, tag="dwv")
            nc.vector.memset(dwk8[64:128, :, 1, :], 0.0)

            def dw_route(pb):
                if pb == 0:
                    return [(dwk8, 0, 0, 128, True)]
                if pb == 1:
                    return [(dwk8, 1, 0, 64, True), (dwv, 0, 64, 128, False)]
                return [(dwv, 1, 0, 128, False)]

            dwconv(kv1, kvdw8_s, dw_route, 3, dwb_s, "D")

            # ---- Phase E: v = newv(v_cc) + stats + dma out (bf16)
            vt = midpool.tile([128, 2, NS], BF16, tag="vt")
            for mb in range(2):
                msz = 128 if mb == 0 else 64
                pss = [pspool.tile([128, 512], F32, tag="ps", name=f"psE{s}_{mb}_{ic}")
                       for ic in range(NS // 512)]
                for j, (src, pb, base, sz) in enumerate(VCC):
                    data = (vq if src == "vq" else dwv)
                    lhsT = nvw_s[base:base + sz, j, mb * 128:mb * 128 + msz]
                    for ic in range(NS // 512):
                        rhs = data[base:base + sz, pb, ic * 512:(ic + 1) * 512]
                        nc.tensor.matmul(pss[ic][:msz, :], lhsT, rhs, start=(j == 0),
                                         stop=(j == 3),
                                         tile_position=(base, 0) if base else None)
                for ic in range(NS // 512):
                    nc.scalar.activation(out=vt[:msz, mb, ic * 512:(ic + 1) * 512],
                                         in_=pss[ic][:msz, :],
                                         func=mybir.ActivationFunctionType.Identity,
                                         bias=nvb_s[:msz, mb:mb + 1], scale=1.0)
            nc.sync.dma_start(out=v_out[0:128, s * NS:(s + 1) * NS], in_=vt[:, 0, :])
            nc.sync.dma_start(out=v_out[128:192, s * NS:(s + 1) * NS], in_=vt[:64, 1, :])
            for sub in range(NS // 512):
                si = s * (NS // 512) + sub
                sl = slice(sub * 512, (sub + 1) * 512)
                nc.vector.bn_stats(out=vstats[:, 0, si, :], in_=vt[:, 0, sl])
                nc.vector.bn_stats(out=vstats[:64, 1, si, :], in_=vt[:64, 1, sl])
                nc.vector.bn_stats(out=qstats[:, 0, si, :], in_=q_t[:, 0, sl])
                nc.vector.bn_stats(out=qstats[:, 1, si, :], in_=q_t[:, 1, sl])

            # ---- Phase F: per 128-n chunk: kT (newk, fp8 DR), qT (transpose),
            # then per chunk-pair: gram (fp8 DR) + kss.
            for ic in range(N128):
                c0 = ic * 128
                gidx = s * N128 + ic
                slot = ic % 2
                pbuf = (ic // 2) % 2
                kps = pspool.tile([128, 192], F32, tag="ps", name=f"kps{s}_{ic}")
                nc.tensor.matmul(kps[:, :], k8[:, ic, :, :], nkw8_s[:, 0, :, :],
                                 start=True, stop=False, perf_mode=DR)
                nc.tensor.matmul(kps[:, :], dwk8[:, ic, :, :], nkw8_s[:, 1, :, :],
                                 start=False, stop=True, perf_mode=DR)
                nc.scalar.copy(out=kq8[:, pbuf, slot, 0:192], in_=kps[:, :])
                nc.scalar.activation(out=kq8[:, pbuf, slot, 192:384], in_=kps[:, :],
                                     func=mybir.ActivationFunctionType.Square,
                                     scale=KSQS / NKS)
                qps = pspool.tile([128, 256], BF16, tag="ps", name=f"qps{s}_{ic}")
                nc.tensor.transpose(qps[:, 0:128], q_t[:, 0, c0:c0 + 128], id_s[:, :])
                nc.tensor.transpose(qps[:, 128:256], q_t[:, 1, c0:c0 + 128], id_s[:, :])
                nc.scalar.mul(out=qT8[:, pbuf, slot, :], in_=qps[:, :], mul=QTS)
                if slot == 1:
                    gp = gidx // 2
                    first, last = gp == 0, gp == n128_total // 2 - 1
                    nc.tensor.matmul(gramA[:, :], qT8[:, pbuf, :, 0:128],
                                     kq8[:, pbuf, :, 0:192], start=first, stop=last,
                                     perf_mode=DR)
                    nc.tensor.matmul(gramB[:, :], qT8[:, pbuf, :, 128:256],
                                     kq8[:, pbuf, :, 0:384], start=first, stop=last,
                                     perf_mode=DR)

        qmv = statpool.tile([128, 2, 2], F32)
        vmv = statpool.tile([128, 2, 2], F32)
        nc.vector.memset(qmv[:], 0.0)
        nc.vector.memset(vmv[:], 0.0)
        nc.vector.bn_aggr(out=qmv[:, 0, :], in_=qstats[:, 0, :, :])
        nc.vector.bn_aggr(out=qmv[:, 1, :], in_=qstats[:, 1, :, :])
        nc.vector.bn_aggr(out=vmv[:, 0, :], in_=vstats[:, 0, :, :])
        nc.vector.bn_aggr(out=vmv[:64, 1, :], in_=vstats[:64, 1, :, :])
        nc.sync.dma_start(out=qstats_out[:], in_=qmv[:])
        nc.sync.dma_start(out=vstats_out[:], in_=vmv[:])
        gA = statpool.tile([128, 192], F32)
        gB = statpool.tile([128, 192], F32)
        kssb = statpool.tile([128, 384], F32)
        nc.scalar.copy(out=gA[:], in_=gramA[:])
        nc.scalar.copy(out=gB[:], in_=gramB[:, 0:192])
        r0 = ONE_ROWS[0]
        nc.scalar.copy(out=kssb[r0:r0 + 1, :], in_=gramB[r0:r0 + 1, 0:384])
        nc.sync.dma_start(out=gramT_out[0:128, :], in_=gA[:])
        nc.sync.dma_start(out=gramT_out[128:256, :], in_=gB[:])
        nc.sync.dma_start(out=kss_out[:], in_=kssb[r0:r0 + 1, :])

    nc.compile()
    return nc


def build_l2(R=64, W=128):
    NS = R * W
    nc = bacc.Bacc("TRN2", target_bir_lowering=False, debug=False, num_devices=8)
    v_in = nc.dram_tensor("v_in", [192, NS], BF16, kind="ExternalInput").ap()
    awT = nc.dram_tensor("awT", [128, 2, 192], BF16, kind="ExternalInput").ap()
    pbias = nc.dram_tensor("pbias", [128, 2], F32, kind="ExternalInput").ap()
    out = nc.dram_tensor("out", [192, NS], BF16, kind="ExternalOutput").ap()

    with tile.TileContext(nc) as tc, ExitStack() as ctx:
        wpool = ctx.enter_context(tc.tile_pool(name="w", bufs=1))
        vpool = ctx.enter_context(tc.tile_pool(name="v", bufs=4))
        opool = ctx.enter_context(tc.tile_pool(name="o", bufs=4))
        pspool = ctx.enter_context(tc.tile_pool(name="ps", bufs=4, space="PSUM"))

        aw = wpool.tile([128, 2, 192], BF16)
        nc.sync.dma_start(out=aw[:], in_=awT[:])
        pb = wpool.tile([128, 2], F32)
        nc.sync.dma_start(out=pb[:], in_=pbias[:])

        for c0 in range(0, NS, 512):
            vt = vpool.tile([128, 2, 512], BF16, tag="vt")
            nc.sync.dma_start(out=vt[:, 0, :], in_=v_in[0:128, c0:c0 + 512])
            nc.sync.dma_start(out=vt[:64, 1, :], in_=v_in[128:192, c0:c0 + 512])
            for mb in range(2):
                msz = 128 if mb == 0 else 64
                ps = pspool.tile([128, 512], F32, tag="ps")
                nc.tensor.matmul(ps[:msz, :], aw[:, 0, mb * 128:mb * 128 + msz],
                                 vt[:, 0, :], start=True, stop=False)
                nc.tensor.matmul(ps[:msz, :], aw[:64, 1, mb * 128:mb * 128 + msz],
                                 vt[:64, 1, :], start=False, stop=True)
                ot = opool.tile([128, 512], BF16, tag="ot")
                nc.scalar.activation(out=ot[:msz, :], in_=ps[:msz, :],
                                     func=mybir.ActivationFunctionType.Identity,
                                     bias=pb[:msz, mb:mb + 1], scale=1.0)
                nc.sync.dma_start(out=out[mb * 128:mb * 128 + msz, c0:c0 + 512],
                                  in_=ot[:msz, :])
    nc.compile()
    return nc


# ---------------- host-side prep ----------------

def prep_weights(w):
    """w: dict of reference weights (numpy f32). Returns dict of L1 input arrays."""
    out = {}
    qw = w["q_w"][:, :, 0, 0]          # (576, 192)
    qwT = np.zeros((128, 2, 768), np.float32)
    for (pb, h, x1b, qb, ch0, nch) in qkv_halves():
        win = 128 * pb + 64 * h
        qwT[0:128, 0, win:win + nch] = qw.T[0:128, ch0:ch0 + nch]
        qwT[0:64, 1, win:win + nch] = qw.T[128:192, ch0:ch0 + nch]
    out["qw8"] = (qwT * WS).astype(f8)

    # qdw: grouped 3x3, (tap, pb) 128x128 block; repack into DR tap pairs
    qdw = w["qdw_w"]                   # (576, 3, 3, 3)
    qdwT = np.zeros((9, 6, 128, 128), np.float32)   # (tap_sorted, pb, row, col)
    for ti, (dy, dx) in enumerate(TAPS):
        for (pb, h, x1b, qb, ch0, nch) in qkv_halves():
            for gl in range(nch // 3):
                for i in range(3):
                    for j in range(3):
                        qdwT[ti, pb, x1b + 3 * gl + i, qb + 3 * gl + j] = \
                            qdw[ch0 + 3 * gl + j, i, dy + 1, dx + 1]
    qdw8 = np.zeros((128, 6, 5, 2, 128), np.float32)
    for pj, (ta, tb) in enumerate(BD_PAIRS):
        for pb in range(6):
            qdw8[:, pb, pj, 0, :] = qdwT[ta, pb]
            if tb is not None:
                qdw8[:, pb, pj, 1, :] = qdwT[tb, pb]
    out["qdw8"] = (qdw8 * WS).astype(f8)

    # kv conv: 14 k-tiles -> 7 DR pairs (order must match CPAIRS in build_l1)
    kvw = w["kv_w"]                    # (384, 192, 3, 3)
    tl = lambda dy, dx: kvw[:, 0:128, dy + 1, dx + 1].T       # (128, 384) lo
    th = lambda dy, dx: kvw[:, 128:192, dy + 1, dx + 1].T     # (64, 384) hi
    ktiles = []
    for dy, dx in TAPS:   # 9 lo tiles (sorted tap order == TAPS order)
        ktiles.append(("lo", (dy, dx)))
    ktiles.append(("d1", -1)); ktiles.append(("d1", 0)); ktiles.append(("d1", 1))
    ktiles.append(("single", None)); ktiles.append(("d2", None))
    kv8 = np.zeros((128, 7, 2, 384), np.float32)
    for pj in range(7):
        for j in range(2):
            kind, arg = ktiles[2 * pj + j]
            blk = np.zeros((128, 384), np.float32)
            if kind == "lo":
                blk[0:128] = tl(*arg)
            elif kind == "d1":
                blk[0:64] = th(-1, arg)
                blk[64:128] = th(0, arg)
            elif kind == "d2":
                blk[0:64] = th(1, -1)
                blk[64:128] = th(1, 1)
            elif kind == "single":
                blk[0:64] = th(1, 0)
            kv8[:, pj, j, :] = blk
    out["kv8"] = (kv8 * WS).astype(f8)

    # kvdw depthwise: plain layout, diag blocks per pblock, DR tap pairs
    kvdw = w["kvdw_w"][:, 0]           # (384, 3, 3)
    kvdw8 = np.zeros((128, 3, 5, 2, 128), np.float32)
    for pj, (ta, tb) in enumerate(BD_PAIRS):
        for pb in range(3):
            dya, dxa = TAPS[ta]
            kvdw8[:, pb, pj, 0, :] = np.diag(kvdw[128 * pb:128 * pb + 128, dya + 1, dxa + 1])
            if tb is not None:
                dyb, dxb = TAPS[tb]
                kvdw8[:, pb, pj, 1, :] = np.diag(kvdw[128 * pb:128 * pb + 128, dyb + 1, dxb + 1])
    out["kvdw8"] = (kvdw8 * WS).astype(f8)

    def dw_channel(pb, p):
        return 128 * pb + p   # plain layout

    # newk (no bias on device; kT = NKS * k0): contraction sources
    # pass0 = (k8 slot0 = qkv pb2, k8 slot1 = qkv pb3)
    # pass1 = (dwk8 slot0 = dw pb0, dwk8 slot1 = dw pb1 parts 0..63)
    KCC = [("qkv", 2, 0, 128), ("qkv", 3, 0, 128), ("dw", 0, 0, 128), ("dw", 1, 0, 64)]
    nk = w["newk_w"][:, :, 0, 0]       # (192, 384): in = [k(192) | k_mask(192)]
    nkm = np.zeros((128, 4, 192), np.float32)
    for j, (src, pb, base, sz) in enumerate(KCC):
        for p in range(base, base + sz):
            if src == "qkv":
                ch = qkv_channel_at(pb, p)
                if ch is not None:
                    nkm[p, j, :] = nk[:, ch - 192]      # k part: qkv ch 192-383
            else:
                ch = dw_channel(pb, p)
                if ch < 192:
                    nkm[p, j, :] = nk[:, 192 + ch]      # k_mask: dw ch 0-191
    out["nkw8"] = (nkm.reshape(128, 2, 2, 192) * NKS).astype(f8)

    nv = w["newv_w"][:, :, 0, 0]       # (192, 384): in = [v(192) | v_mask(192)]
    nvT = np.zeros((128, 4, 192), np.float32)
    VCC_P = [("qkv", 4, 0, 128), ("qkv", 5, 0, 128), ("dw", 1, 64, 64), ("dw", 2, 0, 128)]
    for j, (src, pb, base, sz) in enumerate(VCC_P):
        for p in range(base, base + sz):
            if src == "qkv":
                ch = qkv_channel_at(pb, p)
                if ch is not None:
                    nvT[p, j, :] = nv[:, ch - 384]      # v part: qkv ch 384-575
            else:
                ch = dw_channel(pb, p)
                if ch >= 192:
                    nvT[p, j, :] = nv[:, ch]            # v_mask: dw ch 192-383
    out["newv_wT"] = nvT.astype(bf16)

    out["ident"] = np.eye(128, dtype=bf16)

    x1b = np.zeros((128, 6), np.float32)
    qkvb = np.zeros((128, 6), np.float32)
    for (pb, h, x1b_base, qb, ch0, nch) in qkv_halves():
        x1b[x1b_base:x1b_base + nch, pb] = w["q_b"][ch0:ch0 + nch]
        qkvb[qb:qb + nch, pb] = w["qdw_b"][ch0:ch0 + nch]
    # hijacked ones rows: junk q pb1 partitions become the exact constant
    # 1/QTS, which the qT8 scale turns into 1.0 -> gramB rows = ksum/kss
    for r in ONE_ROWS:
        qkvb[r, 1] = 1.0 / QTS
    out["x1_bias"] = x1b * WS
    out["qkv_bias"] = qkvb

    kvb = np.zeros((128, 3), np.float32)
    dwb = np.zeros((128, 3), np.float32)
    for pb in range(3):
        kvb[:, pb] = w["kv_b"][128 * pb:128 * pb + 128]
        dwb[:, pb] = w["kvdw_b"][128 * pb:128 * pb + 128]
    out["kv_bias"] = kvb * WS
    out["kvdw_bias"] = dwb
    nvb = np.zeros((128, 2), np.float32)
    nvb[:, 0] = w["newv_b"][0:128]
    nvb[0:64, 1] = w["newv_b"][128:192]
    out["newv_bias"] = nvb
    return out


def prep_masks(R, H, half):
    m = np.zeros((R + 2, WP), np.float32)
    for r in range(R + 2):
        g = half * R + (r - 1)
        if 0 <= g < H:
            m[r, 1:129] = 1.0 / WS
    return m.reshape(1, -1)


def prep_core(x, xm, b, half, R, H):
    xp = np.zeros((192, R + 2, WP), np.float32)
    mp = np.zeros((192, R + 4, WP), np.float32)
    for r in range(R + 2):
        g = half * R + (r - 1)
        if 0 <= g < H:
            xp[:, r, 1:129] = x[b, :, g, :]
    for r in range(R + 4):
        g = half * R + (r - 2)
        if 0 <= g < H:
            mp[:, r, 1:129] = xm[b, :, g, :]
    xp = xp.reshape(192, -1)
    x8 = np.zeros((128, 2, xp.shape[1]), np.float32)
    x8[:, 0, :] = xp[0:128]
    x8[0:64, 1, :] = xp[128:192]
    x8[64:128, 1, :] = xp[128:192]   # dup (weights zero) to avoid NaN garbage
    mp = mp.reshape(192, -1)
    L = mp.shape[1]
    hi = mp[128:192]
    d1 = np.zeros((128, L), np.float32)
    d2 = np.zeros((128, L), np.float32)
    d1[0:64] = hi
    d1[64:128, :L - WP] = hi[:, WP:]
    d2[0:64] = hi
    d2[64:128, :L - 2] = hi[:, 2:]
    return {
        "x8": x8.astype(f8),
        "xm_lo": mp[0:128].astype(f8),
        "xm_d1": d1.astype(f8), "xm_d2": d2.astype(f8),
        "mask_rc": prep_masks(R, H, half).astype(bf16),
    }


# ---------------- host glue (unchanged semantics) ----------------

def _q_maps():
    part = np.zeros(192, np.int64)
    pblk = np.zeros(192, np.int64)
    for (pb, h, x1b, qb, ch0, nch) in qkv_halves():
        if pb >= 2:
            continue
        for i in range(nch):
            pblk[ch0 + i] = pb
            part[ch0 + i] = qb + i
    return pblk, part


def _ss_from_qstats(stats, n_half):
    pblk, part = _q_maps()
    mv = stats.astype(np.float64)
    return (mv[part, pblk, 1] + mv[part, pblk, 0] ** 2) * n_half


def _sum_from_qstats(stats, n_half):
    pblk, part = _q_maps()
    mv = stats.astype(np.float64)
    return mv[part, pblk, 0] * n_half


def _ss_from_vstats(stats, n_half):
    ss = np.zeros(192, np.float64)
    mv = stats.astype(np.float64)
    ss[0:128] = (mv[0:128, 0, 1] + mv[0:128, 0, 0] ** 2) * n_half
    ss[128:192] = (mv[0:64, 1, 1] + mv[0:64, 1, 0] ** 2) * n_half
    return ss


def glue(res0, res1, temperature, proj_w, proj_b, n_half):
    """Combine two half-core L1 results -> L2 inputs (awT, pbias)."""
    GT = res0["gramT_out"].astype(np.float64) + res1["gramT_out"].astype(np.float64)
    pblk, part = _q_maps()
    qrow = pblk * 128 + part
    # GT rows are qT cols (pb*128+part); cols are newk out-ch d. Stored values
    # are sum_n (QTS*q) * (NKS*k0).
    Gq = GT[qrow, :] / (QTS * NKS)             # (c, d): sum_n q[c,n] k0[d,n]
    qss = _ss_from_qstats(res0["qstats_out"], n_half) + _ss_from_qstats(res1["qstats_out"], n_half)
    qsum = _sum_from_qstats(res0["qstats_out"], n_half) + _sum_from_qstats(res1["qstats_out"], n_half)
    vss = _ss_from_vstats(res0["vstats_out"], n_half) + _ss_from_vstats(res1["vstats_out"], n_half)
    kss_raw = (res0["kss_out"].astype(np.float64) + res1["kss_out"].astype(np.float64))[0]
    k0sum = kss_raw[0:192] / NKS               # sum_n k0 (ones x kT cols)
    k0ss = kss_raw[192:384] / (KSQS * KSQS)    # sum_n k0^2 (ones x ksq cols)
    return Gq, qss, qsum, vss, k0ss, k0sum


def glue_full(res0, res1, temperature, newk_b, proj_w, proj_b, n_half):
    Gq, qss, qsum, vss, k0ss, k0sum = glue(res0, res1, temperature, proj_w, proj_b, n_half)
    b = newk_b.astype(np.float64)              # (192,)
    # k = k0 + b: gram/kss bias corrections (sums already cover both halves,
    # total n = 2 * n_half)
    G = Gq + qsum[:, None] * b[None, :]        # (c, d): sum_n q k
    kss = k0ss + 2 * b * k0sum + (2 * n_half) * b * b
    qn = np.maximum(np.sqrt(qss), 1e-12)
    kn = np.maximum(np.sqrt(kss), 1e-12)
    vn = np.maximum(np.sqrt(vss), 1e-12)
    A = G / (qn[:, None] * kn[None, :])        # (c, d)
    M = np.zeros((192, 192), np.float64)
    t = np.asarray(temperature).reshape(-1)
    for h in range(8):
        sl = slice(24 * h, 24 * h + 24)
        a = A[sl, sl] * t[h]
        a = a - a.max(axis=-1, keepdims=True)
        e = np.exp(a)
        sm = e / e.sum(axis=-1, keepdims=True)
        M[sl, sl] = sm / vn[None, sl]
    At = proj_w[:, :, 0, 0].astype(np.float64) @ M   # (out-ch o, d)
    awT = np.zeros((128, 2, 192), np.float32)
    awT[:, 0, :] = At.T[0:128]
    awT[0:64, 1, :] = At.T[128:192]
    pbias = np.zeros((128, 2), np.float32)
    pbias[:, 0] = proj_b[0:128]
    pbias[0:64, 1] = proj_b[128:192]
    return {"awT": awT.astype(bf16), "pbias": pbias}


# ---------------- driver: kernel(**inputs) ----------------
from concourse.bass_utils import run_bass_kernel_spmd

R_FULL, H_FULL, B_FULL = 64, 128, 4
_NC1 = None
_NC2 = None


def _get_progs():
    global _NC1, _NC2
    if _NC1 is None:
        _NC1 = build_l1(R=R_FULL, S=16)
        _NC2 = build_l2(R=R_FULL)
    return _NC1, _NC2


def kernel(**inputs):
    inputs = {k: np.asarray(v) for k, v in inputs.items()}
    x, xm = inputs["x"], inputs["x_mask"]
    nc1, nc2 = _get_progs()
    wprep = prep_weights(inputs)
    in_maps = []
    for core in range(8):
        b, half = core // 2, core % 2
        m = dict(wprep)
        m.update(prep_core(x, xm, b, half, R_FULL, H_FULL))
        in_maps.append(m)
    res1 = run_bass_kernel_spmd(nc1, in_maps, list(range(8))).results

    n_half = R_FULL * 128
    in_maps2 = []
    for core in range(8):
        b, half = core // 2, core % 2
        if half == 0:
            l2c = glue_full(res1[2 * b], res1[2 * b + 1], inputs["temperature"],
                            inputs["newk_b"], inputs["proj_w"], inputs["proj_b"],
                            n_half)
        m = dict(l2c)
        m["v_in"] = res1[core]["v_out"]
        in_maps2.append(m)
    res2 = run_bass_kernel_spmd(nc2, in_maps2, list(range(8))).results

    out = np.empty((B_FULL, 192, H_FULL, 128), np.float32)
    for core in range(8):
        b, half = core // 2, core % 2
        out[b, :, half * R_FULL:(half + 1) * R_FULL, :] = \
            res2[core]["out"].reshape(192, R_FULL, 128).astype(np.float32)
    return out


# revision 23
# speedup vs baseline: 1.0119x; 1.0061x over previous
"""Trainium2 Bass kernel for nn_Cross_Attention — fp8 DoubleRow rewrite.

L1: convs + gram partials with fp8 DoubleRow matmuls (phases A-D), fp8 F-phase
(newk/gram/kss via DR), bf16 E. L2: attn-apply with chunked v DMA. Host glue
between (softmax etc.). Sharding: 4 samples x 2 row-halves across 8 cores.
"""
import sys
sys.path.insert(0, "/opt/trn_rl_repo")
import numpy as np
import ml_dtypes

import concourse.bass as bass
import concourse.tile as tile
from concourse import bacc, mybir
from contextlib import ExitStack

BF16 = mybir.dt.bfloat16
F8 = mybir.dt.float8e4
F32 = mybir.dt.float32
bf16 = ml_dtypes.bfloat16
f8 = ml_dtypes.float8_e4m3
DR = mybir.MatmulPerfMode.DoubleRow

WS = 16.0          # fp8 weight pre-scale
WP = 130           # padded row width
NKS = 32.0         # newk weight pre-scale (kT stored as NKS*k0)
QTS = 16.0         # qT pre-scale (qT stored as QTS*q)
KSQS = 64.0        # ksq stored as (KSQS*k0)**2
ONE_ROWS = (32, 33)  # junk q pb1 partitions hijacked as 1.0 cols in qT8 (ksum/kss);
                     # must be 32-aligned (engine partition-base alignment)

TAPS = [(dy, dx) for dy in (-1, 0, 1) for dx in (-1, 0, 1)]
# taps sorted by flat offset dy*WP+dx (they already are, given WP>2)
TAP_OFF = [dy * WP + dx for dy, dx in TAPS]
# DoubleRow tap pairs for B/D: (idx_a, idx_b). The odd tap rides first with a
# zero-weight second tile at stride +1 (always in-bounds since tap 0 has the
# smallest offset).
BD_PAIRS = [(0, None), (1, 2), (3, 4), (5, 6), (7, 8)]


def qkv_halves():
    """Per (pb, half): (x1_base, qkv_base, ch0, nch).  ch0 = qkv-global channel."""
    out = []
    for pb in range(6):
        P, odd = pb // 2, pb % 2
        for h in (0, 1):
            nch = 3 if (odd and h == 1) else 63
            ch0 = 3 * (64 * P + 42 * odd + 21 * h)
            x1b = 64 * h
            qb = 64 * h if not odd else 64 * (1 - h)
            out.append((pb, h, x1b, qb, ch0, nch))
    return out


def qkv_channel_at(pb, p):
    """qkv-global channel stored at partition p of qkv pblock pb, or None."""
    for (pb2, h, x1b, qb, ch0, nch) in qkv_halves():
        if pb2 == pb and qb <= p < qb + nch:
            return ch0 + (p - qb)
    return None


# newk/newv input chunks with PLAIN dw layout (dw pb_i = kv ch 128i..128i+127):
# k = qkv ch 192-383 (pb 2,3); k_mask = kv ch 0-191 = dw pb0 + dw pb1[0:64]
# v = qkv ch 384-575 (pb 4,5); v_mask = kv ch 192-383 = dw pb1[64:128] + dw pb2
# New tile routing: q_t = qkv pb0,1 (bf16); k8 = qkv pb2,3 (f8);
# vq = qkv pb4,5 (bf16); dwk8 = dw pb0 + dw pb1[0:64] (f8, slot1 parts 64-127
# zeroed); dwv = dw pb1[64:128] + dw pb2 (bf16, slot0 parts 64-127 used).
VCC = [("vq", 0, 0, 128), ("vq", 1, 0, 128), ("dwv", 0, 64, 64), ("dwv", 1, 0, 128)]


def _bcast(ap, p):
    return bass.AP(tensor=ap.tensor, offset=ap.offset, ap=[[0, p]] + list(ap.ap[1:]))


def _pair_ap(t, off, delta, dims):
    """AP over tile t at flat free-offset `off`: [partitions, 2 (stride delta), *dims]."""
    return bass.AP(tensor=t.tensor, offset=t.offset + off,
                   ap=[list(t.ap[0]), [delta, 2]] + [list(d) for d in dims])


def build_l1(R=64, W=128, S=16):
    assert W == 128 and R % S == 0 and S % 4 == 0
    NSLAB = R // S
    NS = S * W
    XCOLS = (S + 2) * WP
    MCOLS = (S + 4) * WP
    MC2 = MCOLS + 2
    N128 = NS // 128

    nc = bacc.Bacc("TRN2", target_bir_lowering=False, debug=False, num_devices=8)

    def din(name, shape, dt=F8):
        return nc.dram_tensor(name, shape, dt, kind="ExternalInput").ap()

    def dout(name, shape, dt=F32):
        return nc.dram_tensor(name, shape, dt, kind="ExternalOutput").ap()

    x8 = din("x8", [128, 2, (R + 2) * WP])
    xm_lo = din("xm_lo", [128, (R + 4) * WP])
    xm_d1 = din("xm_d1", [128, (R + 4) * WP])
    xm_d2 = din("xm_d2", [128, (R + 4) * WP])
    qw8 = din("qw8", [128, 2, 768])
    qdw8 = din("qdw8", [128, 6, 5, 2, 128])
    kv8 = din("kv8", [128, 7, 2, 384])
    kvdw8 = din("kvdw8", [128, 3, 5, 2, 128])
    nkw8 = din("nkw8", [128, 2, 2, 192], F8)
    newv_wT = din("newv_wT", [128, 4, 192], BF16)
    ident = din("ident", [128, 128], BF16)
    x1_bias = din("x1_bias", [128, 6], F32)      # x16
    qkv_bias = din("qkv_bias", [128, 6], F32)
    kv_bias = din("kv_bias", [128, 3], F32)      # x16, plain layout
    kvdw_bias = din("kvdw_bias", [128, 3], F32)  # plain layout
    newv_bias = din("newv_bias", [128, 2], F32)
    mask_rc = din("mask_rc", [1, (R + 2) * WP], BF16)  # 1/16 at valid, 0 at pad

    v_out = dout("v_out", [192, R * W], BF16)
    gramT_out = dout("gramT_out", [256, 192])    # rows = qcol space (pb*128+part)
    kss_out = dout("kss_out", [1, 384])          # [sum (KSQS*k0)^2 | sum NKS*k0]
    qstats_out = dout("qstats_out", [128, 2, 2])
    vstats_out = dout("vstats_out", [128, 2, 2])

    # C-phase DoubleRow k-tile pair coordinates in the [128, 3, MC2] msl tile
    # flat free space: region*MC2 + off.  Region 0 = lo channels, 1 = d1
    # (hi | hi<<WP), 2 = d2 (hi | hi<<2).
    def lo(dy, dx):
        return (1 + dy) * WP + dx + 1

    def d1(off):
        return MC2 + off

    def d2(off):
        return 2 * MC2 + off

    # 14 k-tiles -> 7 pairs; weight slot kv8[:, pair, j, :] must match.
    CPAIRS = [
        (lo(-1, -1), lo(-1, 0)), (lo(-1, 1), lo(0, -1)), (lo(0, 0), lo(0, 1)),
        (lo(1, -1), lo(1, 0)), (lo(1, 1), d1(0)), (d1(1), d1(2)),
        (d1(2 * WP + 1), d2(2 * WP)),
    ]

    with tile.TileContext(nc) as tc, ExitStack() as ctx:
        wpool = ctx.enter_context(tc.tile_pool(name="weights", bufs=1))
        xpool = ctx.enter_context(tc.tile_pool(name="xslab", bufs=2))
        bigpool = ctx.enter_context(tc.tile_pool(name="big", bufs=1))
        midpool = ctx.enter_context(tc.tile_pool(name="mid", bufs=2))
        smpool = ctx.enter_context(tc.tile_pool(name="small", bufs=4))
        statpool = ctx.enter_context(tc.tile_pool(name="stats", bufs=1))
        pspool = ctx.enter_context(tc.tile_pool(name="ps", bufs=5, space="PSUM"))
        pspers = ctx.enter_context(tc.tile_pool(name="pspers", bufs=1, space="PSUM"))

        def load1(ap_in, shape, dt=F8, eng=None):
            t = wpool.tile(shape, dt, tag=ap_in.tensor.name)
            (eng or nc.sync).dma_start(out=t[:ap_in.shape[0]], in_=ap_in[:])
            return t

        g = nc.gpsimd
        qw8_s = load1(qw8, [128, 2, 768])
        x1b_s = load1(x1_bias, [128, 6], F32)
        qdw8_s = load1(qdw8, [128, 6, 5, 2, 128], eng=g)
        kv8_s = load1(kv8, [128, 7, 2, 384], eng=g)
        kvdw8_s = load1(kvdw8, [128, 3, 5, 2, 128], eng=g)
        nkw8_s = load1(nkw8, [128, 2, 2, 192], F8, eng=g)
        nvw_s = load1(newv_wT, [128, 4, 192], BF16, eng=g)
        id_s = load1(ident, [128, 128], BF16, eng=g)
        qkvb_s = load1(qkv_bias, [128, 6], F32, eng=g)
        kvb_s = load1(kv_bias, [128, 3], F32, eng=g)
        dwb_s = load1(kvdw_bias, [128, 3], F32, eng=g)
        nvb_s = load1(newv_bias, [128, 2], F32, eng=g)

        # persistent accumulators: one PSUM bank each (zero-region granularity)
        # gramB is 384 wide: cols 192:384 see the ksq half of the moving data;
        # its ONE_ROWS rows (ones in the stationary) yield ksum/kss.
        gramA = pspers.tile([128, 192], F32)
        gramB = pspers.tile([128, 384], F32)

        qstats = statpool.tile([128, 2, NSLAB * (NS // 512), 6], F32)
        vstats = statpool.tile([128, 2, NSLAB * (NS // 512), 6], F32)
        # double-buffered chunk-pair staging for gram/kss (dims: pairbuf, slot).
        # inner dim padded to 400 so the DR pair stride is NOT contiguous with
        # the column dim (contiguous dims get re-flattened in lowering, which
        # breaks the pair interpretation of the moving AP).
        kq8 = statpool.tile([128, 2, 2, 400], F8)    # [0:192]=NKS*k0, [192:384]=ksq
        qT8 = statpool.tile([128, 2, 2, 256], F8)

        n128_total = NSLAB * N128
        CT = [(c0, min(512, XCOLS - c0)) for c0 in range(0, XCOLS, 512)]

        for s in range(NSLAB):
            xsl = xpool.tile([128, 2, XCOLS], F8, tag="xsl")
            msl = xpool.tile([128, 3, MC2], F8, tag="msl")
            off = s * S * WP
            # spread the slab input loads over several DMA queues
            nc.sync.dma_start(out=xsl[:], in_=x8[:, :, off:off + XCOLS])
            nc.scalar.dma_start(out=msl[:, 0, 1:1 + MCOLS], in_=xm_lo[:, off:off + MCOLS])
            nc.gpsimd.dma_start(out=msl[:, 1, 1:1 + MCOLS], in_=xm_d1[:, off:off + MCOLS])
            nc.scalar.dma_start(out=msl[:, 2, 1:1 + MCOLS], in_=xm_d2[:, off:off + MCOLS])
            for r in range(3):
                nc.vector.memset(msl[:, r, 0:1], 0.0)
                nc.vector.memset(msl[:, r, MC2 - 1:MC2], 0.0)
            mtile = xpool.tile([128, XCOLS], BF16, tag="mtile")
            nc.sync.dma_start(out=mtile[:], in_=_bcast(mask_rc[0:1, off:off + XCOLS], 128))

            # ---- Phase A: x1 = 1x1(x); one DR matmul per (pb, col-tile)
            x1 = bigpool.tile([128, 6, XCOLS], F8, tag="x1")
            for pb in range(6):
                for c0, cs in CT:
                    ps = pspool.tile([128, 512], F32, tag="ps", name=f"psA{s}_{pb}_{c0}")
                    nc.tensor.matmul(ps[:, :cs], qw8_s[:, :, 128 * pb:128 * pb + 128],
                                     xsl[:, :, c0:c0 + cs], start=True, stop=True,
                                     perf_mode=DR)
                    nc.vector.scalar_tensor_tensor(
                        out=x1[:, pb, c0:c0 + cs], in0=ps[:, :cs],
                        scalar=x1b_s[:, pb:pb + 1], in1=mtile[:, c0:c0 + cs],
                        op0=mybir.AluOpType.add, op1=mybir.AluOpType.mult)

            # ---- Phase C: kv1 = 3x3(xm); 7 DR pairs per (pb, col-tile)
            kv1 = bigpool.tile([128, 3, XCOLS], F8, tag="kv1")
            for pb in range(3):
                for cset in (CT[:3], CT[3:]):
                    pss = [pspool.tile([128, 512], F32, tag="ps",
                                       name=f"psC{s}_{pb}_{c0}")
                           for (c0, cs) in cset]
                    for pj, (ca, cb) in enumerate(CPAIRS):
                        lhsT = kv8_s[:, pj, :, 128 * pb:128 * pb + 128]
                        for ti, (c0, cs) in enumerate(cset):
                            rhs = _pair_ap(msl, ca + c0, cb - ca, [[1, cs]])
                            nc.tensor.matmul(pss[ti][:, :cs], lhsT, rhs,
                                             start=(pj == 0), stop=(pj == 6),
                                             perf_mode=DR)
                    for ti, (c0, cs) in enumerate(cset):
                        nc.vector.scalar_tensor_tensor(
                            out=kv1[:, pb, c0:c0 + cs], in0=pss[ti][:, :cs],
                            scalar=kvb_s[:, pb:pb + 1], in1=mtile[:, c0:c0 + cs],
                            op0=mybir.AluOpType.add, op1=mybir.AluOpType.mult)

            # ---- Phase B: qkv = qdw(x1); 2-row groups (N=258), 5 DR tap-pairs
            # per group chained in PSUM; 4-group sets amortize weight loads.
            # route: pb -> list of (dst_tile, slot, part_lo, part_hi)
            def dwconv(src, wsrc, route, npb, bias_s, tag):
                for pb in range(npb):
                    for st in range(2):
                        gset = list(range(st * 4, st * 4 + 4))
                        pss = {gi: pspool.tile([128, 258], F32, tag="ps",
                                               name=f"ps{tag}{s}_{pb}_{gi}")
                               for gi in gset}
                        for pj, (ta, tb) in enumerate(BD_PAIRS):
                            oa = TAP_OFF[ta]
                            delta = (TAP_OFF[tb] - oa) if tb is not None else 1
                            lhsT = wsrc[:, pb, pj, :, :]
                            for gi in gset:
                                base = pb * XCOLS + (2 * gi + 1) * WP + 1 + oa
                                rhs = _pair_ap(src, base, delta, [[1, 258]])
                                nc.tensor.matmul(pss[gi][:, :], lhsT, rhs,
                                                 start=(pj == 0), stop=(pj == 4),
                                                 perf_mode=DR)
                        for gi in gset:
                            ps = pss[gi]
                            in_ap = bass.AP(tensor=ps.tensor, offset=ps.offset,
                                            ap=[list(ps.ap[0]), [WP, 2], [1, 128]])
                            for (dst, slot, plo, phi, chunked) in route(pb):
                                if chunked:
                                    out_ap = dst[plo:phi, 2 * gi:2 * gi + 2, slot, :]
                                else:
                                    out_ap = dst[plo:phi, slot,
                                                 2 * gi * 128:(2 * gi + 2) * 128]
                                nc.scalar.activation(
                                    out=out_ap,
                                    in_=in_ap[plo:phi],
                                    func=mybir.ActivationFunctionType.Identity,
                                    bias=bias_s[plo:phi, pb:pb + 1], scale=1.0 / WS)

            # k8/dwk8 are chunk-major [128, N128, 2, 128] so the kps DR
            # stationary pair is contiguous (pair stride 128 — ISA requires
            # small pair strides for dual-fp8 ldweights).
            q_t = bigpool.tile([128, 2, NS], BF16, tag="q_t")
            k8 = bigpool.tile([128, N128, 2, 128], F8, tag="k8")
            vq = bigpool.tile([128, 2, NS], BF16, tag="vq")

            def qkv_route(pb):
                dst = (q_t, k8, vq)[pb // 2]
                return [(dst, pb % 2, 0, 128, pb // 2 == 1)]

            dwconv(x1, qdw8_s, qkv_route, 6, qkvb_s, "B")

            # ---- Phase D: dw = kvdw(kv1); same structure, 3 pblocks
            dwk8 = bigpool.tile([128, N128, 2, 128], F8, tag="dwk8")
            dwv = bigpool.tile([128, 2, NS], BF16, tag="dwv")
            nc.vector.memset(dwk8[64:128, :, 1, :], 0.0)

            def dw_route(pb):
                if pb == 0:
                    return [(dwk8, 0, 0, 128, True)]
                if pb == 1:
                    return [(dwk8, 1, 0, 64, True), (dwv, 0, 64, 128, False)]
                return [(dwv, 1, 0, 128, False)]

            dwconv(kv1, kvdw8_s, dw_route, 3, dwb_s, "D")

            # ---- Phase E: v = newv(v_cc) + stats + dma out (bf16)
            vt = midpool.tile([128, 2, NS], BF16, tag="vt")
            for mb in range(2):
                msz = 128 if mb == 0 else 64
                pss = [pspool.tile([128, 512], F32, tag="ps", name=f"psE{s}_{mb}_{ic}")
                       for ic in range(NS // 512)]
                for j, (src, pb, base, sz) in enumerate(VCC):
                    data = (vq if src == "vq" else dwv)
                    lhsT = nvw_s[base:base + sz, j, mb * 128:mb * 128 + msz]
                    for ic in range(NS // 512):
                        rhs = data[base:base + sz, pb, ic * 512:(ic + 1) * 512]
                        nc.tensor.matmul(pss[ic][:msz, :], lhsT, rhs, start=(j == 0),
                                         stop=(j == 3),
                                         tile_position=(base, 0) if base else None)
                for ic in range(NS // 512):
                    nc.scalar.activation(out=vt[:msz, mb, ic * 512:(ic + 1) * 512],
                                         in_=pss[ic][:msz, :],
                                         func=mybir.ActivationFunctionType.Identity,
                                         bias=nvb_s[:msz, mb:mb + 1], scale=1.0)
            nc.sync.dma_start(out=v_out[0:128, s * NS:(s + 1) * NS], in_=vt[:, 0, :])
            nc.sync.dma_start(out=v_out[128:192, s * NS:(s + 1) * NS], in_=vt[:64, 1, :])
            for sub in range(NS // 512):
                si = s * (NS // 512) + sub
                sl = slice(sub * 512, (sub + 1) * 512)
                nc.vector.bn_stats(out=vstats[:, 0, si, :], in_=vt[:, 0, sl])
                nc.vector.bn_stats(out=vstats[:64, 1, si, :], in_=vt[:64, 1, sl])
                nc.vector.bn_stats(out=qstats[:, 0, si, :], in_=q_t[:, 0, sl])
                nc.vector.bn_stats(out=qstats[:, 1, si, :], in_=q_t[:, 1, sl])

            # ---- Phase F: per 128-n chunk: kT (newk, fp8 DR), qT (transpose),
            # then per chunk-pair: gram (fp8 DR) + kss.  Gram for pair P is
            # emitted one chunk late (during chunk 2P+2) so the scalar-engine
            # kq8/qT8 writes are done before the PE needs them.
            def emit_gram(gp):
                pb = gp % 2
                first, last = gp == 0, gp == n128_total // 2 - 1
                nc.tensor.matmul(gramA[:, :], qT8[:, pb, :, 0:128],
                                 kq8[:, pb, :, 0:192], start=first, stop=last,
                                 perf_mode=DR)
                nc.tensor.matmul(gramB[:, :], qT8[:, pb, :, 128:256],
                                 kq8[:, pb, :, 0:384], start=first, stop=last,
                                 perf_mode=DR)

            for ic in range(N128):
                c0 = ic * 128
                gidx = s * N128 + ic
                slot = ic % 2
                pbuf = (ic // 2) % 2
                kps = pspool.tile([128, 192], F32, tag="ps", name=f"kps{s}_{ic}")
                nc.tensor.matmul(kps[:, :], k8[:, ic, :, :], nkw8_s[:, 0, :, :],
                                 start=True, stop=False, perf_mode=DR)
                nc.tensor.matmul(kps[:, :], dwk8[:, ic, :, :], nkw8_s[:, 1, :, :],
                                 start=False, stop=True, perf_mode=DR)
                nc.scalar.copy(out=kq8[:, pbuf, slot, 0:192], in_=kps[:, :])
                nc.scalar.activation(out=kq8[:, pbuf, slot, 192:384], in_=kps[:, :],
                                     func=mybir.ActivationFunctionType.Square,
                                     scale=KSQS / NKS)
                qps = pspool.tile([128, 256], BF16, tag="ps", name=f"qps{s}_{ic}")
                nc.tensor.transpose(qps[:, 0:128], q_t[:, 0, c0:c0 + 128], id_s[:, :])
                nc.tensor.transpose(qps[:, 128:256], q_t[:, 1, c0:c0 + 128], id_s[:, :])
                nc.scalar.mul(out=qT8[:, pbuf, slot, :], in_=qps[:, :], mul=QTS)
                if slot == 0 and gidx >= 2:
                    emit_gram(gidx // 2 - 1)

        emit_gram(n128_total // 2 - 1)

        qmv = statpool.tile([128, 2, 2], F32)
        vmv = statpool.tile([128, 2, 2], F32)
        nc.vector.memset(qmv[:], 0.0)
        nc.vector.memset(vmv[:], 0.0)
        nc.vector.bn_aggr(out=qmv[:, 0, :], in_=qstats[:, 0, :, :])
        nc.vector.bn_aggr(out=qmv[:, 1, :], in_=qstats[:, 1, :, :])
        nc.vector.bn_aggr(out=vmv[:, 0, :], in_=vstats[:, 0, :, :])
        nc.vector.bn_aggr(out=vmv[:64, 1, :], in_=vstats[:64, 1, :, :])
        nc.sync.dma_start(out=qstats_out[:], in_=qmv[:])
        nc.sync.dma_start(out=vstats_out[:], in_=vmv[:])
        gA = statpool.tile([128, 192], F32)
        gB = statpool.tile([128, 192], F32)
        kssb = statpool.tile([128, 384], F32)
        nc.scalar.copy(out=gA[:], in_=gramA[:])
        nc.scalar.copy(out=gB[:], in_=gramB[:, 0:192])
        r0 = ONE_ROWS[0]
        nc.scalar.copy(out=kssb[r0:r0 + 1, :], in_=gramB[r0:r0 + 1, 0:384])
        nc.sync.dma_start(out=gramT_out[0:128, :], in_=gA[:])
        nc.sync.dma_start(out=gramT_out[128:256, :], in_=gB[:])
        nc.sync.dma_start(out=kss_out[:], in_=kssb[r0:r0 + 1, :])

    nc.compile()
    return nc


def build_l2(R=64, W=128):
    NS = R * W
    nc = bacc.Bacc("TRN2", target_bir_lowering=False, debug=False, num_devices=8)
    v_in = nc.dram_tensor("v_in", [192, NS], BF16, kind="ExternalInput").ap()
    awT = nc.dram_tensor("awT", [128, 2, 192], BF16, kind="ExternalInput").ap()
    pbias = nc.dram_tensor("pbias", [128, 2], F32, kind="ExternalInput").ap()
    out = nc.dram_tensor("out", [192, NS], BF16, kind="ExternalOutput").ap()

    with tile.TileContext(nc) as tc, ExitStack() as ctx:
        wpool = ctx.enter_context(tc.tile_pool(name="w", bufs=1))
        vpool = ctx.enter_context(tc.tile_pool(name="v", bufs=1))
        opool = ctx.enter_context(tc.tile_pool(name="o", bufs=1))
        pspool = ctx.enter_context(tc.tile_pool(name="ps", bufs=4, space="PSUM"))

        aw = wpool.tile([128, 2, 192], BF16)
        nc.sync.dma_start(out=aw[:], in_=awT[:])
        pb = wpool.tile([128, 2], F32)
        nc.sync.dma_start(out=pb[:], in_=pbias[:])

        # batched: v lives in SBUF whole, loaded in 4 chunks on 2 queues;
        # outputs accumulate in SBUF and leave in 2048-col chunks.
        vfull = vpool.tile([128, 2, NS], BF16)
        ofull = opool.tile([128, 2, NS], BF16)
        CL = NS // 4
        for j in range(4):
            sl = slice(j * CL, (j + 1) * CL)
            nc.sync.dma_start(out=vfull[:, 0, sl], in_=v_in[0:128, sl])
            nc.gpsimd.dma_start(out=vfull[:64, 1, sl], in_=v_in[128:192, sl])

        for c0 in range(0, NS, 512):
            for mb in range(2):
                msz = 128 if mb == 0 else 64
                ps = pspool.tile([128, 512], F32, tag="ps")
                nc.tensor.matmul(ps[:msz, :], aw[:, 0, mb * 128:mb * 128 + msz],
                                 vfull[:, 0, c0:c0 + 512], start=True, stop=False)
                nc.tensor.matmul(ps[:msz, :], aw[:64, 1, mb * 128:mb * 128 + msz],
                                 vfull[:64, 1, c0:c0 + 512], start=False, stop=True)
                nc.scalar.activation(out=ofull[:msz, mb, c0:c0 + 512], in_=ps[:msz, :],
                                     func=mybir.ActivationFunctionType.Identity,
                                     bias=pb[:msz, mb:mb + 1], scale=1.0)
            if c0 % 2048 == 2048 - 512:
                b0 = c0 + 512 - 2048
                eng = nc.sync if (b0 // 2048) % 2 == 0 else nc.gpsimd
                eng.dma_start(out=out[0:128, b0:b0 + 2048], in_=ofull[:, 0, b0:b0 + 2048])
                eng.dma_start(out=out[128:192, b0:b0 + 2048], in_=ofull[:64, 1, b0:b0 + 2048])
    nc.compile()
    return nc


# ---------------- host-side prep ----------------

def prep_weights(w):
    """w: dict of reference weights (numpy f32). Returns dict of L1 input arrays."""
    out = {}
    qw = w["q_w"][:, :, 0, 0]          # (576, 192)
    qwT = np.zeros((128, 2, 768), np.float32)
    for (pb, h, x1b, qb, ch0, nch) in qkv_halves():
        win = 128 * pb + 64 * h
        qwT[0:128, 0, win:win + nch] = qw.T[0:128, ch0:ch0 + nch]
        qwT[0:64, 1, win:win + nch] = qw.T[128:192, ch0:ch0 + nch]
    out["qw8"] = (qwT * WS).astype(f8)

    # qdw: grouped 3x3, (tap, pb) 128x128 block; repack into DR tap pairs
    qdw = w["qdw_w"]                   # (576, 3, 3, 3)
    qdwT = np.zeros((9, 6, 128, 128), np.float32)   # (tap_sorted, pb, row, col)
    for ti, (dy, dx) in enumerate(TAPS):
        for (pb, h, x1b, qb, ch0, nch) in qkv_halves():
            for gl in range(nch // 3):
                for i in range(3):
                    for j in range(3):
                        qdwT[ti, pb, x1b + 3 * gl + i, qb + 3 * gl + j] = \
                            qdw[ch0 + 3 * gl + j, i, dy + 1, dx + 1]
    qdw8 = np.zeros((128, 6, 5, 2, 128), np.float32)
    for pj, (ta, tb) in enumerate(BD_PAIRS):
        for pb in range(6):
            qdw8[:, pb, pj, 0, :] = qdwT[ta, pb]
            if tb is not None:
                qdw8[:, pb, pj, 1, :] = qdwT[tb, pb]
    out["qdw8"] = (qdw8 * WS).astype(f8)

    # kv conv: 14 k-tiles -> 7 DR pairs (order must match CPAIRS in build_l1)
    kvw = w["kv_w"]                    # (384, 192, 3, 3)
    tl = lambda dy, dx: kvw[:, 0:128, dy + 1, dx + 1].T       # (128, 384) lo
    th = lambda dy, dx: kvw[:, 128:192, dy + 1, dx + 1].T     # (64, 384) hi
    ktiles = []
    for dy, dx in TAPS:   # 9 lo tiles (sorted tap order == TAPS order)
        ktiles.append(("lo", (dy, dx)))
    ktiles.append(("d1", -1)); ktiles.append(("d1", 0)); ktiles.append(("d1", 1))
    ktiles.append(("single", None)); ktiles.append(("d2", None))
    kv8 = np.zeros((128, 7, 2, 384), np.float32)
    for pj in range(7):
        for j in range(2):
            kind, arg = ktiles[2 * pj + j]
            blk = np.zeros((128, 384), np.float32)
            if kind == "lo":
                blk[0:128] = tl(*arg)
            elif kind == "d1":
                blk[0:64] = th(-1, arg)
                blk[64:128] = th(0, arg)
            elif kind == "d2":
                blk[0:64] = th(1, -1)
                blk[64:128] = th(1, 1)
            elif kind == "single":
                blk[0:64] = th(1, 0)
            kv8[:, pj, j, :] = blk
    out["kv8"] = (kv8 * WS).astype(f8)

    # kvdw depthwise: plain layout, diag blocks per pblock, DR tap pairs
    kvdw = w["kvdw_w"][:, 0]           # (384, 3, 3)
    kvdw8 = np.zeros((128, 3, 5, 2, 128), np.float32)
    for pj, (ta, tb) in enumerate(BD_PAIRS):
        for pb in range(3):
            dya, dxa = TAPS[ta]
            kvdw8[:, pb, pj, 0, :] = np.diag(kvdw[128 * pb:128 * pb + 128, dya + 1, dxa + 1])
            if tb is not None:
                dyb, dxb = TAPS[tb]
                kvdw8[:, pb, pj, 1, :] = np.diag(kvdw[128 * pb:128 * pb + 128, dyb + 1, dxb + 1])
    out["kvdw8"] = (kvdw8 * WS).astype(f8)

    def dw_channel(pb, p):
        return 128 * pb + p   # plain layout

    # newk (no bias on device; kT = NKS * k0): contraction sources
    # pass0 = (k8 slot0 = qkv pb2, k8 slot1 = qkv pb3)
    # pass1 = (dwk8 slot0 = dw pb0, dwk8 slot1 = dw pb1 parts 0..63)
    KCC = [("qkv", 2, 0, 128), ("qkv", 3, 0, 128), ("dw", 0, 0, 128), ("dw", 1, 0, 64)]
    nk = w["newk_w"][:, :, 0, 0]       # (192, 384): in = [k(192) | k_mask(192)]
    nkm = np.zeros((128, 4, 192), np.float32)
    for j, (src, pb, base, sz) in enumerate(KCC):
        for p in range(base, base + sz):
            if src == "qkv":
                ch = qkv_channel_at(pb, p)
                if ch is not None:
                    nkm[p, j, :] = nk[:, ch - 192]      # k part: qkv ch 192-383
            else:
                ch = dw_channel(pb, p)
                if ch < 192:
                    nkm[p, j, :] = nk[:, 192 + ch]      # k_mask: dw ch 0-191
    out["nkw8"] = (nkm.reshape(128, 2, 2, 192) * NKS).astype(f8)

    nv = w["newv_w"][:, :, 0, 0]       # (192, 384): in = [v(192) | v_mask(192)]
    nvT = np.zeros((128, 4, 192), np.float32)
    VCC_P = [("qkv", 4, 0, 128), ("qkv", 5, 0, 128), ("dw", 1, 64, 64), ("dw", 2, 0, 128)]
    for j, (src, pb, base, sz) in enumerate(VCC_P):
        for p in range(base, base + sz):
            if src == "qkv":
                ch = qkv_channel_at(pb, p)
                if ch is not None:
                    nvT[p, j, :] = nv[:, ch - 384]      # v part: qkv ch 384-575
            else:
                ch = dw_channel(pb, p)
                if ch >= 192:
                    nvT[p, j, :] = nv[:, ch]            # v_mask: dw ch 192-383
    out["newv_wT"] = nvT.astype(bf16)

    out["ident"] = np.eye(128, dtype=bf16)

    x1b = np.zeros((128, 6), np.float32)
    qkvb = np.zeros((128, 6), np.float32)
    for (pb, h, x1b_base, qb, ch0, nch) in qkv_halves():
        x1b[x1b_base:x1b_base + nch, pb] = w["q_b"][ch0:ch0 + nch]
        qkvb[qb:qb + nch, pb] = w["qdw_b"][ch0:ch0 + nch]
    # hijacked ones rows: junk q pb1 partitions become the exact constant
    # 1/QTS, which the qT8 scale turns into 1.0 -> gramB rows = ksum/kss
    for r in ONE_ROWS:
        qkvb[r, 1] = 1.0 / QTS
    out["x1_bias"] = x1b * WS
    out["qkv_bias"] = qkvb

    kvb = np.zeros((128, 3), np.float32)
    dwb = np.zeros((128, 3), np.float32)
    for pb in range(3):
        kvb[:, pb] = w["kv_b"][128 * pb:128 * pb + 128]
        dwb[:, pb] = w["kvdw_b"][128 * pb:128 * pb + 128]
    out["kv_bias"] = kvb * WS
    out["kvdw_bias"] = dwb
    nvb = np.zeros((128, 2), np.float32)
    nvb[:, 0] = w["newv_b"][0:128]
    nvb[0:64, 1] = w["newv_b"][128:192]
    out["newv_bias"] = nvb
    return out


def prep_masks(R, H, half):
    m = np.zeros((R + 2, WP), np.float32)
    for r in range(R + 2):
        g = half * R + (r - 1)
        if 0 <= g < H:
            m[r, 1:129] = 1.0 / WS
    return m.reshape(1, -1)


def prep_core(x, xm, b, half, R, H):
    xp = np.zeros((192, R + 2, WP), np.float32)
    mp = np.zeros((192, R + 4, WP), np.float32)
    for r in range(R + 2):
        g = half * R + (r - 1)
        if 0 <= g < H:
            xp[:, r, 1:129] = x[b, :, g, :]
    for r in range(R + 4):
        g = half * R + (r - 2)
        if 0 <= g < H:
            mp[:, r, 1:129] = xm[b, :, g, :]
    xp = xp.reshape(192, -1)
    x8 = np.zeros((128, 2, xp.shape[1]), np.float32)
    x8[:, 0, :] = xp[0:128]
    x8[0:64, 1, :] = xp[128:192]
    x8[64:128, 1, :] = xp[128:192]   # dup (weights zero) to avoid NaN garbage
    mp = mp.reshape(192, -1)
    L = mp.shape[1]
    hi = mp[128:192]
    d1 = np.zeros((128, L), np.float32)
    d2 = np.zeros((128, L), np.float32)
    d1[0:64] = hi
    d1[64:128, :L - WP] = hi[:, WP:]
    d2[0:64] = hi
    d2[64:128, :L - 2] = hi[:, 2:]
    return {
        "x8": x8.astype(f8),
        "xm_lo": mp[0:128].astype(f8),
        "xm_d1": d1.astype(f8), "xm_d2": d2.astype(f8),
        "mask_rc": prep_masks(R, H, half).astype(bf16),
    }


# ---------------- host glue (unchanged semantics) ----------------

def _q_maps():
    part = np.zeros(192, np.int64)
    pblk = np.zeros(192, np.int64)
    for (pb, h, x1b, qb, ch0, nch) in qkv_halves():
        if pb >= 2:
            continue
        for i in range(nch):
            pblk[ch0 + i] = pb
            part[ch0 + i] = qb + i
    return pblk, part


def _ss_from_qstats(stats, n_half):
    pblk, part = _q_maps()
    mv = stats.astype(np.float64)
    return (mv[part, pblk, 1] + mv[part, pblk, 0] ** 2) * n_half


def _sum_from_qstats(stats, n_half):
    pblk, part = _q_maps()
    mv = stats.astype(np.float64)
    return mv[part, pblk, 0] * n_half


def _ss_from_vstats(stats, n_half):
    ss = np.zeros(192, np.float64)
    mv = stats.astype(np.float64)
    ss[0:128] = (mv[0:128, 0, 1] + mv[0:128, 0, 0] ** 2) * n_half
    ss[128:192] = (mv[0:64, 1, 1] + mv[0:64, 1, 0] ** 2) * n_half
    return ss


def glue(res0, res1, temperature, proj_w, proj_b, n_half):
    """Combine two half-core L1 results -> L2 inputs (awT, pbias)."""
    GT = res0["gramT_out"].astype(np.float64) + res1["gramT_out"].astype(np.float64)
    pblk, part = _q_maps()
    qrow = pblk * 128 + part
    # GT rows are qT cols (pb*128+part); cols are newk out-ch d. Stored values
    # are sum_n (QTS*q) * (NKS*k0).
    Gq = GT[qrow, :] / (QTS * NKS)             # (c, d): sum_n q[c,n] k0[d,n]
    qss = _ss_from_qstats(res0["qstats_out"], n_half) + _ss_from_qstats(res1["qstats_out"], n_half)
    qsum = _sum_from_qstats(res0["qstats_out"], n_half) + _sum_from_qstats(res1["qstats_out"], n_half)
    vss = _ss_from_vstats(res0["vstats_out"], n_half) + _ss_from_vstats(res1["vstats_out"], n_half)
    kss_raw = (res0["kss_out"].astype(np.float64) + res1["kss_out"].astype(np.float64))[0]
    k0sum = kss_raw[0:192] / NKS               # sum_n k0 (ones x kT cols)
    k0ss = kss_raw[192:384] / (KSQS * KSQS)    # sum_n k0^2 (ones x ksq cols)
    return Gq, qss, qsum, vss, k0ss, k0sum


def glue_full(res0, res1, temperature, newk_b, proj_w, proj_b, n_half):
    Gq, qss, qsum, vss, k0ss, k0sum = glue(res0, res1, temperature, proj_w, proj_b, n_half)
    b = newk_b.astype(np.float64)              # (192,)
    # k = k0 + b: gram/kss bias corrections (sums already cover both halves,
    # total n = 2 * n_half)
    G = Gq + qsum[:, None] * b[None, :]        # (c, d): sum_n q k
    kss = k0ss + 2 * b * k0sum + (2 * n_half) * b * b
    qn = np.maximum(np.sqrt(qss), 1e-12)
    kn = np.maximum(np.sqrt(kss), 1e-12)
    vn = np.maximum(np.sqrt(vss), 1e-12)
    A = G / (qn[:, None] * kn[None, :])        # (c, d)
    M = np.zeros((192, 192), np.float64)
    t = np.asarray(temperature).reshape(-1)
    for h in range(8):
        sl = slice(24 * h, 24 * h + 24)
        a = A[sl, sl] * t[h]
        a = a - a.max(axis=-1, keepdims=True)
        e = np.exp(a)
        sm = e / e.sum(axis=-1, keepdims=True)
        M[sl, sl] = sm / vn[None, sl]
    At = proj_w[:, :, 0, 0].astype(np.float64) @ M   # (out-ch o, d)
    awT = np.zeros((128, 2, 192), np.float32)
    awT[:, 0, :] = At.T[0:128]
    awT[0:64, 1, :] = At.T[128:192]
    pbias = np.zeros((128, 2), np.float32)
    pbias[:, 0] = proj_b[0:128]
    pbias[0:64, 1] = proj_b[128:192]
    return {"awT": awT.astype(bf16), "pbias": pbias}


# ---------------- driver: kernel(**inputs) ----------------
from concourse.bass_utils import run_bass_kernel_spmd

R_FULL, H_FULL, B_FULL = 64, 128, 4
_NC1 = None
_NC2 = None


def _get_progs():
    global _NC1, _NC2
    if _NC1 is None:
        _NC1 = build_l1(R=R_FULL, S=16)
        _NC2 = build_l2(R=R_FULL)
    return _NC1, _NC2


def kernel(**inputs):
    inputs = {k: np.asarray(v) for k, v in inputs.items()}
    x, xm = inputs["x"], inputs["x_mask"]
    nc1, nc2 = _get_progs()
    wprep = prep_weights(inputs)
    in_maps = []
    for core in range(8):
        b, half = core // 2, core % 2
        m = dict(wprep)
        m.update(prep_core(x, xm, b, half, R_FULL, H_FULL))
        in_maps.append(m)
    res1 = run_bass_kernel_spmd(nc1, in_maps, list(range(8))).results

    n_half = R_FULL * 128
    in_maps2 = []
    for core in range(8):
        b, half = core // 2, core % 2
        if half == 0:
            l2c = glue_full(res1[2 * b], res1[2 * b + 1], inputs["temperature"],
                            inputs["newk_b"], inputs["proj_w"], inputs["proj_b"],
                            n_half)
        m = dict(l2c)
        m["v_in"] = res1[core]["v_out"]
        in_maps2.append(m)
    res2 = run_bass_kernel_spmd(nc2, in_maps2, list(range(8))).results

    out = np.empty((B_FULL, 192, H_FULL, 128), np.float32)
    for core in range(8):
        b, half = core // 2, core % 2
        out[b, :, half * R_FULL:(half + 1) * R_FULL, :] = \
            res2[core]["out"].reshape(192, R_FULL, 128).astype(np.float32)
    return out


# revision 38
# speedup vs baseline: 1.1046x; 1.0916x over previous
"""Trainium2 Bass kernel for nn_Cross_Attention — fp8 DoubleRow rewrite.

L1: convs + gram partials with fp8 DoubleRow matmuls (phases A-D), fp8 F-phase
(newk/gram/kss via DR), bf16 E. L2: attn-apply with chunked v DMA. Host glue
between (softmax etc.). Sharding: 4 samples x 2 row-halves across 8 cores.
"""
import sys
sys.path.insert(0, "/opt/trn_rl_repo")
import numpy as np
import ml_dtypes

import concourse.bass as bass
import concourse.tile as tile
from concourse import bacc, mybir
from contextlib import ExitStack

BF16 = mybir.dt.bfloat16
F8 = mybir.dt.float8e4
F32 = mybir.dt.float32
bf16 = ml_dtypes.bfloat16
f8 = ml_dtypes.float8_e4m3
DR = mybir.MatmulPerfMode.DoubleRow

WS = 16.0          # fp8 weight pre-scale
WP = 130           # padded row width
NKS = 32.0         # newk weight pre-scale (kT stored as NKS*k0)
QTS = 16.0         # qT pre-scale (qT stored as QTS*q)
KSQS = 64.0        # ksq stored as (KSQS*k0)**2
NVS = 32.0         # newv weight pre-scale
VS = 8.0           # v8 storage scale (v stored as VS*v)
AWS = 512.0        # attn-weight (aw8) pre-scale
ONE_ROWS = (32, 33)  # junk q pb1 partitions hijacked as 1.0 cols in qT8 (ksum/kss);
                     # must be 32-aligned (engine partition-base alignment)

TAPS = [(dy, dx) for dy in (-1, 0, 1) for dx in (-1, 0, 1)]
# taps sorted by flat offset dy*WP+dx (they already are, given WP>2)
TAP_OFF = [dy * WP + dx for dy, dx in TAPS]
# DoubleRow tap pairs for B/D: (idx_a, idx_b). The odd tap rides first with a
# zero-weight second tile at stride +1 (always in-bounds since tap 0 has the
# smallest offset).
BD_PAIRS = [(0, None), (1, 2), (3, 4), (5, 6), (7, 8)]


def qkv_halves():
    """Per (pb, half): (x1_base, qkv_base, ch0, nch).  ch0 = qkv-global channel."""
    out = []
    for pb in range(6):
        P, odd = pb // 2, pb % 2
        for h in (0, 1):
            nch = 3 if (odd and h == 1) else 63
            ch0 = 3 * (64 * P + 42 * odd + 21 * h)
            x1b = 64 * h
            qb = 64 * h if not odd else 64 * (1 - h)
            out.append((pb, h, x1b, qb, ch0, nch))
    return out


def qkv_channel_at(pb, p):
    """qkv-global channel stored at partition p of qkv pblock pb, or None."""
    for (pb2, h, x1b, qb, ch0, nch) in qkv_halves():
        if pb2 == pb and qb <= p < qb + nch:
            return ch0 + (p - qb)
    return None


# newk/newv input chunks with PLAIN dw layout (dw pb_i = kv ch 128i..128i+127):
# k = qkv ch 192-383 (pb 2,3); k_mask = kv ch 0-191 = dw pb0 + dw pb1[0:64]
# v = qkv ch 384-575 (pb 4,5); v_mask = kv ch 192-383 = dw pb1[64:128] + dw pb2
# Tile routing: q_t = qkv pb0,1 (bf16); k8 = qkv pb2,3 (f8, chunk-major);
# vq = qkv pb4,5 (f8); dwk8 = dw pb0 + dw pb1[0:64] (f8, chunk-major);
# dwv = dw pb1[64:128] + dw pb2 (f8, slot0 parts 64-127 used).


def _bcast(ap, p):
    return bass.AP(tensor=ap.tensor, offset=ap.offset, ap=[[0, p]] + list(ap.ap[1:]))


def _pair_ap(t, off, delta, dims):
    """AP over tile t at flat free-offset `off`: [partitions, 2 (stride delta), *dims]."""
    return bass.AP(tensor=t.tensor, offset=t.offset + off,
                   ap=[list(t.ap[0]), [delta, 2]] + [list(d) for d in dims])


def build_l1(R=64, W=128, S=16):
    assert W == 128 and R % S == 0 and S % 4 == 0
    NSLAB = R // S
    NS = S * W
    XCOLS = (S + 2) * WP
    MCOLS = (S + 4) * WP
    MC2 = MCOLS + 2
    N128 = NS // 128

    nc = bacc.Bacc("TRN2", target_bir_lowering=False, debug=False, num_devices=8)

    def din(name, shape, dt=F8):
        return nc.dram_tensor(name, shape, dt, kind="ExternalInput").ap()

    def dout(name, shape, dt=F32):
        return nc.dram_tensor(name, shape, dt, kind="ExternalOutput").ap()

    x8 = din("x8", [128, 2, (R + 2) * WP])
    xm_lo = din("xm_lo", [128, (R + 4) * WP])
    xm_d1 = din("xm_d1", [128, (R + 4) * WP])
    xm_d2 = din("xm_d2", [128, (R + 4) * WP])
    qw8 = din("qw8", [128, 2, 768])
    qdw8 = din("qdw8", [128, 6, 5, 2, 128])
    kv8 = din("kv8", [128, 7, 2, 384])
    kvdw8 = din("kvdw8", [128, 3, 5, 2, 128])
    nkw8 = din("nkw8", [128, 2, 2, 192], F8)
    nvw8 = din("nvw8", [128, 2, 2, 192], F8)
    ident = din("ident", [128, 128], BF16)
    x1_bias = din("x1_bias", [128, 6], F32)      # x16
    qkv_bias = din("qkv_bias", [128, 6], F32)
    kv_bias = din("kv_bias", [128, 3], F32)      # x16, plain layout
    kvdw_bias = din("kvdw_bias", [128, 3], F32)  # plain layout
    newv_bias = din("newv_bias", [128, 2], F32)  # x VS
    edge = din("edge", [128, 2], F32)            # per-core edge-row multipliers

    v_out = dout("v_out", [192, R * W], F8)
    gramT_out = dout("gramT_out", [256, 192])    # rows = qcol space (pb*128+part)
    kss_out = dout("kss_out", [1, 384])          # [sum (KSQS*k0)^2 | sum NKS*k0]
    qstats_out = dout("qstats_out", [128, 2, 2])
    vstats_out = dout("vstats_out", [128, 2, 2])

    # C-phase DoubleRow k-tile pair coordinates in the [128, 3, MC2] msl tile
    # flat free space: region*MC2 + off.  Region 0 = lo channels, 1 = d1
    # (hi | hi<<WP), 2 = d2 (hi | hi<<2).
    def lo(dy, dx):
        return (1 + dy) * WP + dx + 1

    def d1(off):
        return MC2 + off

    def d2(off):
        return 2 * MC2 + off

    # 14 k-tiles -> 7 pairs; weight slot kv8[:, pair, j, :] must match.
    CPAIRS = [
        (lo(-1, -1), lo(-1, 0)), (lo(-1, 1), lo(0, -1)), (lo(0, 0), lo(0, 1)),
        (lo(1, -1), lo(1, 0)), (lo(1, 1), d1(0)), (d1(1), d1(2)),
        (d1(2 * WP + 1), d2(2 * WP)),
    ]

    with tile.TileContext(nc) as tc, ExitStack() as ctx:
        wpool = ctx.enter_context(tc.tile_pool(name="weights", bufs=1))
        xpool = ctx.enter_context(tc.tile_pool(name="xslab", bufs=2))
        bigpool = ctx.enter_context(tc.tile_pool(name="big", bufs=1))
        midpool = ctx.enter_context(tc.tile_pool(name="mid", bufs=2))
        smpool = ctx.enter_context(tc.tile_pool(name="small", bufs=4))
        statpool = ctx.enter_context(tc.tile_pool(name="stats", bufs=1))
        pspool = ctx.enter_context(tc.tile_pool(name="ps", bufs=6, space="PSUM"))
        pspers = ctx.enter_context(tc.tile_pool(name="pspers", bufs=1, space="PSUM"))

        def load1(ap_in, shape, dt=F8, eng=None):
            t = wpool.tile(shape, dt, tag=ap_in.tensor.name)
            (eng or nc.sync).dma_start(out=t[:ap_in.shape[0]], in_=ap_in[:])
            return t

        g = nc.gpsimd
        qw8_s = load1(qw8, [128, 2, 768])
        x1b_s = load1(x1_bias, [128, 6], F32)
        qdw8_s = load1(qdw8, [128, 6, 5, 2, 128], eng=g)
        kv8_s = load1(kv8, [128, 7, 2, 384], eng=g)
        kvdw8_s = load1(kvdw8, [128, 3, 5, 2, 128], eng=g)
        nkw8_s = load1(nkw8, [128, 2, 2, 192], F8, eng=g)
        nvw8_s = load1(nvw8, [128, 2, 2, 192], F8, eng=g)
        id_s = load1(ident, [128, 128], BF16, eng=g)
        qkvb_s = load1(qkv_bias, [128, 6], F32, eng=g)
        kvb_s = load1(kv_bias, [128, 3], F32, eng=g)
        dwb_s = load1(kvdw_bias, [128, 3], F32, eng=g)
        nvb_s = load1(newv_bias, [128, 2], F32, eng=g)
        edge_s = load1(edge, [128, 2], F32, eng=g)

        # persistent accumulators: one PSUM bank each (zero-region granularity)
        # gramB is 384 wide: cols 192:384 see the ksq half of the moving data;
        # its ONE_ROWS rows (ones in the stationary) yield ksum/kss.
        gramA = pspers.tile([128, 192], F32)
        gramB = pspers.tile([128, 384], F32)

        qstats = statpool.tile([128, 2, NSLAB * (NS // 512), 6], F32)
        vstats = statpool.tile([128, 2, NSLAB * (NS // 512), 6], F32)
        # double-buffered chunk-pair staging for gram/kss (dims: pairbuf, slot).
        # inner dim padded to 400 so the DR pair stride is NOT contiguous with
        # the column dim (contiguous dims get re-flattened in lowering, which
        # breaks the pair interpretation of the moving AP).
        kq8 = statpool.tile([128, 2, 2, 400], F8)    # [0:192]=NKS*k0, [192:384]=ksq
        qT8 = statpool.tile([128, 2, 2, 256], F8)

        n128_total = NSLAB * N128
        CT = [(c0, min(512, XCOLS - c0)) for c0 in range(0, XCOLS, 512)]

        for s in range(NSLAB):
            xsl = xpool.tile([128, 2, XCOLS], F8, tag="xsl")
            msl = xpool.tile([128, 3, MC2], F8, tag="msl")
            off = s * S * WP
            # spread the slab input loads over several DMA queues
            nc.sync.dma_start(out=xsl[:], in_=x8[:, :, off:off + XCOLS])
            nc.scalar.dma_start(out=msl[:, 0, 1:1 + MCOLS], in_=xm_lo[:, off:off + MCOLS])
            nc.gpsimd.dma_start(out=msl[:, 1, 1:1 + MCOLS], in_=xm_d1[:, off:off + MCOLS])
            nc.scalar.dma_start(out=msl[:, 2, 1:1 + MCOLS], in_=xm_d2[:, off:off + MCOLS])
            for r in range(3):
                nc.vector.memset(msl[:, r, 0:1], 0.0)
                nc.vector.memset(msl[:, r, MC2 - 1:MC2], 0.0)

            def pad_zero(t, npb):
                # zero the 2 pad columns of every row: flat r*WP + {0, 129}
                ap = bass.AP(tensor=t.tensor, offset=t.offset,
                             ap=[list(t.ap[0]), [XCOLS, npb], [WP, S + 2], [WP - 1, 2]])
                nc.vector.memset(ap, 0.0)
                # per-core edge pad row: slab0 row0 (half 0) / slab3 last row
                # (half 1), selected by the `edge` input multipliers
                if s == 0:
                    nc.vector.tensor_scalar(
                        out=t[:, :, 0:WP], in0=t[:, :, 0:WP],
                        scalar1=edge_s[:, 0:1], scalar2=None,
                        op0=mybir.AluOpType.mult)
                if s == NSLAB - 1:
                    nc.vector.tensor_scalar(
                        out=t[:, :, (S + 1) * WP:(S + 2) * WP],
                        in0=t[:, :, (S + 1) * WP:(S + 2) * WP],
                        scalar1=edge_s[:, 1:2], scalar2=None,
                        op0=mybir.AluOpType.mult)

            # ---- Phase A: x1 = 1x1(x); one DR matmul per (pb, col-tile)
            x1 = bigpool.tile([128, 6, XCOLS], F8, tag="x1")
            for pb in range(6):
                for c0, cs in CT:
                    ps = pspool.tile([128, 512], F32, tag="ps", name=f"psA{s}_{pb}_{c0}")
                    nc.tensor.matmul(ps[:, :cs], qw8_s[:, :, 128 * pb:128 * pb + 128],
                                     xsl[:, :, c0:c0 + cs], start=True, stop=True,
                                     perf_mode=DR)
                    nc.vector.tensor_scalar(
                        out=x1[:, pb, c0:c0 + cs], in0=ps[:, :cs],
                        scalar1=x1b_s[:, pb:pb + 1], scalar2=1.0 / WS,
                        op0=mybir.AluOpType.add, op1=mybir.AluOpType.mult)
            pad_zero(x1, 6)

            # ---- Phase C: kv1 = 3x3(xm); 7 DR pairs per (pb, col-tile)
            kv1 = bigpool.tile([128, 3, XCOLS], F8, tag="kv1")
            for pb in range(3):
                for cset in (CT[:3], CT[3:]):
                    pss = [pspool.tile([128, 512], F32, tag="ps",
                                       name=f"psC{s}_{pb}_{c0}")
                           for (c0, cs) in cset]
                    for pj, (ca, cb) in enumerate(CPAIRS):
                        lhsT = kv8_s[:, pj, :, 128 * pb:128 * pb + 128]
                        for ti, (c0, cs) in enumerate(cset):
                            rhs = _pair_ap(msl, ca + c0, cb - ca, [[1, cs]])
                            nc.tensor.matmul(pss[ti][:, :cs], lhsT, rhs,
                                             start=(pj == 0), stop=(pj == 6),
                                             perf_mode=DR)
                    for ti, (c0, cs) in enumerate(cset):
                        nc.vector.tensor_scalar(
                            out=kv1[:, pb, c0:c0 + cs], in0=pss[ti][:, :cs],
                            scalar1=kvb_s[:, pb:pb + 1], scalar2=1.0 / WS,
                            op0=mybir.AluOpType.add, op1=mybir.AluOpType.mult)
            pad_zero(kv1, 3)

            # ---- Phase B: qkv = qdw(x1); 2-row groups (N=258), 5 DR tap-pairs
            # per group chained in PSUM; 4-group sets amortize weight loads.
            # route: pb -> list of (dst_tile, slot, part_lo, part_hi)
            def dwconv(src, wsrc, route, npb, bias_s, tag):
                for pb in range(npb):
                    for st in range(2):
                        gset = list(range(st * 4, st * 4 + 4))
                        pss = {gi: pspool.tile([128, 258], F32, tag="ps",
                                               name=f"ps{tag}{s}_{pb}_{gi}")
                               for gi in gset}
                        for pj, (ta, tb) in enumerate(BD_PAIRS):
                            oa = TAP_OFF[ta]
                            delta = (TAP_OFF[tb] - oa) if tb is not None else 1
                            lhsT = wsrc[:, pb, pj, :, :]
                            for gi in gset:
                                base = pb * XCOLS + (2 * gi + 1) * WP + 1 + oa
                                rhs = _pair_ap(src, base, delta, [[1, 258]])
                                nc.tensor.matmul(pss[gi][:, :], lhsT, rhs,
                                                 start=(pj == 0), stop=(pj == 4),
                                                 perf_mode=DR)
                        for gi in gset:
                            ps = pss[gi]
                            in_ap = bass.AP(tensor=ps.tensor, offset=ps.offset,
                                            ap=[list(ps.ap[0]), [WP, 2], [1, 128]])
                            for (dst, slot, plo, phi, chunked) in route(pb):
                                if chunked:
                                    out_ap = dst[plo:phi, 2 * gi:2 * gi + 2, slot, :]
                                else:
                                    out_ap = dst[plo:phi, slot,
                                                 2 * gi * 128:(2 * gi + 2) * 128]
                                nc.scalar.activation(
                                    out=out_ap,
                                    in_=in_ap[plo:phi],
                                    func=mybir.ActivationFunctionType.Identity,
                                    bias=bias_s[plo:phi, pb:pb + 1], scale=1.0 / WS)

            # k8/dwk8 are chunk-major [128, N128, 2, 128] so the kps DR
            # stationary pair is contiguous (pair stride 128 — ISA requires
            # small pair strides for dual-fp8 ldweights).
            q_t = bigpool.tile([128, 2, NS], BF16, tag="q_t")
            k8 = bigpool.tile([128, N128, 2, 128], F8, tag="k8")
            vq = bigpool.tile([128, 2, NS], F8, tag="vq")

            def qkv_route(pb):
                dst = (q_t, k8, vq)[pb // 2]
                return [(dst, pb % 2, 0, 128, pb // 2 == 1)]

            dwconv(x1, qdw8_s, qkv_route, 6, qkvb_s, "B")

            # ---- Phase D: dw = kvdw(kv1); same structure, 3 pblocks
            dwk8 = bigpool.tile([128, N128, 2, 128], F8, tag="dwk8")
            dwv = bigpool.tile([128, 2, NS], F8, tag="dwv")
            nc.vector.memset(dwk8[64:128, :, 1, :], 0.0)
            nc.vector.memset(dwv[0:64, 0, :], 0.0)

            def dw_route(pb):
                if pb == 0:
                    return [(dwk8, 0, 0, 128, True)]
                if pb == 1:
                    return [(dwk8, 1, 0, 64, True), (dwv, 0, 64, 128, False)]
                return [(dwv, 1, 0, 128, False)]

            dwconv(kv1, kvdw8_s, dw_route, 3, dwb_s, "D")

            # ---- Phase E: v = newv(v_cc), fp8 DR (pair over the two source
            # tiles of each pass); psum holds NVS*(v - b); bn_stats on psum.
            vt = midpool.tile([128, 2, NS], F8, tag="vt")
            for mb in range(2):
                msz = 128 if mb == 0 else 64
                for ic in range(NS // 512):
                    sl = slice(ic * 512, (ic + 1) * 512)
                    ps = pspool.tile([128, 512], F32, tag="ps",
                                     name=f"psE{s}_{mb}_{ic}")
                    nc.tensor.matmul(ps[:msz, :],
                                     nvw8_s[:, 0, :, mb * 128:mb * 128 + msz],
                                     vq[:, :, sl], start=True, stop=False,
                                     perf_mode=DR)
                    nc.tensor.matmul(ps[:msz, :],
                                     nvw8_s[:, 1, :, mb * 128:mb * 128 + msz],
                                     dwv[:, :, sl], start=False, stop=True,
                                     perf_mode=DR)
                    si = s * (NS // 512) + ic
                    nc.vector.bn_stats(out=vstats[:msz, mb, si, :], in_=ps[:msz, :])
                    nc.scalar.activation(out=vt[:msz, mb, sl], in_=ps[:msz, :],
                                         func=mybir.ActivationFunctionType.Identity,
                                         bias=nvb_s[:msz, mb:mb + 1], scale=VS / NVS)
            nc.sync.dma_start(out=v_out[0:128, s * NS:(s + 1) * NS], in_=vt[:, 0, :])
            nc.sync.dma_start(out=v_out[128:192, s * NS:(s + 1) * NS], in_=vt[:64, 1, :])
            for sub in range(NS // 512):
                si = s * (NS // 512) + sub
                sl = slice(sub * 512, (sub + 1) * 512)
                nc.vector.bn_stats(out=qstats[:, 0, si, :], in_=q_t[:, 0, sl])
                nc.vector.bn_stats(out=qstats[:, 1, si, :], in_=q_t[:, 1, sl])

            # ---- Phase F: per 128-n chunk: kT (newk, fp8 DR), qT (transpose),
            # then per chunk-pair: gram (fp8 DR) + kss.  Gram for pair P is
            # emitted one chunk late (during chunk 2P+2) so the scalar-engine
            # kq8/qT8 writes are done before the PE needs them.
            def emit_gram(gp):
                pb = gp % 2
                first, last = gp == 0, gp == n128_total // 2 - 1
                nc.tensor.matmul(gramA[:, :], qT8[:, pb, :, 0:128],
                                 kq8[:, pb, :, 0:192], start=first, stop=last,
                                 perf_mode=DR)
                nc.tensor.matmul(gramB[:, :], qT8[:, pb, :, 128:256],
                                 kq8[:, pb, :, 0:384], start=first, stop=last,
                                 perf_mode=DR)

            for ic in range(N128):
                c0 = ic * 128
                gidx = s * N128 + ic
                slot = ic % 2
                pbuf = (ic // 2) % 2
                kps = pspool.tile([128, 192], F32, tag="ps", name=f"kps{s}_{ic}")
                nc.tensor.matmul(kps[:, :], k8[:, ic, :, :], nkw8_s[:, 0, :, :],
                                 start=True, stop=False, perf_mode=DR)
                nc.tensor.matmul(kps[:, :], dwk8[:, ic, :, :], nkw8_s[:, 1, :, :],
                                 start=False, stop=True, perf_mode=DR)
                nc.scalar.copy(out=kq8[:, pbuf, slot, 0:192], in_=kps[:, :])
                nc.scalar.activation(out=kq8[:, pbuf, slot, 192:384], in_=kps[:, :],
                                     func=mybir.ActivationFunctionType.Square,
                                     scale=KSQS / NKS)
                qps = pspool.tile([128, 256], BF16, tag="ps", name=f"qps{s}_{ic}")
                nc.tensor.transpose(qps[:, 0:128], q_t[:, 0, c0:c0 + 128], id_s[:, :])
                nc.tensor.transpose(qps[:, 128:256], q_t[:, 1, c0:c0 + 128], id_s[:, :])
                nc.scalar.mul(out=qT8[:, pbuf, slot, :], in_=qps[:, :], mul=QTS)
                if slot == 0 and gidx >= 2:
                    emit_gram(gidx // 2 - 1)

        emit_gram(n128_total // 2 - 1)

        qmv = statpool.tile([128, 2, 2], F32)
        vmv = statpool.tile([128, 2, 2], F32)
        nc.vector.memset(qmv[:], 0.0)
        nc.vector.memset(vmv[:], 0.0)
        nc.vector.bn_aggr(out=qmv[:, 0, :], in_=qstats[:, 0, :, :])
        nc.vector.bn_aggr(out=qmv[:, 1, :], in_=qstats[:, 1, :, :])
        nc.vector.bn_aggr(out=vmv[:, 0, :], in_=vstats[:, 0, :, :])
        nc.vector.bn_aggr(out=vmv[:64, 1, :], in_=vstats[:64, 1, :, :])
        nc.sync.dma_start(out=qstats_out[:], in_=qmv[:])
        nc.sync.dma_start(out=vstats_out[:], in_=vmv[:])
        gA = statpool.tile([128, 192], F32)
        gB = statpool.tile([128, 192], F32)
        kssb = statpool.tile([128, 384], F32)
        nc.scalar.copy(out=gA[:], in_=gramA[:])
        nc.scalar.copy(out=gB[:], in_=gramB[:, 0:192])
        r0 = ONE_ROWS[0]
        nc.scalar.copy(out=kssb[r0:r0 + 1, :], in_=gramB[r0:r0 + 1, 0:384])
        nc.sync.dma_start(out=gramT_out[0:128, :], in_=gA[:])
        nc.sync.dma_start(out=gramT_out[128:256, :], in_=gB[:])
        nc.sync.dma_start(out=kss_out[:], in_=kssb[r0:r0 + 1, :])

    nc.compile()
    return nc


def build_l2(R=64, W=128):
    NS = R * W
    nc = bacc.Bacc("TRN2", target_bir_lowering=False, debug=False, num_devices=8)
    v_in = nc.dram_tensor("v_in", [192, NS], F8, kind="ExternalInput").ap()
    awT = nc.dram_tensor("awT", [128, 2, 192], F8, kind="ExternalInput").ap()
    pbias = nc.dram_tensor("pbias", [128, 2], F32, kind="ExternalInput").ap()
    out = nc.dram_tensor("out", [192, NS], BF16, kind="ExternalOutput").ap()

    with tile.TileContext(nc) as tc, ExitStack() as ctx:
        wpool = ctx.enter_context(tc.tile_pool(name="w", bufs=1))
        vpool = ctx.enter_context(tc.tile_pool(name="v", bufs=1))
        opool = ctx.enter_context(tc.tile_pool(name="o", bufs=1))
        pspool = ctx.enter_context(tc.tile_pool(name="ps", bufs=8, space="PSUM"))

        aw = wpool.tile([128, 2, 192], F8)
        nc.sync.dma_start(out=aw[:], in_=awT[:])
        pb = wpool.tile([128, 2], F32)
        nc.sync.dma_start(out=pb[:], in_=pbias[:])

        # batched: v lives in SBUF whole (fp8), loaded in chunks on 2 queues;
        # outputs accumulate in SBUF and leave in 2048-col chunks.
        vfull = vpool.tile([128, 2, NS], F8)
        ofull = opool.tile([128, 2, NS], BF16)
        nc.vector.memset(vfull[64:128, 1, :], 0.0)
        CL = NS // 4
        for j in range(4):
            sl = slice(j * CL, (j + 1) * CL)
            nc.sync.dma_start(out=vfull[:, 0, sl], in_=v_in[0:128, sl])
            nc.gpsimd.dma_start(out=vfull[:64, 1, sl], in_=v_in[128:192, sl])

        for c0 in range(0, NS, 512):
            for mb in range(2):
                msz = 128 if mb == 0 else 64
                ps = pspool.tile([128, 512], F32, tag="ps")
                nc.tensor.matmul(ps[:msz, :], aw[:, :, mb * 128:mb * 128 + msz],
                                 vfull[:, :, c0:c0 + 512], start=True, stop=True,
                                 perf_mode=DR)
                if (c0 // 512 + mb) % 2 == 0:
                    nc.scalar.activation(out=ofull[:msz, mb, c0:c0 + 512],
                                         in_=ps[:msz, :],
                                         func=mybir.ActivationFunctionType.Identity,
                                         bias=pb[:msz, mb:mb + 1],
                                         scale=1.0 / (AWS * VS))
                else:
                    nc.vector.tensor_scalar(
                        out=ofull[:msz, mb, c0:c0 + 512], in0=ps[:msz, :],
                        scalar1=1.0 / (AWS * VS), scalar2=pb[:msz, mb:mb + 1],
                        op0=mybir.AluOpType.mult, op1=mybir.AluOpType.add)
            if c0 % 2048 == 2048 - 512:
                b0 = c0 + 512 - 2048
                eng = nc.sync if (b0 // 2048) % 2 == 0 else nc.gpsimd
                eng.dma_start(out=out[0:128, b0:b0 + 2048], in_=ofull[:, 0, b0:b0 + 2048])
                eng.dma_start(out=out[128:192, b0:b0 + 2048], in_=ofull[:64, 1, b0:b0 + 2048])
    nc.compile()
    return nc


# ---------------- host-side prep ----------------

def prep_weights(w):
    """w: dict of reference weights (numpy f32). Returns dict of L1 input arrays."""
    out = {}
    qw = w["q_w"][:, :, 0, 0]          # (576, 192)
    qwT = np.zeros((128, 2, 768), np.float32)
    for (pb, h, x1b, qb, ch0, nch) in qkv_halves():
        win = 128 * pb + 64 * h
        qwT[0:128, 0, win:win + nch] = qw.T[0:128, ch0:ch0 + nch]
        qwT[0:64, 1, win:win + nch] = qw.T[128:192, ch0:ch0 + nch]
    out["qw8"] = (qwT * WS).astype(f8)

    # qdw: grouped 3x3, (tap, pb) 128x128 block; repack into DR tap pairs
    qdw = w["qdw_w"]                   # (576, 3, 3, 3)
    qdwT = np.zeros((9, 6, 128, 128), np.float32)   # (tap_sorted, pb, row, col)
    for ti, (dy, dx) in enumerate(TAPS):
        for (pb, h, x1b, qb, ch0, nch) in qkv_halves():
            for gl in range(nch // 3):
                for i in range(3):
                    for j in range(3):
                        qdwT[ti, pb, x1b + 3 * gl + i, qb + 3 * gl + j] = \
                            qdw[ch0 + 3 * gl + j, i, dy + 1, dx + 1]
    qdw8 = np.zeros((128, 6, 5, 2, 128), np.float32)
    for pj, (ta, tb) in enumerate(BD_PAIRS):
        for pb in range(6):
            qdw8[:, pb, pj, 0, :] = qdwT[ta, pb]
            if tb is not None:
                qdw8[:, pb, pj, 1, :] = qdwT[tb, pb]
    out["qdw8"] = (qdw8 * WS).astype(f8)

    # kv conv: 14 k-tiles -> 7 DR pairs (order must match CPAIRS in build_l1)
    kvw = w["kv_w"]                    # (384, 192, 3, 3)
    tl = lambda dy, dx: kvw[:, 0:128, dy + 1, dx + 1].T       # (128, 384) lo
    th = lambda dy, dx: kvw[:, 128:192, dy + 1, dx + 1].T     # (64, 384) hi
    ktiles = []
    for dy, dx in TAPS:   # 9 lo tiles (sorted tap order == TAPS order)
        ktiles.append(("lo", (dy, dx)))
    ktiles.append(("d1", -1)); ktiles.append(("d1", 0)); ktiles.append(("d1", 1))
    ktiles.append(("single", None)); ktiles.append(("d2", None))
    kv8 = np.zeros((128, 7, 2, 384), np.float32)
    for pj in range(7):
        for j in range(2):
            kind, arg = ktiles[2 * pj + j]
            blk = np.zeros((128, 384), np.float32)
            if kind == "lo":
                blk[0:128] = tl(*arg)
            elif kind == "d1":
                blk[0:64] = th(-1, arg)
                blk[64:128] = th(0, arg)
            elif kind == "d2":
                blk[0:64] = th(1, -1)
                blk[64:128] = th(1, 1)
            elif kind == "single":
                blk[0:64] = th(1, 0)
            kv8[:, pj, j, :] = blk
    out["kv8"] = (kv8 * WS).astype(f8)

    # kvdw depthwise: plain layout, diag blocks per pblock, DR tap pairs
    kvdw = w["kvdw_w"][:, 0]           # (384, 3, 3)
    kvdw8 = np.zeros((128, 3, 5, 2, 128), np.float32)
    for pj, (ta, tb) in enumerate(BD_PAIRS):
        for pb in range(3):
            dya, dxa = TAPS[ta]
            kvdw8[:, pb, pj, 0, :] = np.diag(kvdw[128 * pb:128 * pb + 128, dya + 1, dxa + 1])
            if tb is not None:
                dyb, dxb = TAPS[tb]
                kvdw8[:, pb, pj, 1, :] = np.diag(kvdw[128 * pb:128 * pb + 128, dyb + 1, dxb + 1])
    out["kvdw8"] = (kvdw8 * WS).astype(f8)

    def dw_channel(pb, p):
        return 128 * pb + p   # plain layout

    # newk (no bias on device; kT = NKS * k0): contraction sources
    # pass0 = (k8 slot0 = qkv pb2, k8 slot1 = qkv pb3)
    # pass1 = (dwk8 slot0 = dw pb0, dwk8 slot1 = dw pb1 parts 0..63)
    KCC = [("qkv", 2, 0, 128), ("qkv", 3, 0, 128), ("dw", 0, 0, 128), ("dw", 1, 0, 64)]
    nk = w["newk_w"][:, :, 0, 0]       # (192, 384): in = [k(192) | k_mask(192)]
    nkm = np.zeros((128, 4, 192), np.float32)
    for j, (src, pb, base, sz) in enumerate(KCC):
        for p in range(base, base + sz):
            if src == "qkv":
                ch = qkv_channel_at(pb, p)
                if ch is not None:
                    nkm[p, j, :] = nk[:, ch - 192]      # k part: qkv ch 192-383
            else:
                ch = dw_channel(pb, p)
                if ch < 192:
                    nkm[p, j, :] = nk[:, 192 + ch]      # k_mask: dw ch 0-191
    out["nkw8"] = (nkm.reshape(128, 2, 2, 192) * NKS).astype(f8)

    nv = w["newv_w"][:, :, 0, 0]       # (192, 384): in = [v(192) | v_mask(192)]
    nvT = np.zeros((128, 4, 192), np.float32)
    VCC_P = [("qkv", 4, 0, 128), ("qkv", 5, 0, 128), ("dw", 1, 64, 64), ("dw", 2, 0, 128)]
    for j, (src, pb, base, sz) in enumerate(VCC_P):
        for p in range(base, base + sz):
            if src == "qkv":
                ch = qkv_channel_at(pb, p)
                if ch is not None:
                    nvT[p, j, :] = nv[:, ch - 384]      # v part: qkv ch 384-575
            else:
                ch = dw_channel(pb, p)
                if ch >= 192:
                    nvT[p, j, :] = nv[:, ch]            # v_mask: dw ch 192-383
    out["nvw8"] = (nvT.reshape(128, 2, 2, 192) * NVS).astype(f8)

    out["ident"] = np.eye(128, dtype=bf16)

    x1b = np.zeros((128, 6), np.float32)
    qkvb = np.zeros((128, 6), np.float32)
    for (pb, h, x1b_base, qb, ch0, nch) in qkv_halves():
        x1b[x1b_base:x1b_base + nch, pb] = w["q_b"][ch0:ch0 + nch]
        qkvb[qb:qb + nch, pb] = w["qdw_b"][ch0:ch0 + nch]
    # hijacked ones rows: junk q pb1 partitions become the exact constant
    # 1/QTS, which the qT8 scale turns into 1.0 -> gramB rows = ksum/kss
    for r in ONE_ROWS:
        qkvb[r, 1] = 1.0 / QTS
    out["x1_bias"] = x1b * WS
    out["qkv_bias"] = qkvb

    kvb = np.zeros((128, 3), np.float32)
    dwb = np.zeros((128, 3), np.float32)
    for pb in range(3):
        kvb[:, pb] = w["kv_b"][128 * pb:128 * pb + 128]
        dwb[:, pb] = w["kvdw_b"][128 * pb:128 * pb + 128]
    out["kv_bias"] = kvb * WS
    out["kvdw_bias"] = dwb
    nvb = np.zeros((128, 2), np.float32)
    nvb[:, 0] = w["newv_b"][0:128]
    nvb[0:64, 1] = w["newv_b"][128:192]
    out["newv_bias"] = nvb * VS
    return out


def prep_core(x, xm, b, half, R, H):
    xp = np.zeros((192, R + 2, WP), np.float32)
    mp = np.zeros((192, R + 4, WP), np.float32)
    for r in range(R + 2):
        g = half * R + (r - 1)
        if 0 <= g < H:
            xp[:, r, 1:129] = x[b, :, g, :]
    for r in range(R + 4):
        g = half * R + (r - 2)
        if 0 <= g < H:
            mp[:, r, 1:129] = xm[b, :, g, :]
    xp = xp.reshape(192, -1)
    x8 = np.zeros((128, 2, xp.shape[1]), np.float32)
    x8[:, 0, :] = xp[0:128]
    x8[0:64, 1, :] = xp[128:192]
    x8[64:128, 1, :] = xp[128:192]   # dup (weights zero) to avoid NaN garbage
    mp = mp.reshape(192, -1)
    L = mp.shape[1]
    hi = mp[128:192]
    d1 = np.zeros((128, L), np.float32)
    d2 = np.zeros((128, L), np.float32)
    d1[0:64] = hi
    d1[64:128, :L - WP] = hi[:, WP:]
    d2[0:64] = hi
    d2[64:128, :L - 2] = hi[:, 2:]
    edge = np.ones((128, 2), np.float32)
    if half == 0:
        edge[:, 0] = 0.0     # slab0 row0 = image row -1
    else:
        edge[:, 1] = 0.0     # last slab last row = image row H
    return {
        "x8": x8.astype(f8),
        "xm_lo": mp[0:128].astype(f8),
        "xm_d1": d1.astype(f8), "xm_d2": d2.astype(f8),
        "edge": edge,
    }


# ---------------- host glue (unchanged semantics) ----------------

def _q_maps():
    part = np.zeros(192, np.int64)
    pblk = np.zeros(192, np.int64)
    for (pb, h, x1b, qb, ch0, nch) in qkv_halves():
        if pb >= 2:
            continue
        for i in range(nch):
            pblk[ch0 + i] = pb
            part[ch0 + i] = qb + i
    return pblk, part


def _ss_from_qstats(stats, n_half):
    pblk, part = _q_maps()
    mv = stats.astype(np.float64)
    return (mv[part, pblk, 1] + mv[part, pblk, 0] ** 2) * n_half


def _sum_from_qstats(stats, n_half):
    pblk, part = _q_maps()
    mv = stats.astype(np.float64)
    return mv[part, pblk, 0] * n_half


def _ss_from_vstats(stats, newv_b, n_half):
    # stats are of NVS*(v - b): undo the scale, add back the bias
    m = np.zeros(192, np.float64)
    w = np.zeros(192, np.float64)
    mv = stats.astype(np.float64)
    m[0:128] = mv[0:128, 0, 0]
    w[0:128] = mv[0:128, 0, 1]
    m[128:192] = mv[0:64, 1, 0]
    w[128:192] = mv[0:64, 1, 1]
    m = m / NVS + newv_b.astype(np.float64)
    w = w / (NVS * NVS)
    return (w + m * m) * n_half


def glue(res0, res1, newv_b, n_half):
    """Combine two half-core L1 results into raw sums."""
    GT = res0["gramT_out"].astype(np.float64) + res1["gramT_out"].astype(np.float64)
    pblk, part = _q_maps()
    qrow = pblk * 128 + part
    # GT rows are qT cols (pb*128+part); cols are newk out-ch d. Stored values
    # are sum_n (QTS*q) * (NKS*k0).
    Gq = GT[qrow, :] / (QTS * NKS)             # (c, d): sum_n q[c,n] k0[d,n]
    qss = _ss_from_qstats(res0["qstats_out"], n_half) + _ss_from_qstats(res1["qstats_out"], n_half)
    qsum = _sum_from_qstats(res0["qstats_out"], n_half) + _sum_from_qstats(res1["qstats_out"], n_half)
    vss = _ss_from_vstats(res0["vstats_out"], newv_b, n_half) + \
        _ss_from_vstats(res1["vstats_out"], newv_b, n_half)
    kss_raw = (res0["kss_out"].astype(np.float64) + res1["kss_out"].astype(np.float64))[0]
    k0sum = kss_raw[0:192] / NKS               # sum_n k0 (ones x kT cols)
    k0ss = kss_raw[192:384] / (KSQS * KSQS)    # sum_n k0^2 (ones x ksq cols)
    return Gq, qss, qsum, vss, k0ss, k0sum


def glue_full(res0, res1, temperature, newk_b, newv_b, proj_w, proj_b, n_half):
    Gq, qss, qsum, vss, k0ss, k0sum = glue(res0, res1, newv_b, n_half)
    b = newk_b.astype(np.float64)              # (192,)
    # k = k0 + b: gram/kss bias corrections (sums already cover both halves,
    # total n = 2 * n_half)
    G = Gq + qsum[:, None] * b[None, :]        # (c, d): sum_n q k
    kss = k0ss + 2 * b * k0sum + (2 * n_half) * b * b
    qn = np.maximum(np.sqrt(qss), 1e-12)
    kn = np.maximum(np.sqrt(kss), 1e-12)
    vn = np.maximum(np.sqrt(vss), 1e-12)
    A = G / (qn[:, None] * kn[None, :])        # (c, d)
    M = np.zeros((192, 192), np.float64)
    t = np.asarray(temperature).reshape(-1)
    for h in range(8):
        sl = slice(24 * h, 24 * h + 24)
        a = A[sl, sl] * t[h]
        a = a - a.max(axis=-1, keepdims=True)
        e = np.exp(a)
        sm = e / e.sum(axis=-1, keepdims=True)
        M[sl, sl] = sm / vn[None, sl]
    At = proj_w[:, :, 0, 0].astype(np.float64) @ M   # (out-ch o, d)
    awT = np.zeros((128, 2, 192), np.float32)
    awT[:, 0, :] = At.T[0:128] * AWS
    awT[0:64, 1, :] = At.T[128:192] * AWS
    pbias = np.zeros((128, 2), np.float32)
    pbias[:, 0] = proj_b[0:128]
    pbias[0:64, 1] = proj_b[128:192]
    return {"awT": awT.astype(f8), "pbias": pbias}


# ---------------- driver: kernel(**inputs) ----------------
from concourse.bass_utils import run_bass_kernel_spmd

R_FULL, H_FULL, B_FULL = 64, 128, 4
_NC1 = None
_NC2 = None


def _get_progs():
    global _NC1, _NC2
    if _NC1 is None:
        _NC1 = build_l1(R=R_FULL, S=16)
        _NC2 = build_l2(R=R_FULL)
    return _NC1, _NC2


def kernel(**inputs):
    inputs = {k: np.asarray(v) for k, v in inputs.items()}
    x, xm = inputs["x"], inputs["x_mask"]
    nc1, nc2 = _get_progs()
    wprep = prep_weights(inputs)
    in_maps = []
    for core in range(8):
        b, half = core // 2, core % 2
        m = dict(wprep)
        m.update(prep_core(x, xm, b, half, R_FULL, H_FULL))
        in_maps.append(m)
    res1 = run_bass_kernel_spmd(nc1, in_maps, list(range(8))).results

    n_half = R_FULL * 128
    in_maps2 = []
    for core in range(8):
        b, half = core // 2, core % 2
        if half == 0:
            l2c = glue_full(res1[2 * b], res1[2 * b + 1], inputs["temperature"],
                            inputs["newk_b"], inputs["newv_b"],
                            inputs["proj_w"], inputs["proj_b"], n_half)
        m = dict(l2c)
        m["v_in"] = res1[core]["v_out"]
        in_maps2.append(m)
    res2 = run_bass_kernel_spmd(nc2, in_maps2, list(range(8))).results

    out = np.empty((B_FULL, 192, H_FULL, 128), np.float32)
    for core in range(8):
        b, half = core // 2, core % 2
        out[b, :, half * R_FULL:(half + 1) * R_FULL, :] = \
            res2[core]["out"].reshape(192, R_FULL, 128).astype(np.float32)
    return out


# revision 41
# speedup vs baseline: 1.1326x; 1.0253x over previous
"""Trainium2 Bass kernel for nn_Cross_Attention — fp8 DoubleRow rewrite.

L1: convs + gram partials with fp8 DoubleRow matmuls (phases A-D), fp8 F-phase
(newk/gram/kss via DR), bf16 E. L2: attn-apply with chunked v DMA. Host glue
between (softmax etc.). Sharding: 4 samples x 2 row-halves across 8 cores.
"""
import sys
sys.path.insert(0, "/opt/trn_rl_repo")
import numpy as np
import ml_dtypes

import concourse.bass as bass
import concourse.tile as tile
from concourse import bacc, mybir
from contextlib import ExitStack

BF16 = mybir.dt.bfloat16
F8 = mybir.dt.float8e4
F32 = mybir.dt.float32
bf16 = ml_dtypes.bfloat16
f8 = ml_dtypes.float8_e4m3
DR = mybir.MatmulPerfMode.DoubleRow

WS = 16.0          # fp8 weight pre-scale
WP = 130           # padded row width
NKS = 32.0         # newk weight pre-scale (kT stored as NKS*k0)
QTS = 16.0         # qT pre-scale (qT stored as QTS*q)
KSQS = 64.0        # ksq stored as (KSQS*k0)**2
NVS = 32.0         # newv weight pre-scale
VS = 8.0           # v8 storage scale (v stored as VS*v)
AWS = 512.0        # attn-weight (aw8) pre-scale
ONE_ROWS = (32, 33)  # junk q pb1 partitions hijacked as 1.0 cols in qT8 (ksum/kss);
                     # must be 32-aligned (engine partition-base alignment)

TAPS = [(dy, dx) for dy in (-1, 0, 1) for dx in (-1, 0, 1)]
# taps sorted by flat offset dy*WP+dx (they already are, given WP>2)
TAP_OFF = [dy * WP + dx for dy, dx in TAPS]
# DoubleRow tap pairs for B/D: (idx_a, idx_b). The odd tap rides first with a
# zero-weight second tile at stride +1 (always in-bounds since tap 0 has the
# smallest offset).
BD_PAIRS = [(0, None), (1, 2), (3, 4), (5, 6), (7, 8)]


def qkv_halves():
    """Per (pb, half): (x1_base, qkv_base, ch0, nch).  ch0 = qkv-global channel."""
    out = []
    for pb in range(6):
        P, odd = pb // 2, pb % 2
        for h in (0, 1):
            nch = 3 if (odd and h == 1) else 63
            ch0 = 3 * (64 * P + 42 * odd + 21 * h)
            x1b = 64 * h
            qb = 64 * h if not odd else 64 * (1 - h)
            out.append((pb, h, x1b, qb, ch0, nch))
    return out


def qkv_channel_at(pb, p):
    """qkv-global channel stored at partition p of qkv pblock pb, or None."""
    for (pb2, h, x1b, qb, ch0, nch) in qkv_halves():
        if pb2 == pb and qb <= p < qb + nch:
            return ch0 + (p - qb)
    return None


# newk/newv input chunks with PLAIN dw layout (dw pb_i = kv ch 128i..128i+127):
# k = qkv ch 192-383 (pb 2,3); k_mask = kv ch 0-191 = dw pb0 + dw pb1[0:64]
# v = qkv ch 384-575 (pb 4,5); v_mask = kv ch 192-383 = dw pb1[64:128] + dw pb2
# Tile routing: q_t = qkv pb0,1 (bf16); k8 = qkv pb2,3 (f8, chunk-major);
# vq = qkv pb4,5 (f8); dwk8 = dw pb0 + dw pb1[0:64] (f8, chunk-major);
# dwv = dw pb1[64:128] + dw pb2 (f8, slot0 parts 64-127 used).


def _bcast(ap, p):
    return bass.AP(tensor=ap.tensor, offset=ap.offset, ap=[[0, p]] + list(ap.ap[1:]))


def _pair_ap(t, off, delta, dims):
    """AP over tile t at flat free-offset `off`: [partitions, 2 (stride delta), *dims]."""
    return bass.AP(tensor=t.tensor, offset=t.offset + off,
                   ap=[list(t.ap[0]), [delta, 2]] + [list(d) for d in dims])


def build_l1(R=64, W=128, S=16):
    assert W == 128 and R % S == 0 and S % 4 == 0
    NSLAB = R // S
    NS = S * W
    XCOLS = (S + 2) * WP
    MCOLS = (S + 4) * WP
    MC2 = MCOLS + 2
    N128 = NS // 128

    nc = bacc.Bacc("TRN2", target_bir_lowering=False, debug=False, num_devices=8)

    def din(name, shape, dt=F8):
        return nc.dram_tensor(name, shape, dt, kind="ExternalInput").ap()

    def dout(name, shape, dt=F32):
        return nc.dram_tensor(name, shape, dt, kind="ExternalOutput").ap()

    x8 = din("x8", [128, 2, (R + 2) * WP])
    xm_lo = din("xm_lo", [128, (R + 4) * WP])
    xm_d1 = din("xm_d1", [128, (R + 4) * WP])
    xm_d2 = din("xm_d2", [128, (R + 4) * WP])
    qw8 = din("qw8", [128, 2, 768])
    qdw8 = din("qdw8", [128, 6, 5, 2, 128])
    kv8 = din("kv8", [128, 7, 2, 384])
    kvdw8 = din("kvdw8", [128, 3, 5, 2, 128])
    nkw8 = din("nkw8", [128, 2, 2, 192], F8)
    nvw8 = din("nvw8", [128, 2, 2, 192], F8)
    ident = din("ident", [128, 128], BF16)
    x1_bias = din("x1_bias", [128, 6], F32)      # x16
    qkv_bias = din("qkv_bias", [128, 6], F32)
    kv_bias = din("kv_bias", [128, 3], F32)      # x16, plain layout
    kvdw_bias = din("kvdw_bias", [128, 3], F32)  # plain layout
    newv_bias = din("newv_bias", [128, 2], F32)  # x VS
    edge = din("edge", [128, 2], F32)            # per-core edge-row multipliers

    v_out = dout("v_out", [192, R * W], F8)
    gramT_out = dout("gramT_out", [256, 192])    # rows = qcol space (pb*128+part)
    kss_out = dout("kss_out", [1, 384])          # [sum (KSQS*k0)^2 | sum NKS*k0]
    qstats_out = dout("qstats_out", [128, 2, 2])
    vstats_out = dout("vstats_out", [128, 2, 2])

    # C-phase DoubleRow k-tile pair coordinates in the [128, 3, MC2] msl tile
    # flat free space: region*MC2 + off.  Region 0 = lo channels, 1 = d1
    # (hi | hi<<WP), 2 = d2 (hi | hi<<2).
    def lo(dy, dx):
        return (1 + dy) * WP + dx + 1

    def d1(off):
        return MC2 + off

    def d2(off):
        return 2 * MC2 + off

    # 14 k-tiles -> 7 pairs; weight slot kv8[:, pair, j, :] must match.
    CPAIRS = [
        (lo(-1, -1), lo(-1, 0)), (lo(-1, 1), lo(0, -1)), (lo(0, 0), lo(0, 1)),
        (lo(1, -1), lo(1, 0)), (lo(1, 1), d1(0)), (d1(1), d1(2)),
        (d1(2 * WP + 1), d2(2 * WP)),
    ]

    with tile.TileContext(nc) as tc, ExitStack() as ctx:
        wpool = ctx.enter_context(tc.tile_pool(name="weights", bufs=1))
        xpool = ctx.enter_context(tc.tile_pool(name="xslab", bufs=2))
        bigpool = ctx.enter_context(tc.tile_pool(name="big", bufs=1))
        midpool = ctx.enter_context(tc.tile_pool(name="mid", bufs=2))
        smpool = ctx.enter_context(tc.tile_pool(name="small", bufs=4))
        statpool = ctx.enter_context(tc.tile_pool(name="stats", bufs=1))
        pspool = ctx.enter_context(tc.tile_pool(name="ps", bufs=6, space="PSUM"))
        pspers = ctx.enter_context(tc.tile_pool(name="pspers", bufs=1, space="PSUM"))

        def load1(ap_in, shape, dt=F8, eng=None):
            t = wpool.tile(shape, dt, tag=ap_in.tensor.name)
            (eng or nc.sync).dma_start(out=t[:ap_in.shape[0]], in_=ap_in[:])
            return t

        g = nc.gpsimd
        qw8_s = load1(qw8, [128, 2, 768])
        x1b_s = load1(x1_bias, [128, 6], F32)
        qdw8_s = load1(qdw8, [128, 6, 5, 2, 128], eng=g)
        kv8_s = load1(kv8, [128, 7, 2, 384], eng=g)
        kvdw8_s = load1(kvdw8, [128, 3, 5, 2, 128], eng=g)
        nkw8_s = load1(nkw8, [128, 2, 2, 192], F8, eng=g)
        nvw8_s = load1(nvw8, [128, 2, 2, 192], F8, eng=g)
        id_s = load1(ident, [128, 128], BF16, eng=g)
        qkvb_s = load1(qkv_bias, [128, 6], F32, eng=g)
        kvb_s = load1(kv_bias, [128, 3], F32, eng=g)
        dwb_s = load1(kvdw_bias, [128, 3], F32, eng=g)
        nvb_s = load1(newv_bias, [128, 2], F32, eng=g)
        edge_s = load1(edge, [128, 2], F32, eng=g)

        # persistent accumulators: one PSUM bank each (zero-region granularity)
        # gramB is 384 wide: cols 192:384 see the ksq half of the moving data;
        # its ONE_ROWS rows (ones in the stationary) yield ksum/kss.
        gramA = pspers.tile([128, 192], F32)
        gramB = pspers.tile([128, 384], F32)

        qstats = statpool.tile([128, 2, NSLAB * (NS // 512), 6], F32)
        vstats = statpool.tile([128, 2, NSLAB * (NS // 512), 6], F32)
        # double-buffered chunk-pair staging for gram/kss (dims: pairbuf, slot).
        # inner dim padded to 400 so the DR pair stride is NOT contiguous with
        # the column dim (contiguous dims get re-flattened in lowering, which
        # breaks the pair interpretation of the moving AP).
        kq8 = statpool.tile([128, 2, 2, 400], F8)    # [0:192]=NKS*k0, [192:384]=ksq
        qT8 = statpool.tile([128, 2, 2, 256], F8)

        n128_total = NSLAB * N128
        CT = [(c0, min(512, XCOLS - c0)) for c0 in range(0, XCOLS, 512)]
        # slabs > 0 reuse the previous slab's last 2 rows of x1/kv1 (halo
        # copy) and only compute the remaining S rows
        CT2 = [(2 * WP + 416 * k, 416) for k in range(5)]
        x1 = bigpool.tile([128, 6, XCOLS], F8, tag="x1")
        kv1 = bigpool.tile([128, 3, XCOLS], F8, tag="kv1")

        for s in range(NSLAB):
            xsl = xpool.tile([128, 2, XCOLS], F8, tag="xsl")
            msl = xpool.tile([128, 3, MC2], F8, tag="msl")
            off = s * S * WP
            # spread the slab input loads over several DMA queues
            nc.sync.dma_start(out=xsl[:], in_=x8[:, :, off:off + XCOLS])
            nc.scalar.dma_start(out=msl[:, 0, 1:1 + MCOLS], in_=xm_lo[:, off:off + MCOLS])
            nc.gpsimd.dma_start(out=msl[:, 1, 1:1 + MCOLS], in_=xm_d1[:, off:off + MCOLS])
            nc.scalar.dma_start(out=msl[:, 2, 1:1 + MCOLS], in_=xm_d2[:, off:off + MCOLS])
            for r in range(3):
                nc.vector.memset(msl[:, r, 0:1], 0.0)
                nc.vector.memset(msl[:, r, MC2 - 1:MC2], 0.0)

            def pad_zero(t, npb):
                # zero the 2 pad columns of every row: flat r*WP + {0, 129}
                ap = bass.AP(tensor=t.tensor, offset=t.offset,
                             ap=[list(t.ap[0]), [XCOLS, npb], [WP, S + 2], [WP - 1, 2]])
                nc.vector.memset(ap, 0.0)
                # per-core edge pad row: slab0 row0 (half 0) / slab3 last row
                # (half 1), selected by the `edge` input multipliers
                if s == 0:
                    nc.vector.tensor_scalar(
                        out=t[:, :, 0:WP], in0=t[:, :, 0:WP],
                        scalar1=edge_s[:, 0:1], scalar2=None,
                        op0=mybir.AluOpType.mult)
                if s == NSLAB - 1:
                    nc.vector.tensor_scalar(
                        out=t[:, :, (S + 1) * WP:(S + 2) * WP],
                        in0=t[:, :, (S + 1) * WP:(S + 2) * WP],
                        scalar1=edge_s[:, 1:2], scalar2=None,
                        op0=mybir.AluOpType.mult)

            # ---- Phase A: x1 = 1x1(x); one DR matmul per (pb, col-tile)
            if s > 0:
                nc.sync.dma_start(out=x1[:, :, 0:2 * WP],
                                  in_=x1[:, :, S * WP:(S + 2) * WP])
                nc.gpsimd.dma_start(out=kv1[:, :, 0:2 * WP],
                                    in_=kv1[:, :, S * WP:(S + 2) * WP])
            for pb in range(6):
                for c0, cs in (CT if s == 0 else CT2):
                    ps = pspool.tile([128, 512], F32, tag="ps", name=f"psA{s}_{pb}_{c0}")
                    nc.tensor.matmul(ps[:, :cs], qw8_s[:, :, 128 * pb:128 * pb + 128],
                                     xsl[:, :, c0:c0 + cs], start=True, stop=True,
                                     perf_mode=DR)
                    nc.vector.tensor_scalar(
                        out=x1[:, pb, c0:c0 + cs], in0=ps[:, :cs],
                        scalar1=x1b_s[:, pb:pb + 1], scalar2=1.0 / WS,
                        op0=mybir.AluOpType.add, op1=mybir.AluOpType.mult)
            pad_zero(x1, 6)

            # ---- Phase C: kv1 = 3x3(xm); 7 DR pairs per (pb, col-tile)
            for pb in range(3):
                for cset in ((CT[:3], CT[3:]) if s == 0 else (CT2[:3], CT2[3:])):
                    pss = [pspool.tile([128, 512], F32, tag="ps",
                                       name=f"psC{s}_{pb}_{c0}")
                           for (c0, cs) in cset]
                    for pj, (ca, cb) in enumerate(CPAIRS):
                        lhsT = kv8_s[:, pj, :, 128 * pb:128 * pb + 128]
                        for ti, (c0, cs) in enumerate(cset):
                            rhs = _pair_ap(msl, ca + c0, cb - ca, [[1, cs]])
                            nc.tensor.matmul(pss[ti][:, :cs], lhsT, rhs,
                                             start=(pj == 0), stop=(pj == 6),
                                             perf_mode=DR)
                    for ti, (c0, cs) in enumerate(cset):
                        nc.vector.tensor_scalar(
                            out=kv1[:, pb, c0:c0 + cs], in0=pss[ti][:, :cs],
                            scalar1=kvb_s[:, pb:pb + 1], scalar2=1.0 / WS,
                            op0=mybir.AluOpType.add, op1=mybir.AluOpType.mult)
            pad_zero(kv1, 3)

            # ---- Phase B: qkv = qdw(x1); 2-row groups (N=258), 5 DR tap-pairs
            # per group chained in PSUM; 4-group sets amortize weight loads.
            # route: pb -> list of (dst_tile, slot, part_lo, part_hi)
            def dwconv(src, wsrc, route, npb, bias_s, tag):
                for pb in range(npb):
                    for st in range(2):
                        gset = list(range(st * 4, st * 4 + 4))
                        pss = {gi: pspool.tile([128, 258], F32, tag="ps",
                                               name=f"ps{tag}{s}_{pb}_{gi}")
                               for gi in gset}
                        for pj, (ta, tb) in enumerate(BD_PAIRS):
                            oa = TAP_OFF[ta]
                            delta = (TAP_OFF[tb] - oa) if tb is not None else 1
                            lhsT = wsrc[:, pb, pj, :, :]
                            for gi in gset:
                                base = pb * XCOLS + (2 * gi + 1) * WP + 1 + oa
                                rhs = _pair_ap(src, base, delta, [[1, 258]])
                                nc.tensor.matmul(pss[gi][:, :], lhsT, rhs,
                                                 start=(pj == 0), stop=(pj == 4),
                                                 perf_mode=DR)
                        for gi in gset:
                            ps = pss[gi]
                            in_ap = bass.AP(tensor=ps.tensor, offset=ps.offset,
                                            ap=[list(ps.ap[0]), [WP, 2], [1, 128]])
                            for (dst, slot, plo, phi, chunked) in route(pb):
                                if chunked:
                                    out_ap = dst[plo:phi, 2 * gi:2 * gi + 2, slot, :]
                                else:
                                    out_ap = dst[plo:phi, slot,
                                                 2 * gi * 128:(2 * gi + 2) * 128]
                                nc.scalar.activation(
                                    out=out_ap,
                                    in_=in_ap[plo:phi],
                                    func=mybir.ActivationFunctionType.Identity,
                                    bias=bias_s[plo:phi, pb:pb + 1], scale=1.0 / WS)

            # k8/dwk8 are chunk-major [128, N128, 2, 128] so the kps DR
            # stationary pair is contiguous (pair stride 128 — ISA requires
            # small pair strides for dual-fp8 ldweights).
            q_t = bigpool.tile([128, 2, NS], BF16, tag="q_t")
            k8 = bigpool.tile([128, N128, 2, 128], F8, tag="k8")
            vq = bigpool.tile([128, 2, NS], F8, tag="vq")

            def qkv_route(pb):
                dst = (q_t, k8, vq)[pb // 2]
                return [(dst, pb % 2, 0, 128, pb // 2 == 1)]

            dwconv(x1, qdw8_s, qkv_route, 6, qkvb_s, "B")

            # ---- Phase D: dw = kvdw(kv1); same structure, 3 pblocks
            dwk8 = bigpool.tile([128, N128, 2, 128], F8, tag="dwk8")
            dwv = bigpool.tile([128, 2, NS], F8, tag="dwv")
            nc.vector.memset(dwk8[64:128, :, 1, :], 0.0)
            nc.vector.memset(dwv[0:64, 0, :], 0.0)

            def dw_route(pb):
                if pb == 0:
                    return [(dwk8, 0, 0, 128, True)]
                if pb == 1:
                    return [(dwk8, 1, 0, 64, True), (dwv, 0, 64, 128, False)]
                return [(dwv, 1, 0, 128, False)]

            dwconv(kv1, kvdw8_s, dw_route, 3, dwb_s, "D")

            # ---- Phase E: v = newv(v_cc), fp8 DR (pair over the two source
            # tiles of each pass); psum holds NVS*(v - b); bn_stats on psum.
            vt = midpool.tile([128, 2, NS], F8, tag="vt")
            for mb in range(2):
                msz = 128 if mb == 0 else 64
                for ic in range(NS // 512):
                    sl = slice(ic * 512, (ic + 1) * 512)
                    ps = pspool.tile([128, 512], F32, tag="ps",
                                     name=f"psE{s}_{mb}_{ic}")
                    nc.tensor.matmul(ps[:msz, :],
                                     nvw8_s[:, 0, :, mb * 128:mb * 128 + msz],
                                     vq[:, :, sl], start=True, stop=False,
                                     perf_mode=DR)
                    nc.tensor.matmul(ps[:msz, :],
                                     nvw8_s[:, 1, :, mb * 128:mb * 128 + msz],
                                     dwv[:, :, sl], start=False, stop=True,
                                     perf_mode=DR)
                    si = s * (NS // 512) + ic
                    nc.vector.bn_stats(out=vstats[:msz, mb, si, :], in_=ps[:msz, :])
                    nc.scalar.activation(out=vt[:msz, mb, sl], in_=ps[:msz, :],
                                         func=mybir.ActivationFunctionType.Identity,
                                         bias=nvb_s[:msz, mb:mb + 1], scale=VS / NVS)
            nc.sync.dma_start(out=v_out[0:128, s * NS:(s + 1) * NS], in_=vt[:, 0, :])
            nc.sync.dma_start(out=v_out[128:192, s * NS:(s + 1) * NS], in_=vt[:64, 1, :])
            for sub in range(NS // 512):
                si = s * (NS // 512) + sub
                sl = slice(sub * 512, (sub + 1) * 512)
                nc.vector.bn_stats(out=qstats[:, 0, si, :], in_=q_t[:, 0, sl])
                nc.vector.bn_stats(out=qstats[:, 1, si, :], in_=q_t[:, 1, sl])

            # ---- Phase F: per 128-n chunk: kT (newk, fp8 DR), qT (transpose),
            # then per chunk-pair: gram (fp8 DR) + kss.  Gram for pair P is
            # emitted one chunk late (during chunk 2P+2) so the scalar-engine
            # kq8/qT8 writes are done before the PE needs them.
            def emit_gram(gp):
                pb = gp % 2
                first, last = gp == 0, gp == n128_total // 2 - 1
                nc.tensor.matmul(gramA[:, :], qT8[:, pb, :, 0:128],
                                 kq8[:, pb, :, 0:192], start=first, stop=last,
                                 perf_mode=DR)
                nc.tensor.matmul(gramB[:, :], qT8[:, pb, :, 128:256],
                                 kq8[:, pb, :, 0:384], start=first, stop=last,
                                 perf_mode=DR)

            for ic in range(N128):
                c0 = ic * 128
                gidx = s * N128 + ic
                slot = ic % 2
                pbuf = (ic // 2) % 2
                kps = pspool.tile([128, 192], F32, tag="ps", name=f"kps{s}_{ic}")
                nc.tensor.matmul(kps[:, :], k8[:, ic, :, :], nkw8_s[:, 0, :, :],
                                 start=True, stop=False, perf_mode=DR)
                nc.tensor.matmul(kps[:, :], dwk8[:, ic, :, :], nkw8_s[:, 1, :, :],
                                 start=False, stop=True, perf_mode=DR)
                nc.scalar.copy(out=kq8[:, pbuf, slot, 0:192], in_=kps[:, :])
                nc.scalar.activation(out=kq8[:, pbuf, slot, 192:384], in_=kps[:, :],
                                     func=mybir.ActivationFunctionType.Square,
                                     scale=KSQS / NKS)
                qps = pspool.tile([128, 256], BF16, tag="ps", name=f"qps{s}_{ic}")
                nc.tensor.transpose(qps[:, 0:128], q_t[:, 0, c0:c0 + 128], id_s[:, :])
                nc.tensor.transpose(qps[:, 128:256], q_t[:, 1, c0:c0 + 128], id_s[:, :])
                nc.scalar.mul(out=qT8[:, pbuf, slot, :], in_=qps[:, :], mul=QTS)
                if slot == 0 and gidx >= 2:
                    emit_gram(gidx // 2 - 1)

        emit_gram(n128_total // 2 - 1)

        qmv = statpool.tile([128, 2, 2], F32)
        vmv = statpool.tile([128, 2, 2], F32)
        nc.vector.memset(qmv[:], 0.0)
        nc.vector.memset(vmv[:], 0.0)
        nc.vector.bn_aggr(out=qmv[:, 0, :], in_=qstats[:, 0, :, :])
        nc.vector.bn_aggr(out=qmv[:, 1, :], in_=qstats[:, 1, :, :])
        nc.vector.bn_aggr(out=vmv[:, 0, :], in_=vstats[:, 0, :, :])
        nc.vector.bn_aggr(out=vmv[:64, 1, :], in_=vstats[:64, 1, :, :])
        nc.sync.dma_start(out=qstats_out[:], in_=qmv[:])
        nc.sync.dma_start(out=vstats_out[:], in_=vmv[:])
        gA = statpool.tile([128, 192], F32)
        gB = statpool.tile([128, 192], F32)
        kssb = statpool.tile([128, 384], F32)
        nc.scalar.copy(out=gA[:], in_=gramA[:])
        nc.scalar.copy(out=gB[:], in_=gramB[:, 0:192])
        r0 = ONE_ROWS[0]
        nc.scalar.copy(out=kssb[r0:r0 + 1, :], in_=gramB[r0:r0 + 1, 0:384])
        nc.sync.dma_start(out=gramT_out[0:128, :], in_=gA[:])
        nc.sync.dma_start(out=gramT_out[128:256, :], in_=gB[:])
        nc.sync.dma_start(out=kss_out[:], in_=kssb[r0:r0 + 1, :])

    nc.compile()
    return nc


def build_l2(R=64, W=128):
    NS = R * W
    nc = bacc.Bacc("TRN2", target_bir_lowering=False, debug=False, num_devices=8)
    v_in = nc.dram_tensor("v_in", [192, NS], F8, kind="ExternalInput").ap()
    awT = nc.dram_tensor("awT", [128, 2, 192], F8, kind="ExternalInput").ap()
    pbias = nc.dram_tensor("pbias", [128, 2], F32, kind="ExternalInput").ap()
    out = nc.dram_tensor("out", [192, NS], BF16, kind="ExternalOutput").ap()

    with tile.TileContext(nc) as tc, ExitStack() as ctx:
        wpool = ctx.enter_context(tc.tile_pool(name="w", bufs=1))
        vpool = ctx.enter_context(tc.tile_pool(name="v", bufs=1))
        opool = ctx.enter_context(tc.tile_pool(name="o", bufs=1))
        pspool = ctx.enter_context(tc.tile_pool(name="ps", bufs=8, space="PSUM"))

        aw = wpool.tile([128, 2, 192], F8)
        nc.sync.dma_start(out=aw[:], in_=awT[:])
        pb = wpool.tile([128, 2], F32)
        nc.sync.dma_start(out=pb[:], in_=pbias[:])

        # batched: v lives in SBUF whole (fp8), loaded in chunks on 2 queues;
        # outputs accumulate in SBUF and leave in 2048-col chunks.
        vfull = vpool.tile([128, 2, NS], F8)
        ofull = opool.tile([128, 2, NS], BF16)
        nc.vector.memset(vfull[64:128, 1, :], 0.0)
        CL = NS // 4
        for j in range(4):
            sl = slice(j * CL, (j + 1) * CL)
            nc.sync.dma_start(out=vfull[:, 0, sl], in_=v_in[0:128, sl])
            nc.gpsimd.dma_start(out=vfull[:64, 1, sl], in_=v_in[128:192, sl])

        for c0 in range(0, NS, 512):
            for mb in range(2):
                msz = 128 if mb == 0 else 64
                ps = pspool.tile([128, 512], F32, tag="ps")
                nc.tensor.matmul(ps[:msz, :], aw[:, :, mb * 128:mb * 128 + msz],
                                 vfull[:, :, c0:c0 + 512], start=True, stop=True,
                                 perf_mode=DR)
                if (c0 // 512 + mb) % 2 == 0:
                    nc.scalar.activation(out=ofull[:msz, mb, c0:c0 + 512],
                                         in_=ps[:msz, :],
                                         func=mybir.ActivationFunctionType.Identity,
                                         bias=pb[:msz, mb:mb + 1],
                                         scale=1.0 / (AWS * VS))
                else:
                    nc.vector.tensor_scalar(
                        out=ofull[:msz, mb, c0:c0 + 512], in0=ps[:msz, :],
                        scalar1=1.0 / (AWS * VS), scalar2=pb[:msz, mb:mb + 1],
                        op0=mybir.AluOpType.mult, op1=mybir.AluOpType.add)
            if c0 % 2048 == 2048 - 512:
                b0 = c0 + 512 - 2048
                eng = nc.sync if (b0 // 2048) % 2 == 0 else nc.gpsimd
                eng.dma_start(out=out[0:128, b0:b0 + 2048], in_=ofull[:, 0, b0:b0 + 2048])
                eng.dma_start(out=out[128:192, b0:b0 + 2048], in_=ofull[:64, 1, b0:b0 + 2048])
    nc.compile()
    return nc


# ---------------- host-side prep ----------------

def prep_weights(w):
    """w: dict of reference weights (numpy f32). Returns dict of L1 input arrays."""
    out = {}
    qw = w["q_w"][:, :, 0, 0]          # (576, 192)
    qwT = np.zeros((128, 2, 768), np.float32)
    for (pb, h, x1b, qb, ch0, nch) in qkv_halves():
        win = 128 * pb + 64 * h
        qwT[0:128, 0, win:win + nch] = qw.T[0:128, ch0:ch0 + nch]
        qwT[0:64, 1, win:win + nch] = qw.T[128:192, ch0:ch0 + nch]
    out["qw8"] = (qwT * WS).astype(f8)

    # qdw: grouped 3x3, (tap, pb) 128x128 block; repack into DR tap pairs
    qdw = w["qdw_w"]                   # (576, 3, 3, 3)
    qdwT = np.zeros((9, 6, 128, 128), np.float32)   # (tap_sorted, pb, row, col)
    for ti, (dy, dx) in enumerate(TAPS):
        for (pb, h, x1b, qb, ch0, nch) in qkv_halves():
            for gl in range(nch // 3):
                for i in range(3):
                    for j in range(3):
                        qdwT[ti, pb, x1b + 3 * gl + i, qb + 3 * gl + j] = \
                            qdw[ch0 + 3 * gl + j, i, dy + 1, dx + 1]
    qdw8 = np.zeros((128, 6, 5, 2, 128), np.float32)
    for pj, (ta, tb) in enumerate(BD_PAIRS):
        for pb in range(6):
            qdw8[:, pb, pj, 0, :] = qdwT[ta, pb]
            if tb is not None:
                qdw8[:, pb, pj, 1, :] = qdwT[tb, pb]
    out["qdw8"] = (qdw8 * WS).astype(f8)

    # kv conv: 14 k-tiles -> 7 DR pairs (order must match CPAIRS in build_l1)
    kvw = w["kv_w"]                    # (384, 192, 3, 3)
    tl = lambda dy, dx: kvw[:, 0:128, dy + 1, dx + 1].T       # (128, 384) lo
    th = lambda dy, dx: kvw[:, 128:192, dy + 1, dx + 1].T     # (64, 384) hi
    ktiles = []
    for dy, dx in TAPS:   # 9 lo tiles (sorted tap order == TAPS order)
        ktiles.append(("lo", (dy, dx)))
    ktiles.append(("d1", -1)); ktiles.append(("d1", 0)); ktiles.append(("d1", 1))
    ktiles.append(("single", None)); ktiles.append(("d2", None))
    kv8 = np.zeros((128, 7, 2, 384), np.float32)
    for pj in range(7):
        for j in range(2):
            kind, arg = ktiles[2 * pj + j]
            blk = np.zeros((128, 384), np.float32)
            if kind == "lo":
                blk[0:128] = tl(*arg)
            elif kind == "d1":
                blk[0:64] = th(-1, arg)
                blk[64:128] = th(0, arg)
            elif kind == "d2":
                blk[0:64] = th(1, -1)
                blk[64:128] = th(1, 1)
            elif kind == "single":
                blk[0:64] = th(1, 0)
            kv8[:, pj, j, :] = blk
    out["kv8"] = (kv8 * WS).astype(f8)

    # kvdw depthwise: plain layout, diag blocks per pblock, DR tap pairs
    kvdw = w["kvdw_w"][:, 0]           # (384, 3, 3)
    kvdw8 = np.zeros((128, 3, 5, 2, 128), np.float32)
    for pj, (ta, tb) in enumerate(BD_PAIRS):
        for pb in range(3):
            dya, dxa = TAPS[ta]
            kvdw8[:, pb, pj, 0, :] = np.diag(kvdw[128 * pb:128 * pb + 128, dya + 1, dxa + 1])
            if tb is not None:
                dyb, dxb = TAPS[tb]
                kvdw8[:, pb, pj, 1, :] = np.diag(kvdw[128 * pb:128 * pb + 128, dyb + 1, dxb + 1])
    out["kvdw8"] = (kvdw8 * WS).astype(f8)

    def dw_channel(pb, p):
        return 128 * pb + p   # plain layout

    # newk (no bias on device; kT = NKS * k0): contraction sources
    # pass0 = (k8 slot0 = qkv pb2, k8 slot1 = qkv pb3)
    # pass1 = (dwk8 slot0 = dw pb0, dwk8 slot1 = dw pb1 parts 0..63)
    KCC = [("qkv", 2, 0, 128), ("qkv", 3, 0, 128), ("dw", 0, 0, 128), ("dw", 1, 0, 64)]
    nk = w["newk_w"][:, :, 0, 0]       # (192, 384): in = [k(192) | k_mask(192)]
    nkm = np.zeros((128, 4, 192), np.float32)
    for j, (src, pb, base, sz) in enumerate(KCC):
        for p in range(base, base + sz):
            if src == "qkv":
                ch = qkv_channel_at(pb, p)
                if ch is not None:
                    nkm[p, j, :] = nk[:, ch - 192]      # k part: qkv ch 192-383
            else:
                ch = dw_channel(pb, p)
                if ch < 192:
                    nkm[p, j, :] = nk[:, 192 + ch]      # k_mask: dw ch 0-191
    out["nkw8"] = (nkm.reshape(128, 2, 2, 192) * NKS).astype(f8)

    nv = w["newv_w"][:, :, 0, 0]       # (192, 384): in = [v(192) | v_mask(192)]
    nvT = np.zeros((128, 4, 192), np.float32)
    VCC_P = [("qkv", 4, 0, 128), ("qkv", 5, 0, 128), ("dw", 1, 64, 64), ("dw", 2, 0, 128)]
    for j, (src, pb, base, sz) in enumerate(VCC_P):
        for p in range(base, base + sz):
            if src == "qkv":
                ch = qkv_channel_at(pb, p)
                if ch is not None:
                    nvT[p, j, :] = nv[:, ch - 384]      # v part: qkv ch 384-575
            else:
                ch = dw_channel(pb, p)
                if ch >= 192:
                    nvT[p, j, :] = nv[:, ch]            # v_mask: dw ch 192-383
    out["nvw8"] = (nvT.reshape(128, 2, 2, 192) * NVS).astype(f8)

    out["ident"] = np.eye(128, dtype=bf16)

    x1b = np.zeros((128, 6), np.float32)
    qkvb = np.zeros((128, 6), np.float32)
    for (pb, h, x1b_base, qb, ch0, nch) in qkv_halves():
        x1b[x1b_base:x1b_base + nch, pb] = w["q_b"][ch0:ch0 + nch]
        qkvb[qb:qb + nch, pb] = w["qdw_b"][ch0:ch0 + nch]
    # hijacked ones rows: junk q pb1 partitions become the exact constant
    # 1/QTS, which the qT8 scale turns into 1.0 -> gramB rows = ksum/kss
    for r in ONE_ROWS:
        qkvb[r, 1] = 1.0 / QTS
    out["x1_bias"] = x1b * WS
    out["qkv_bias"] = qkvb

    kvb = np.zeros((128, 3), np.float32)
    dwb = np.zeros((128, 3), np.float32)
    for pb in range(3):
        kvb[:, pb] = w["kv_b"][128 * pb:128 * pb + 128]
        dwb[:, pb] = w["kvdw_b"][128 * pb:128 * pb + 128]
    out["kv_bias"] = kvb * WS
    out["kvdw_bias"] = dwb
    nvb = np.zeros((128, 2), np.float32)
    nvb[:, 0] = w["newv_b"][0:128]
    nvb[0:64, 1] = w["newv_b"][128:192]
    out["newv_bias"] = nvb * VS
    return out


def prep_core(x, xm, b, half, R, H):
    xp = np.zeros((192, R + 2, WP), np.float32)
    mp = np.zeros((192, R + 4, WP), np.float32)
    for r in range(R + 2):
        g = half * R + (r - 1)
        if 0 <= g < H:
            xp[:, r, 1:129] = x[b, :, g, :]
    for r in range(R + 4):
        g = half * R + (r - 2)
        if 0 <= g < H:
            mp[:, r, 1:129] = xm[b, :, g, :]
    xp = xp.reshape(192, -1)
    x8 = np.zeros((128, 2, xp.shape[1]), np.float32)
    x8[:, 0, :] = xp[0:128]
    x8[0:64, 1, :] = xp[128:192]
    x8[64:128, 1, :] = xp[128:192]   # dup (weights zero) to avoid NaN garbage
    mp = mp.reshape(192, -1)
    L = mp.shape[1]
    hi = mp[128:192]
    d1 = np.zeros((128, L), np.float32)
    d2 = np.zeros((128, L), np.float32)
    d1[0:64] = hi
    d1[64:128, :L - WP] = hi[:, WP:]
    d2[0:64] = hi
    d2[64:128, :L - 2] = hi[:, 2:]
    edge = np.ones((128, 2), np.float32)
    if half == 0:
        edge[:, 0] = 0.0     # slab0 row0 = image row -1
    else:
        edge[:, 1] = 0.0     # last slab last row = image row H
    return {
        "x8": x8.astype(f8),
        "xm_lo": mp[0:128].astype(f8),
        "xm_d1": d1.astype(f8), "xm_d2": d2.astype(f8),
        "edge": edge,
    }


# ---------------- host glue (unchanged semantics) ----------------

def _q_maps():
    part = np.zeros(192, np.int64)
    pblk = np.zeros(192, np.int64)
    for (pb, h, x1b, qb, ch0, nch) in qkv_halves():
        if pb >= 2:
            continue
        for i in range(nch):
            pblk[ch0 + i] = pb
            part[ch0 + i] = qb + i
    return pblk, part


def _ss_from_qstats(stats, n_half):
    pblk, part = _q_maps()
    mv = stats.astype(np.float64)
    return (mv[part, pblk, 1] + mv[part, pblk, 0] ** 2) * n_half


def _sum_from_qstats(stats, n_half):
    pblk, part = _q_maps()
    mv = stats.astype(np.float64)
    return mv[part, pblk, 0] * n_half


def _ss_from_vstats(stats, newv_b, n_half):
    # stats are of NVS*(v - b): undo the scale, add back the bias
    m = np.zeros(192, np.float64)
    w = np.zeros(192, np.float64)
    mv = stats.astype(np.float64)
    m[0:128] = mv[0:128, 0, 0]
    w[0:128] = mv[0:128, 0, 1]
    m[128:192] = mv[0:64, 1, 0]
    w[128:192] = mv[0:64, 1, 1]
    m = m / NVS + newv_b.astype(np.float64)
    w = w / (NVS * NVS)
    return (w + m * m) * n_half


def glue(res0, res1, newv_b, n_half):
    """Combine two half-core L1 results into raw sums."""
    GT = res0["gramT_out"].astype(np.float64) + res1["gramT_out"].astype(np.float64)
    pblk, part = _q_maps()
    qrow = pblk * 128 + part
    # GT rows are qT cols (pb*128+part); cols are newk out-ch d. Stored values
    # are sum_n (QTS*q) * (NKS*k0).
    Gq = GT[qrow, :] / (QTS * NKS)             # (c, d): sum_n q[c,n] k0[d,n]
    qss = _ss_from_qstats(res0["qstats_out"], n_half) + _ss_from_qstats(res1["qstats_out"], n_half)
    qsum = _sum_from_qstats(res0["qstats_out"], n_half) + _sum_from_qstats(res1["qstats_out"], n_half)
    vss = _ss_from_vstats(res0["vstats_out"], newv_b, n_half) + \
        _ss_from_vstats(res1["vstats_out"], newv_b, n_half)
    kss_raw = (res0["kss_out"].astype(np.float64) + res1["kss_out"].astype(np.float64))[0]
    k0sum = kss_raw[0:192] / NKS               # sum_n k0 (ones x kT cols)
    k0ss = kss_raw[192:384] / (KSQS * KSQS)    # sum_n k0^2 (ones x ksq cols)
    return Gq, qss, qsum, vss, k0ss, k0sum


def glue_full(res0, res1, temperature, newk_b, newv_b, proj_w, proj_b, n_half):
    Gq, qss, qsum, vss, k0ss, k0sum = glue(res0, res1, newv_b, n_half)
    b = newk_b.astype(np.float64)              # (192,)
    # k = k0 + b: gram/kss bias corrections (sums already cover both halves,
    # total n = 2 * n_half)
    G = Gq + qsum[:, None] * b[None, :]        # (c, d): sum_n q k
    kss = k0ss + 2 * b * k0sum + (2 * n_half) * b * b
    qn = np.maximum(np.sqrt(qss), 1e-12)
    kn = np.maximum(np.sqrt(kss), 1e-12)
    vn = np.maximum(np.sqrt(vss), 1e-12)
    A = G / (qn[:, None] * kn[None, :])        # (c, d)
    M = np.zeros((192, 192), np.float64)
    t = np.asarray(temperature).reshape(-1)
    for h in range(8):
        sl = slice(24 * h, 24 * h + 24)
        a = A[sl, sl] * t[h]
        a = a - a.max(axis=-1, keepdims=True)
        e = np.exp(a)
        sm = e / e.sum(axis=-1, keepdims=True)
        M[sl, sl] = sm / vn[None, sl]
    At = proj_w[:, :, 0, 0].astype(np.float64) @ M   # (out-ch o, d)
    awT = np.zeros((128, 2, 192), np.float32)
    awT[:, 0, :] = At.T[0:128] * AWS
    awT[0:64, 1, :] = At.T[128:192] * AWS
    pbias = np.zeros((128, 2), np.float32)
    pbias[:, 0] = proj_b[0:128]
    pbias[0:64, 1] = proj_b[128:192]
    return {"awT": awT.astype(f8), "pbias": pbias}


# ---------------- driver: kernel(**inputs) ----------------
from concourse.bass_utils import run_bass_kernel_spmd

R_FULL, H_FULL, B_FULL = 64, 128, 4
_NC1 = None
_NC2 = None


def _get_progs():
    global _NC1, _NC2
    if _NC1 is None:
        _NC1 = build_l1(R=R_FULL, S=16)
        _NC2 = build_l2(R=R_FULL)
    return _NC1, _NC2


def kernel(**inputs):
    inputs = {k: np.asarray(v) for k, v in inputs.items()}
    x, xm = inputs["x"], inputs["x_mask"]
    nc1, nc2 = _get_progs()
    wprep = prep_weights(inputs)
    in_maps = []
    for core in range(8):
        b, half = core // 2, core % 2
        m = dict(wprep)
        m.update(prep_core(x, xm, b, half, R_FULL, H_FULL))
        in_maps.append(m)
    res1 = run_bass_kernel_spmd(nc1, in_maps, list(range(8))).results

    n_half = R_FULL * 128
    in_maps2 = []
    for core in range(8):
        b, half = core // 2, core % 2
        if half == 0:
            l2c = glue_full(res1[2 * b], res1[2 * b + 1], inputs["temperature"],
                            inputs["newk_b"], inputs["newv_b"],
                            inputs["proj_w"], inputs["proj_b"], n_half)
        m = dict(l2c)
        m["v_in"] = res1[core]["v_out"]
        in_maps2.append(m)
    res2 = run_bass_kernel_spmd(nc2, in_maps2, list(range(8))).results

    out = np.empty((B_FULL, 192, H_FULL, 128), np.float32)
    for core in range(8):
        b, half = core // 2, core % 2
        out[b, :, half * R_FULL:(half + 1) * R_FULL, :] = \
            res2[core]["out"].reshape(192, R_FULL, 128).astype(np.float32)
    return out


# revision 44
# speedup vs baseline: 1.1463x; 1.0121x over previous
"""Trainium2 Bass kernel for nn_Cross_Attention — fp8 DoubleRow rewrite.

L1: convs + gram partials with fp8 DoubleRow matmuls (phases A-D), fp8 F-phase
(newk/gram/kss via DR), bf16 E. L2: attn-apply with chunked v DMA. Host glue
between (softmax etc.). Sharding: 4 samples x 2 row-halves across 8 cores.
"""
import sys
sys.path.insert(0, "/opt/trn_rl_repo")
import numpy as np
import ml_dtypes

import concourse.bass as bass
import concourse.tile as tile
from concourse import bacc, mybir
from contextlib import ExitStack

BF16 = mybir.dt.bfloat16
F8 = mybir.dt.float8e4
F32 = mybir.dt.float32
bf16 = ml_dtypes.bfloat16
f8 = ml_dtypes.float8_e4m3
DR = mybir.MatmulPerfMode.DoubleRow

WS = 16.0          # fp8 weight pre-scale
WP = 130           # padded row width
NKS = 32.0         # newk weight pre-scale (kT stored as NKS*k0)
QTS = 16.0         # qT pre-scale (qT stored as QTS*q)
KSQS = 64.0        # ksq stored as (KSQS*k0)**2
NVS = 32.0         # newv weight pre-scale
VS = 8.0           # v8 storage scale (v stored as VS*v)
AWS = 512.0        # attn-weight (aw8) pre-scale
ONE_ROWS = (32, 33)  # junk q pb1 partitions hijacked as 1.0 cols in qT8 (ksum/kss);
                     # must be 32-aligned (engine partition-base alignment)

TAPS = [(dy, dx) for dy in (-1, 0, 1) for dx in (-1, 0, 1)]
# taps sorted by flat offset dy*WP+dx (they already are, given WP>2)
TAP_OFF = [dy * WP + dx for dy, dx in TAPS]
# DoubleRow tap pairs for B/D: (idx_a, idx_b). The odd tap rides first with a
# zero-weight second tile at stride +1 (always in-bounds since tap 0 has the
# smallest offset).
BD_PAIRS = [(0, None), (1, 2), (3, 4), (5, 6), (7, 8)]


def qkv_halves():
    """Per (pb, half): (x1_base, qkv_base, ch0, nch).  ch0 = qkv-global channel."""
    out = []
    for pb in range(6):
        P, odd = pb // 2, pb % 2
        for h in (0, 1):
            nch = 3 if (odd and h == 1) else 63
            ch0 = 3 * (64 * P + 42 * odd + 21 * h)
            x1b = 64 * h
            qb = 64 * h if not odd else 64 * (1 - h)
            out.append((pb, h, x1b, qb, ch0, nch))
    return out


def qkv_channel_at(pb, p):
    """qkv-global channel stored at partition p of qkv pblock pb, or None."""
    for (pb2, h, x1b, qb, ch0, nch) in qkv_halves():
        if pb2 == pb and qb <= p < qb + nch:
            return ch0 + (p - qb)
    return None


# newk/newv input chunks with PLAIN dw layout (dw pb_i = kv ch 128i..128i+127):
# k = qkv ch 192-383 (pb 2,3); k_mask = kv ch 0-191 = dw pb0 + dw pb1[0:64]
# v = qkv ch 384-575 (pb 4,5); v_mask = kv ch 192-383 = dw pb1[64:128] + dw pb2
# Tile routing: q_t = qkv pb0,1 (bf16); k8 = qkv pb2,3 (f8, chunk-major);
# vq = qkv pb4,5 (f8); dwk8 = dw pb0 + dw pb1[0:64] (f8, chunk-major);
# dwv = dw pb1[64:128] + dw pb2 (f8, slot0 parts 64-127 used).


def _bcast(ap, p):
    return bass.AP(tensor=ap.tensor, offset=ap.offset, ap=[[0, p]] + list(ap.ap[1:]))


def _pair_ap(t, off, delta, dims):
    """AP over tile t at flat free-offset `off`: [partitions, 2 (stride delta), *dims]."""
    return bass.AP(tensor=t.tensor, offset=t.offset + off,
                   ap=[list(t.ap[0]), [delta, 2]] + [list(d) for d in dims])


def build_l1(R=64, W=128, S=16):
    assert W == 128 and R % S == 0 and S % 4 == 0
    NSLAB = R // S
    NS = S * W
    XCOLS = (S + 2) * WP
    MCOLS = (S + 4) * WP
    MC2 = MCOLS + 2
    N128 = NS // 128

    nc = bacc.Bacc("TRN2", target_bir_lowering=False, debug=False, num_devices=8)

    def din(name, shape, dt=F8):
        return nc.dram_tensor(name, shape, dt, kind="ExternalInput").ap()

    def dout(name, shape, dt=F32):
        return nc.dram_tensor(name, shape, dt, kind="ExternalOutput").ap()

    x8 = din("x8", [128, 2, (R + 2) * WP])
    xm_lo = din("xm_lo", [128, (R + 4) * WP])
    xm_d1 = din("xm_d1", [128, (R + 4) * WP])
    xm_d2 = din("xm_d2", [128, (R + 4) * WP])
    qw8 = din("qw8", [128, 2, 768])
    qdw8 = din("qdw8", [128, 6, 5, 2, 128])
    kv8 = din("kv8", [128, 7, 2, 384])
    kvdw8 = din("kvdw8", [128, 3, 5, 2, 128])
    nkw8 = din("nkw8", [128, 2, 2, 192], F8)
    nvw8 = din("nvw8", [128, 2, 2, 192], F8)
    ident = din("ident", [128, 128], BF16)
    x1_bias = din("x1_bias", [128, 6], F32)      # x16
    qkv_bias = din("qkv_bias", [128, 6], F32)
    kv_bias = din("kv_bias", [128, 3], F32)      # x16, plain layout
    kvdw_bias = din("kvdw_bias", [128, 3], F32)  # plain layout
    newv_bias = din("newv_bias", [128, 2], F32)  # x VS
    edge = din("edge", [128, 2], F32)            # per-core edge-row multipliers

    v_out = dout("v_out", [192, R * W], F8)
    gramT_out = dout("gramT_out", [256, 192])    # rows = qcol space (pb*128+part)
    kss_out = dout("kss_out", [1, 384])          # [sum (KSQS*k0)^2 | sum NKS*k0]
    qstats_out = dout("qstats_out", [128, 2, 2])
    vstats_out = dout("vstats_out", [128, 2, 2])

    # C-phase DoubleRow k-tile pair coordinates in the [128, 3, MC2] msl tile
    # flat free space: region*MC2 + off.  Region 0 = lo channels, 1 = d1
    # (hi | hi<<WP), 2 = d2 (hi | hi<<2).
    def lo(dy, dx):
        return (1 + dy) * WP + dx + 1

    def d1(off):
        return MC2 + off

    def d2(off):
        return 2 * MC2 + off

    # 14 k-tiles -> 7 pairs; weight slot kv8[:, pair, j, :] must match.
    CPAIRS = [
        (lo(-1, -1), lo(-1, 0)), (lo(-1, 1), lo(0, -1)), (lo(0, 0), lo(0, 1)),
        (lo(1, -1), lo(1, 0)), (lo(1, 1), d1(0)), (d1(1), d1(2)),
        (d1(2 * WP + 1), d2(2 * WP)),
    ]

    with tile.TileContext(nc) as tc, ExitStack() as ctx:
        wpool = ctx.enter_context(tc.tile_pool(name="weights", bufs=1))
        xpool = ctx.enter_context(tc.tile_pool(name="xslab", bufs=2))
        bigpool = ctx.enter_context(tc.tile_pool(name="big", bufs=1))
        midpool = ctx.enter_context(tc.tile_pool(name="mid", bufs=2))
        smpool = ctx.enter_context(tc.tile_pool(name="small", bufs=4))
        statpool = ctx.enter_context(tc.tile_pool(name="stats", bufs=1))
        pspool = ctx.enter_context(tc.tile_pool(name="ps", bufs=6, space="PSUM"))
        pspers = ctx.enter_context(tc.tile_pool(name="pspers", bufs=1, space="PSUM"))

        def load1(ap_in, shape, dt=F8, eng=None):
            t = wpool.tile(shape, dt, tag=ap_in.tensor.name)
            (eng or nc.sync).dma_start(out=t[:ap_in.shape[0]], in_=ap_in[:])
            return t

        g = nc.gpsimd
        qw8_s = load1(qw8, [128, 2, 768], eng=g)
        x1b_s = load1(x1_bias, [128, 6], F32, eng=g)
        qdw8_s = load1(qdw8, [128, 6, 5, 2, 128], eng=g)
        kv8_s = load1(kv8, [128, 7, 2, 384], eng=g)
        kvdw8_s = load1(kvdw8, [128, 3, 5, 2, 128], eng=g)
        nkw8_s = load1(nkw8, [128, 2, 2, 192], F8, eng=g)
        nvw8_s = load1(nvw8, [128, 2, 2, 192], F8, eng=g)
        id_s = load1(ident, [128, 128], BF16, eng=g)
        qkvb_s = load1(qkv_bias, [128, 6], F32, eng=g)
        kvb_s = load1(kv_bias, [128, 3], F32, eng=g)
        dwb_s = load1(kvdw_bias, [128, 3], F32, eng=g)
        nvb_s = load1(newv_bias, [128, 2], F32, eng=g)
        edge_s = load1(edge, [128, 2], F32, eng=g)

        # persistent accumulators: one PSUM bank each (zero-region granularity)
        # gramB is 384 wide: cols 192:384 see the ksq half of the moving data;
        # its ONE_ROWS rows (ones in the stationary) yield ksum/kss.
        gramA = pspers.tile([128, 192], F32)
        gramB = pspers.tile([128, 384], F32)

        qstats = statpool.tile([128, 2, NSLAB * (NS // 512), 6], F32)
        vstats = statpool.tile([128, 2, NSLAB * (NS // 512), 6], F32)
        # double-buffered chunk-pair staging for gram/kss (dims: pairbuf, slot).
        # inner dim padded to 400 so the DR pair stride is NOT contiguous with
        # the column dim (contiguous dims get re-flattened in lowering, which
        # breaks the pair interpretation of the moving AP).
        kq8 = statpool.tile([128, 2, 2, 400], F8)    # [0:192]=NKS*k0, [192:384]=ksq
        qT8 = statpool.tile([128, 2, 2, 256], F8)

        n128_total = NSLAB * N128
        CT = [(c0, min(512, XCOLS - c0)) for c0 in range(0, XCOLS, 512)]
        # slabs > 0 reuse the previous slab's last 2 rows of x1/kv1 (halo
        # copy) and only compute the remaining S rows
        CT2 = [(2 * WP + 416 * k, 416) for k in range(5)]
        x1 = bigpool.tile([128, 6, XCOLS], F8, tag="x1")
        kv1 = bigpool.tile([128, 3, XCOLS], F8, tag="kv1")

        for s in range(NSLAB):
            xsl = xpool.tile([128, 2, XCOLS], F8, tag="xsl")
            msl = xpool.tile([128, 3, MC2], F8, tag="msl")
            off = s * S * WP
            # spread the slab input loads over several DMA queues; slab 0's
            # x load is split so the first phase-A matmuls gate on only the
            # first half (sync's queue is otherwise empty at start)
            if s == 0:
                h = 9 * WP
                nc.sync.dma_start(out=xsl[:, :, 0:h], in_=x8[:, :, off:off + h])
                nc.sync.dma_start(out=xsl[:, :, h:XCOLS],
                                  in_=x8[:, :, off + h:off + XCOLS])
            else:
                nc.sync.dma_start(out=xsl[:], in_=x8[:, :, off:off + XCOLS])
            nc.scalar.dma_start(out=msl[:, 0, 1:1 + MCOLS], in_=xm_lo[:, off:off + MCOLS])
            nc.gpsimd.dma_start(out=msl[:, 1, 1:1 + MCOLS], in_=xm_d1[:, off:off + MCOLS])
            nc.scalar.dma_start(out=msl[:, 2, 1:1 + MCOLS], in_=xm_d2[:, off:off + MCOLS])
            for r in range(3):
                nc.vector.memset(msl[:, r, 0:1], 0.0)
                nc.vector.memset(msl[:, r, MC2 - 1:MC2], 0.0)

            def pad_zero(t, npb):
                # zero the 2 pad columns of every row: flat r*WP + {0, 129}
                ap = bass.AP(tensor=t.tensor, offset=t.offset,
                             ap=[list(t.ap[0]), [XCOLS, npb], [WP, S + 2], [WP - 1, 2]])
                nc.vector.memset(ap, 0.0)
                # per-core edge pad row: slab0 row0 (half 0) / slab3 last row
                # (half 1), selected by the `edge` input multipliers
                if s == 0:
                    nc.vector.tensor_scalar(
                        out=t[:, :, 0:WP], in0=t[:, :, 0:WP],
                        scalar1=edge_s[:, 0:1], scalar2=None,
                        op0=mybir.AluOpType.mult)
                if s == NSLAB - 1:
                    nc.vector.tensor_scalar(
                        out=t[:, :, (S + 1) * WP:(S + 2) * WP],
                        in0=t[:, :, (S + 1) * WP:(S + 2) * WP],
                        scalar1=edge_s[:, 1:2], scalar2=None,
                        op0=mybir.AluOpType.mult)

            # ---- Phase A: x1 = 1x1(x); one DR matmul per (pb, col-tile)
            if s > 0:
                nc.sync.dma_start(out=x1[:, :, 0:2 * WP],
                                  in_=x1[:, :, S * WP:(S + 2) * WP])
                nc.gpsimd.dma_start(out=kv1[:, :, 0:2 * WP],
                                    in_=kv1[:, :, S * WP:(S + 2) * WP])
            for pb in range(6):
                for c0, cs in (CT if s == 0 else CT2):
                    ps = pspool.tile([128, 512], F32, tag="ps", name=f"psA{s}_{pb}_{c0}")
                    nc.tensor.matmul(ps[:, :cs], qw8_s[:, :, 128 * pb:128 * pb + 128],
                                     xsl[:, :, c0:c0 + cs], start=True, stop=True,
                                     perf_mode=DR)
                    nc.vector.tensor_scalar(
                        out=x1[:, pb, c0:c0 + cs], in0=ps[:, :cs],
                        scalar1=x1b_s[:, pb:pb + 1], scalar2=1.0 / WS,
                        op0=mybir.AluOpType.add, op1=mybir.AluOpType.mult)
            pad_zero(x1, 6)

            # ---- Phase C: kv1 = 3x3(xm); 7 DR pairs per (pb, col-tile)
            for pb in range(3):
                for cset in ((CT[:3], CT[3:]) if s == 0 else (CT2[:3], CT2[3:])):
                    pss = [pspool.tile([128, 512], F32, tag="ps",
                                       name=f"psC{s}_{pb}_{c0}")
                           for (c0, cs) in cset]
                    for pj, (ca, cb) in enumerate(CPAIRS):
                        lhsT = kv8_s[:, pj, :, 128 * pb:128 * pb + 128]
                        for ti, (c0, cs) in enumerate(cset):
                            rhs = _pair_ap(msl, ca + c0, cb - ca, [[1, cs]])
                            nc.tensor.matmul(pss[ti][:, :cs], lhsT, rhs,
                                             start=(pj == 0), stop=(pj == 6),
                                             perf_mode=DR)
                    for ti, (c0, cs) in enumerate(cset):
                        nc.vector.tensor_scalar(
                            out=kv1[:, pb, c0:c0 + cs], in0=pss[ti][:, :cs],
                            scalar1=kvb_s[:, pb:pb + 1], scalar2=1.0 / WS,
                            op0=mybir.AluOpType.add, op1=mybir.AluOpType.mult)
            pad_zero(kv1, 3)

            # ---- Phase B: qkv = qdw(x1); 2-row groups (N=258), 5 DR tap-pairs
            # per group chained in PSUM; 4-group sets amortize weight loads.
            # route: pb -> list of (dst_tile, slot, part_lo, part_hi)
            def dwconv(src, wsrc, route, npb, bias_s, tag):
                for pb in range(npb):
                    for st in range(2):
                        gset = list(range(st * 4, st * 4 + 4))
                        pss = {gi: pspool.tile([128, 258], F32, tag="ps",
                                               name=f"ps{tag}{s}_{pb}_{gi}")
                               for gi in gset}
                        for pj, (ta, tb) in enumerate(BD_PAIRS):
                            oa = TAP_OFF[ta]
                            delta = (TAP_OFF[tb] - oa) if tb is not None else 1
                            lhsT = wsrc[:, pb, pj, :, :]
                            for gi in gset:
                                base = pb * XCOLS + (2 * gi + 1) * WP + 1 + oa
                                rhs = _pair_ap(src, base, delta, [[1, 258]])
                                nc.tensor.matmul(pss[gi][:, :], lhsT, rhs,
                                                 start=(pj == 0), stop=(pj == 4),
                                                 perf_mode=DR)
                        for gi in gset:
                            ps = pss[gi]
                            in_ap = bass.AP(tensor=ps.tensor, offset=ps.offset,
                                            ap=[list(ps.ap[0]), [WP, 2], [1, 128]])
                            for (dst, slot, plo, phi, chunked) in route(pb):
                                if chunked:
                                    out_ap = dst[plo:phi, 2 * gi:2 * gi + 2, slot, :]
                                else:
                                    out_ap = dst[plo:phi, slot,
                                                 2 * gi * 128:(2 * gi + 2) * 128]
                                nc.scalar.activation(
                                    out=out_ap,
                                    in_=in_ap[plo:phi],
                                    func=mybir.ActivationFunctionType.Identity,
                                    bias=bias_s[plo:phi, pb:pb + 1], scale=1.0 / WS)

            # k8/dwk8 are chunk-major [128, N128, 2, 128] so the kps DR
            # stationary pair is contiguous (pair stride 128 — ISA requires
            # small pair strides for dual-fp8 ldweights).
            q_t = bigpool.tile([128, 2, NS], BF16, tag="q_t")
            k8 = bigpool.tile([128, N128, 2, 128], F8, tag="k8")
            vq = bigpool.tile([128, 2, NS], F8, tag="vq")

            def qkv_route(pb):
                dst = (q_t, k8, vq)[pb // 2]
                return [(dst, pb % 2, 0, 128, pb // 2 == 1)]

            dwconv(x1, qdw8_s, qkv_route, 6, qkvb_s, "B")

            # ---- Phase D: dw = kvdw(kv1); same structure, 3 pblocks
            dwk8 = bigpool.tile([128, N128, 2, 128], F8, tag="dwk8")
            dwv = bigpool.tile([128, 2, NS], F8, tag="dwv")
            nc.vector.memset(dwk8[64:128, :, 1, :], 0.0)
            nc.vector.memset(dwv[0:64, 0, :], 0.0)

            def dw_route(pb):
                if pb == 0:
                    return [(dwk8, 0, 0, 128, True)]
                if pb == 1:
                    return [(dwk8, 1, 0, 64, True), (dwv, 0, 64, 128, False)]
                return [(dwv, 1, 0, 128, False)]

            dwconv(kv1, kvdw8_s, dw_route, 3, dwb_s, "D")

            # ---- Phase E: v = newv(v_cc), fp8 DR (pair over the two source
            # tiles of each pass); psum holds NVS*(v - b); bn_stats on psum.
            vt = midpool.tile([128, 2, NS], F8, tag="vt")
            for mb in range(2):
                msz = 128 if mb == 0 else 64
                for ic in range(NS // 512):
                    sl = slice(ic * 512, (ic + 1) * 512)
                    ps = pspool.tile([128, 512], F32, tag="ps",
                                     name=f"psE{s}_{mb}_{ic}")
                    nc.tensor.matmul(ps[:msz, :],
                                     nvw8_s[:, 0, :, mb * 128:mb * 128 + msz],
                                     vq[:, :, sl], start=True, stop=False,
                                     perf_mode=DR)
                    nc.tensor.matmul(ps[:msz, :],
                                     nvw8_s[:, 1, :, mb * 128:mb * 128 + msz],
                                     dwv[:, :, sl], start=False, stop=True,
                                     perf_mode=DR)
                    si = s * (NS // 512) + ic
                    nc.vector.bn_stats(out=vstats[:msz, mb, si, :], in_=ps[:msz, :])
                    nc.scalar.activation(out=vt[:msz, mb, sl], in_=ps[:msz, :],
                                         func=mybir.ActivationFunctionType.Identity,
                                         bias=nvb_s[:msz, mb:mb + 1], scale=VS / NVS)
            nc.sync.dma_start(out=v_out[0:128, s * NS:(s + 1) * NS], in_=vt[:, 0, :])
            nc.sync.dma_start(out=v_out[128:192, s * NS:(s + 1) * NS], in_=vt[:64, 1, :])
            for sub in range(NS // 512):
                si = s * (NS // 512) + sub
                sl = slice(sub * 512, (sub + 1) * 512)
                nc.vector.bn_stats(out=qstats[:, 0, si, :], in_=q_t[:, 0, sl])
                nc.vector.bn_stats(out=qstats[:, 1, si, :], in_=q_t[:, 1, sl])

            # ---- Phase F: per 128-n chunk: kT (newk, fp8 DR), qT (transpose),
            # then per chunk-pair: gram (fp8 DR) + kss.  Gram for pair P is
            # emitted one chunk late (during chunk 2P+2) so the scalar-engine
            # kq8/qT8 writes are done before the PE needs them.
            def emit_gram(gp):
                pb = gp % 2
                first, last = gp == 0, gp == n128_total // 2 - 1
                nc.tensor.matmul(gramA[:, :], qT8[:, pb, :, 0:128],
                                 kq8[:, pb, :, 0:192], start=first, stop=last,
                                 perf_mode=DR)
                nc.tensor.matmul(gramB[:, :], qT8[:, pb, :, 128:256],
                                 kq8[:, pb, :, 0:384], start=first, stop=last,
                                 perf_mode=DR)

            for ic in range(N128):
                c0 = ic * 128
                gidx = s * N128 + ic
                slot = ic % 2
                pbuf = (ic // 2) % 2
                kps = pspool.tile([128, 192], F32, tag="ps", name=f"kps{s}_{ic}")
                nc.tensor.matmul(kps[:, :], k8[:, ic, :, :], nkw8_s[:, 0, :, :],
                                 start=True, stop=False, perf_mode=DR)
                nc.tensor.matmul(kps[:, :], dwk8[:, ic, :, :], nkw8_s[:, 1, :, :],
                                 start=False, stop=True, perf_mode=DR)
                nc.scalar.copy(out=kq8[:, pbuf, slot, 0:192], in_=kps[:, :])
                nc.scalar.activation(out=kq8[:, pbuf, slot, 192:384], in_=kps[:, :],
                                     func=mybir.ActivationFunctionType.Square,
                                     scale=KSQS / NKS)
                qps = pspool.tile([128, 256], BF16, tag="ps", name=f"qps{s}_{ic}")
                nc.tensor.transpose(qps[:, 0:128], q_t[:, 0, c0:c0 + 128], id_s[:, :])
                nc.tensor.transpose(qps[:, 128:256], q_t[:, 1, c0:c0 + 128], id_s[:, :])
                nc.scalar.mul(out=qT8[:, pbuf, slot, :], in_=qps[:, :], mul=QTS)
                if slot == 0 and gidx >= 2:
                    emit_gram(gidx // 2 - 1)

        emit_gram(n128_total // 2 - 1)

        qmv = statpool.tile([128, 2, 2], F32)
        vmv = statpool.tile([128, 2, 2], F32)
        nc.vector.memset(qmv[:], 0.0)
        nc.vector.memset(vmv[:], 0.0)
        nc.vector.bn_aggr(out=qmv[:, 0, :], in_=qstats[:, 0, :, :])
        nc.vector.bn_aggr(out=qmv[:, 1, :], in_=qstats[:, 1, :, :])
        nc.vector.bn_aggr(out=vmv[:, 0, :], in_=vstats[:, 0, :, :])
        nc.vector.bn_aggr(out=vmv[:64, 1, :], in_=vstats[:64, 1, :, :])
        nc.sync.dma_start(out=qstats_out[:], in_=qmv[:])
        nc.sync.dma_start(out=vstats_out[:], in_=vmv[:])
        gA = statpool.tile([128, 192], F32)
        gB = statpool.tile([128, 192], F32)
        kssb = statpool.tile([128, 384], F32)
        nc.scalar.copy(out=gA[:], in_=gramA[:])
        nc.scalar.copy(out=gB[:], in_=gramB[:, 0:192])
        r0 = ONE_ROWS[0]
        nc.scalar.copy(out=kssb[r0:r0 + 1, :], in_=gramB[r0:r0 + 1, 0:384])
        nc.sync.dma_start(out=gramT_out[0:128, :], in_=gA[:])
        nc.sync.dma_start(out=gramT_out[128:256, :], in_=gB[:])
        nc.sync.dma_start(out=kss_out[:], in_=kssb[r0:r0 + 1, :])

    nc.compile()
    return nc


def build_l2(R=64, W=128):
    NS = R * W
    nc = bacc.Bacc("TRN2", target_bir_lowering=False, debug=False, num_devices=8)
    v_in = nc.dram_tensor("v_in", [256, NS], F8, kind="ExternalInput").ap()
    awT = nc.dram_tensor("awT", [128, 2, 192], F8, kind="ExternalInput").ap()
    pbias = nc.dram_tensor("pbias", [128, 2], F32, kind="ExternalInput").ap()
    out = nc.dram_tensor("out", [192, NS], BF16, kind="ExternalOutput").ap()

    with tile.TileContext(nc) as tc, ExitStack() as ctx:
        wpool = ctx.enter_context(tc.tile_pool(name="w", bufs=1))
        vpool = ctx.enter_context(tc.tile_pool(name="v", bufs=1))
        opool = ctx.enter_context(tc.tile_pool(name="o", bufs=1))
        pspool = ctx.enter_context(tc.tile_pool(name="ps", bufs=8, space="PSUM"))

        aw = wpool.tile([128, 2, 192], F8)
        nc.sync.dma_start(out=aw[:], in_=awT[:])
        pb = wpool.tile([128, 2], F32)
        nc.sync.dma_start(out=pb[:], in_=pbias[:])

        # batched: v lives in SBUF whole (fp8), loaded in chunks on 2 queues;
        # outputs accumulate in SBUF and leave in 2048-col chunks.
        vfull = vpool.tile([128, 2, NS], F8)
        ofull = opool.tile([128, 2, NS], BF16)
        nc.vector.memset(vfull[64:128, 1, :], 0.0)
        CL = NS // 4
        for j in range(4):
            sl = slice(j * CL, (j + 1) * CL)
            nc.sync.dma_start(out=vfull[:, 0, sl], in_=v_in[0:128, sl])
            nc.gpsimd.dma_start(out=vfull[:64, 1, sl], in_=v_in[128:192, sl])

        for c0 in range(0, NS, 512):
            for mb in range(2):
                msz = 128 if mb == 0 else 64
                ps = pspool.tile([128, 512], F32, tag="ps")
                nc.tensor.matmul(ps[:msz, :], aw[:, :, mb * 128:mb * 128 + msz],
                                 vfull[:, :, c0:c0 + 512], start=True, stop=True,
                                 perf_mode=DR)
                if (c0 // 512 + mb) % 2 == 0:
                    nc.scalar.activation(out=ofull[:msz, mb, c0:c0 + 512],
                                         in_=ps[:msz, :],
                                         func=mybir.ActivationFunctionType.Identity,
                                         bias=pb[:msz, mb:mb + 1],
                                         scale=1.0 / (AWS * VS))
                else:
                    nc.vector.tensor_scalar(
                        out=ofull[:msz, mb, c0:c0 + 512], in0=ps[:msz, :],
                        scalar1=1.0 / (AWS * VS), scalar2=pb[:msz, mb:mb + 1],
                        op0=mybir.AluOpType.mult, op1=mybir.AluOpType.add)
            if c0 % 2048 == 2048 - 512:
                b0 = c0 + 512 - 2048
                eng = nc.sync if (b0 // 2048) % 2 == 0 else nc.gpsimd
                eng.dma_start(out=out[0:128, b0:b0 + 2048], in_=ofull[:, 0, b0:b0 + 2048])
                eng.dma_start(out=out[128:192, b0:b0 + 2048], in_=ofull[:64, 1, b0:b0 + 2048])
    nc.compile()
    return nc


# ---------------- host-side prep ----------------

def prep_weights(w):
    """w: dict of reference weights (numpy f32). Returns dict of L1 input arrays."""
    out = {}
    qw = w["q_w"][:, :, 0, 0]          # (576, 192)
    qwT = np.zeros((128, 2, 768), np.float32)
    for (pb, h, x1b, qb, ch0, nch) in qkv_halves():
        win = 128 * pb + 64 * h
        qwT[0:128, 0, win:win + nch] = qw.T[0:128, ch0:ch0 + nch]
        qwT[0:64, 1, win:win + nch] = qw.T[128:192, ch0:ch0 + nch]
    out["qw8"] = (qwT * WS).astype(f8)

    # qdw: grouped 3x3, (tap, pb) 128x128 block; repack into DR tap pairs
    qdw = w["qdw_w"]                   # (576, 3, 3, 3)
    qdwT = np.zeros((9, 6, 128, 128), np.float32)   # (tap_sorted, pb, row, col)
    for ti, (dy, dx) in enumerate(TAPS):
        for (pb, h, x1b, qb, ch0, nch) in qkv_halves():
            for gl in range(nch // 3):
                for i in range(3):
                    for j in range(3):
                        qdwT[ti, pb, x1b + 3 * gl + i, qb + 3 * gl + j] = \
                            qdw[ch0 + 3 * gl + j, i, dy + 1, dx + 1]
    qdw8 = np.zeros((128, 6, 5, 2, 128), np.float32)
    for pj, (ta, tb) in enumerate(BD_PAIRS):
        for pb in range(6):
            qdw8[:, pb, pj, 0, :] = qdwT[ta, pb]
            if tb is not None:
                qdw8[:, pb, pj, 1, :] = qdwT[tb, pb]
    out["qdw8"] = (qdw8 * WS).astype(f8)

    # kv conv: 14 k-tiles -> 7 DR pairs (order must match CPAIRS in build_l1)
    kvw = w["kv_w"]                    # (384, 192, 3, 3)
    tl = lambda dy, dx: kvw[:, 0:128, dy + 1, dx + 1].T       # (128, 384) lo
    th = lambda dy, dx: kvw[:, 128:192, dy + 1, dx + 1].T     # (64, 384) hi
    ktiles = []
    for dy, dx in TAPS:   # 9 lo tiles (sorted tap order == TAPS order)
        ktiles.append(("lo", (dy, dx)))
    ktiles.append(("d1", -1)); ktiles.append(("d1", 0)); ktiles.append(("d1", 1))
    ktiles.append(("single", None)); ktiles.append(("d2", None))
    kv8 = np.zeros((128, 7, 2, 384), np.float32)
    for pj in range(7):
        for j in range(2):
            kind, arg = ktiles[2 * pj + j]
            blk = np.zeros((128, 384), np.float32)
            if kind == "lo":
                blk[0:128] = tl(*arg)
            elif kind == "d1":
                blk[0:64] = th(-1, arg)
                blk[64:128] = th(0, arg)
            elif kind == "d2":
                blk[0:64] = th(1, -1)
                blk[64:128] = th(1, 1)
            elif kind == "single":
                blk[0:64] = th(1, 0)
            kv8[:, pj, j, :] = blk
    out["kv8"] = (kv8 * WS).astype(f8)

    # kvdw depthwise: plain layout, diag blocks per pblock, DR tap pairs
    kvdw = w["kvdw_w"][:, 0]           # (384, 3, 3)
    kvdw8 = np.zeros((128, 3, 5, 2, 128), np.float32)
    for pj, (ta, tb) in enumerate(BD_PAIRS):
        for pb in range(3):
            dya, dxa = TAPS[ta]
            kvdw8[:, pb, pj, 0, :] = np.diag(kvdw[128 * pb:128 * pb + 128, dya + 1, dxa + 1])
            if tb is not None:
                dyb, dxb = TAPS[tb]
                kvdw8[:, pb, pj, 1, :] = np.diag(kvdw[128 * pb:128 * pb + 128, dyb + 1, dxb + 1])
    out["kvdw8"] = (kvdw8 * WS).astype(f8)

    def dw_channel(pb, p):
        return 128 * pb + p   # plain layout

    # newk (no bias on device; kT = NKS * k0): contraction sources
    # pass0 = (k8 slot0 = qkv pb2, k8 slot1 = qkv pb3)
    # pass1 = (dwk8 slot0 = dw pb0, dwk8 slot1 = dw pb1 parts 0..63)
    KCC = [("qkv", 2, 0, 128), ("qkv", 3, 0, 128), ("dw", 0, 0, 128), ("dw", 1, 0, 64)]
    nk = w["newk_w"][:, :, 0, 0]       # (192, 384): in = [k(192) | k_mask(192)]
    nkm = np.zeros((128, 4, 192), np.float32)
    for j, (src, pb, base, sz) in enumerate(KCC):
        for p in range(base, base + sz):
            if src == "qkv":
                ch = qkv_channel_at(pb, p)
                if ch is not None:
                    nkm[p, j, :] = nk[:, ch - 192]      # k part: qkv ch 192-383
            else:
                ch = dw_channel(pb, p)
                if ch < 192:
                    nkm[p, j, :] = nk[:, 192 + ch]      # k_mask: dw ch 0-191
    out["nkw8"] = (nkm.reshape(128, 2, 2, 192) * NKS).astype(f8)

    nv = w["newv_w"][:, :, 0, 0]       # (192, 384): in = [v(192) | v_mask(192)]
    nvT = np.zeros((128, 4, 192), np.float32)
    VCC_P = [("qkv", 4, 0, 128), ("qkv", 5, 0, 128), ("dw", 1, 64, 64), ("dw", 2, 0, 128)]
    for j, (src, pb, base, sz) in enumerate(VCC_P):
        for p in range(base, base + sz):
            if src == "qkv":
                ch = qkv_channel_at(pb, p)
                if ch is not None:
                    nvT[p, j, :] = nv[:, ch - 384]      # v part: qkv ch 384-575
            else:
                ch = dw_channel(pb, p)
                if ch >= 192:
                    nvT[p, j, :] = nv[:, ch]            # v_mask: dw ch 192-383
    out["nvw8"] = (nvT.reshape(128, 2, 2, 192) * NVS).astype(f8)

    out["ident"] = np.eye(128, dtype=bf16)

    x1b = np.zeros((128, 6), np.float32)
    qkvb = np.zeros((128, 6), np.float32)
    for (pb, h, x1b_base, qb, ch0, nch) in qkv_halves():
        x1b[x1b_base:x1b_base + nch, pb] = w["q_b"][ch0:ch0 + nch]
        qkvb[qb:qb + nch, pb] = w["qdw_b"][ch0:ch0 + nch]
    # hijacked ones rows: junk q pb1 partitions become the exact constant
    # 1/QTS, which the qT8 scale turns into 1.0 -> gramB rows = ksum/kss
    for r in ONE_ROWS:
        qkvb[r, 1] = 1.0 / QTS
    out["x1_bias"] = x1b * WS
    out["qkv_bias"] = qkvb

    kvb = np.zeros((128, 3), np.float32)
    dwb = np.zeros((128, 3), np.float32)
    for pb in range(3):
        kvb[:, pb] = w["kv_b"][128 * pb:128 * pb + 128]
        dwb[:, pb] = w["kvdw_b"][128 * pb:128 * pb + 128]
    out["kv_bias"] = kvb * WS
    out["kvdw_bias"] = dwb
    nvb = np.zeros((128, 2), np.float32)
    nvb[:, 0] = w["newv_b"][0:128]
    nvb[0:64, 1] = w["newv_b"][128:192]
    out["newv_bias"] = nvb * VS
    return out


def prep_core(x, xm, b, half, R, H):
    xp = np.zeros((192, R + 2, WP), np.float32)
    mp = np.zeros((192, R + 4, WP), np.float32)
    for r in range(R + 2):
        g = half * R + (r - 1)
        if 0 <= g < H:
            xp[:, r, 1:129] = x[b, :, g, :]
    for r in range(R + 4):
        g = half * R + (r - 2)
        if 0 <= g < H:
            mp[:, r, 1:129] = xm[b, :, g, :]
    xp = xp.reshape(192, -1)
    x8 = np.zeros((128, 2, xp.shape[1]), np.float32)
    x8[:, 0, :] = xp[0:128]
    x8[0:64, 1, :] = xp[128:192]
    x8[64:128, 1, :] = xp[128:192]   # dup (weights zero) to avoid NaN garbage
    mp = mp.reshape(192, -1)
    L = mp.shape[1]
    hi = mp[128:192]
    d1 = np.zeros((128, L), np.float32)
    d2 = np.zeros((128, L), np.float32)
    d1[0:64] = hi
    d1[64:128, :L - WP] = hi[:, WP:]
    d2[0:64] = hi
    d2[64:128, :L - 2] = hi[:, 2:]
    edge = np.ones((128, 2), np.float32)
    if half == 0:
        edge[:, 0] = 0.0     # slab0 row0 = image row -1
    else:
        edge[:, 1] = 0.0     # last slab last row = image row H
    return {
        "x8": x8.astype(f8),
        "xm_lo": mp[0:128].astype(f8),
        "xm_d1": d1.astype(f8), "xm_d2": d2.astype(f8),
        "edge": edge,
    }


# ---------------- host glue (unchanged semantics) ----------------

def _q_maps():
    part = np.zeros(192, np.int64)
    pblk = np.zeros(192, np.int64)
    for (pb, h, x1b, qb, ch0, nch) in qkv_halves():
        if pb >= 2:
            continue
        for i in range(nch):
            pblk[ch0 + i] = pb
            part[ch0 + i] = qb + i
    return pblk, part


def _ss_from_qstats(stats, n_half):
    pblk, part = _q_maps()
    mv = stats.astype(np.float64)
    return (mv[part, pblk, 1] + mv[part, pblk, 0] ** 2) * n_half


def _sum_from_qstats(stats, n_half):
    pblk, part = _q_maps()
    mv = stats.astype(np.float64)
    return mv[part, pblk, 0] * n_half


def _ss_from_vstats(stats, newv_b, n_half):
    # stats are of NVS*(v - b): undo the scale, add back the bias
    m = np.zeros(192, np.float64)
    w = np.zeros(192, np.float64)
    mv = stats.astype(np.float64)
    m[0:128] = mv[0:128, 0, 0]
    w[0:128] = mv[0:128, 0, 1]
    m[128:192] = mv[0:64, 1, 0]
    w[128:192] = mv[0:64, 1, 1]
    m = m / NVS + newv_b.astype(np.float64)
    w = w / (NVS * NVS)
    return (w + m * m) * n_half


def glue(res0, res1, newv_b, n_half):
    """Combine two half-core L1 results into raw sums."""
    GT = res0["gramT_out"].astype(np.float64) + res1["gramT_out"].astype(np.float64)
    pblk, part = _q_maps()
    qrow = pblk * 128 + part
    # GT rows are qT cols (pb*128+part); cols are newk out-ch d. Stored values
    # are sum_n (QTS*q) * (NKS*k0).
    Gq = GT[qrow, :] / (QTS * NKS)             # (c, d): sum_n q[c,n] k0[d,n]
    qss = _ss_from_qstats(res0["qstats_out"], n_half) + _ss_from_qstats(res1["qstats_out"], n_half)
    qsum = _sum_from_qstats(res0["qstats_out"], n_half) + _sum_from_qstats(res1["qstats_out"], n_half)
    vss = _ss_from_vstats(res0["vstats_out"], newv_b, n_half) + \
        _ss_from_vstats(res1["vstats_out"], newv_b, n_half)
    kss_raw = (res0["kss_out"].astype(np.float64) + res1["kss_out"].astype(np.float64))[0]
    k0sum = kss_raw[0:192] / NKS               # sum_n k0 (ones x kT cols)
    k0ss = kss_raw[192:384] / (KSQS * KSQS)    # sum_n k0^2 (ones x ksq cols)
    return Gq, qss, qsum, vss, k0ss, k0sum


def glue_full(res0, res1, temperature, newk_b, newv_b, proj_w, proj_b, n_half):
    Gq, qss, qsum, vss, k0ss, k0sum = glue(res0, res1, newv_b, n_half)
    b = newk_b.astype(np.float64)              # (192,)
    # k = k0 + b: gram/kss bias corrections (sums already cover both halves,
    # total n = 2 * n_half)
    G = Gq + qsum[:, None] * b[None, :]        # (c, d): sum_n q k
    kss = k0ss + 2 * b * k0sum + (2 * n_half) * b * b
    qn = np.maximum(np.sqrt(qss), 1e-12)
    kn = np.maximum(np.sqrt(kss), 1e-12)
    vn = np.maximum(np.sqrt(vss), 1e-12)
    A = G / (qn[:, None] * kn[None, :])        # (c, d)
    M = np.zeros((192, 192), np.float64)
    t = np.asarray(temperature).reshape(-1)
    for h in range(8):
        sl = slice(24 * h, 24 * h + 24)
        a = A[sl, sl] * t[h]
        a = a - a.max(axis=-1, keepdims=True)
        e = np.exp(a)
        sm = e / e.sum(axis=-1, keepdims=True)
        M[sl, sl] = sm / vn[None, sl]
    At = proj_w[:, :, 0, 0].astype(np.float64) @ M   # (out-ch o, d)
    awT = np.zeros((128, 2, 192), np.float32)
    awT[:, 0, :] = At.T[0:128] * AWS
    awT[0:64, 1, :] = At.T[128:192] * AWS
    pbias = np.zeros((128, 2), np.float32)
    pbias[:, 0] = proj_b[0:128]
    pbias[0:64, 1] = proj_b[128:192]
    return {"awT": awT.astype(f8), "pbias": pbias}


# ---------------- driver: kernel(**inputs) ----------------
from concourse.bass_utils import run_bass_kernel_spmd

R_FULL, H_FULL, B_FULL = 64, 128, 4
_NC1 = None
_NC2 = None


def _get_progs():
    global _NC1, _NC2
    if _NC1 is None:
        _NC1 = build_l1(R=R_FULL, S=16)
        _NC2 = build_l2(R=R_FULL)
    return _NC1, _NC2


def kernel(**inputs):
    inputs = {k: np.asarray(v) for k, v in inputs.items()}
    x, xm = inputs["x"], inputs["x_mask"]
    nc1, nc2 = _get_progs()
    wprep = prep_weights(inputs)
    in_maps = []
    for core in range(8):
        b, half = core // 2, core % 2
        m = dict(wprep)
        m.update(prep_core(x, xm, b, half, R_FULL, H_FULL))
        in_maps.append(m)
    res1 = run_bass_kernel_spmd(nc1, in_maps, list(range(8))).results

    n_half = R_FULL * 128
    in_maps2 = []
    for core in range(8):
        b, half = core // 2, core % 2
        if half == 0:
            l2c = glue_full(res1[2 * b], res1[2 * b + 1], inputs["temperature"],
                            inputs["newk_b"], inputs["newv_b"],
                            inputs["proj_w"], inputs["proj_b"], n_half)
        m = dict(l2c)
        m["v_in"] = res1[core]["v_out"]
        in_maps2.append(m)
    res2 = run_bass_kernel_spmd(nc2, in_maps2, list(range(8))).results

    out = np.empty((B_FULL, 192, H_FULL, 128), np.float32)
    for core in range(8):
        b, half = core // 2, core % 2
        out[b, :, half * R_FULL:(half + 1) * R_FULL, :] = \
            res2[core]["out"].reshape(192, R_FULL, 128).astype(np.float32)
    return out


# revision 47
# speedup vs baseline: 1.1480x; 1.0015x over previous
"""Trainium2 Bass kernel for nn_Cross_Attention — fp8 DoubleRow rewrite.

L1: convs + gram partials with fp8 DoubleRow matmuls (phases A-D), fp8 F-phase
(newk/gram/kss via DR), bf16 E. L2: attn-apply with chunked v DMA. Host glue
between (softmax etc.). Sharding: 4 samples x 2 row-halves across 8 cores.
"""
import sys
sys.path.insert(0, "/opt/trn_rl_repo")
import numpy as np
import ml_dtypes

import concourse.bass as bass
import concourse.tile as tile
from concourse import bacc, mybir
from contextlib import ExitStack

BF16 = mybir.dt.bfloat16
F8 = mybir.dt.float8e4
F32 = mybir.dt.float32
bf16 = ml_dtypes.bfloat16
f8 = ml_dtypes.float8_e4m3
DR = mybir.MatmulPerfMode.DoubleRow

WS = 16.0          # fp8 weight pre-scale
WP = 130           # padded row width
NKS = 32.0         # newk weight pre-scale (kT stored as NKS*k0)
QTS = 16.0         # qT pre-scale (qT stored as QTS*q)
KSQS = 64.0        # ksq stored as (KSQS*k0)**2
NVS = 32.0         # newv weight pre-scale
VS = 8.0           # v8 storage scale (v stored as VS*v)
AWS = 512.0        # attn-weight (aw8) pre-scale
ONE_ROWS = (32, 33)  # junk q pb1 partitions hijacked as 1.0 cols in qT8 (ksum/kss);
                     # must be 32-aligned (engine partition-base alignment)

TAPS = [(dy, dx) for dy in (-1, 0, 1) for dx in (-1, 0, 1)]
# taps sorted by flat offset dy*WP+dx (they already are, given WP>2)
TAP_OFF = [dy * WP + dx for dy, dx in TAPS]
# DoubleRow tap pairs for B/D: (idx_a, idx_b). The odd tap rides first with a
# zero-weight second tile at stride +1 (always in-bounds since tap 0 has the
# smallest offset).
BD_PAIRS = [(0, None), (1, 2), (3, 4), (5, 6), (7, 8)]


def qkv_halves():
    """Per (pb, half): (x1_base, qkv_base, ch0, nch).  ch0 = qkv-global channel."""
    out = []
    for pb in range(6):
        P, odd = pb // 2, pb % 2
        for h in (0, 1):
            nch = 3 if (odd and h == 1) else 63
            ch0 = 3 * (64 * P + 42 * odd + 21 * h)
            x1b = 64 * h
            qb = 64 * h if not odd else 64 * (1 - h)
            out.append((pb, h, x1b, qb, ch0, nch))
    return out


def qkv_channel_at(pb, p):
    """qkv-global channel stored at partition p of qkv pblock pb, or None."""
    for (pb2, h, x1b, qb, ch0, nch) in qkv_halves():
        if pb2 == pb and qb <= p < qb + nch:
            return ch0 + (p - qb)
    return None


# newk/newv input chunks with PLAIN dw layout (dw pb_i = kv ch 128i..128i+127):
# k = qkv ch 192-383 (pb 2,3); k_mask = kv ch 0-191 = dw pb0 + dw pb1[0:64]
# v = qkv ch 384-575 (pb 4,5); v_mask = kv ch 192-383 = dw pb1[64:128] + dw pb2
# Tile routing: q_t = qkv pb0,1 (bf16); k8 = qkv pb2,3 (f8, chunk-major);
# vq = qkv pb4,5 (f8); dwk8 = dw pb0 + dw pb1[0:64] (f8, chunk-major);
# dwv = dw pb1[64:128] + dw pb2 (f8, slot0 parts 64-127 used).


def _bcast(ap, p):
    return bass.AP(tensor=ap.tensor, offset=ap.offset, ap=[[0, p]] + list(ap.ap[1:]))


def _pair_ap(t, off, delta, dims):
    """AP over tile t at flat free-offset `off`: [partitions, 2 (stride delta), *dims]."""
    return bass.AP(tensor=t.tensor, offset=t.offset + off,
                   ap=[list(t.ap[0]), [delta, 2]] + [list(d) for d in dims])


def build_l1(R=64, W=128, S=16):
    assert W == 128 and R % S == 0 and S % 4 == 0
    NSLAB = R // S
    NS = S * W
    XCOLS = (S + 2) * WP
    MCOLS = (S + 4) * WP
    MC2 = MCOLS + 2
    N128 = NS // 128

    nc = bacc.Bacc("TRN2", target_bir_lowering=False, debug=False, num_devices=8)

    def din(name, shape, dt=F8):
        return nc.dram_tensor(name, shape, dt, kind="ExternalInput").ap()

    def dout(name, shape, dt=F32):
        return nc.dram_tensor(name, shape, dt, kind="ExternalOutput").ap()

    x8 = din("x8", [128, 2, (R + 2) * WP])
    xm_lo = din("xm_lo", [128, (R + 4) * WP])
    xm_d1 = din("xm_d1", [128, (R + 4) * WP])
    xm_d2 = din("xm_d2", [128, (R + 4) * WP])
    qw8 = din("qw8", [128, 2, 768])
    qdw8 = din("qdw8", [128, 6, 5, 2, 128])
    kv8 = din("kv8", [128, 7, 2, 384])
    kvdw8 = din("kvdw8", [128, 3, 5, 2, 128])
    nkw8 = din("nkw8", [128, 2, 2, 192], F8)
    nvw8 = din("nvw8", [128, 2, 2, 192], F8)
    ident = din("ident", [128, 128], BF16)
    x1_bias = din("x1_bias", [128, 6], F32)      # x16
    qkv_bias = din("qkv_bias", [128, 6], F32)
    kv_bias = din("kv_bias", [128, 3], F32)      # x16, plain layout
    kvdw_bias = din("kvdw_bias", [128, 3], F32)  # plain layout
    newv_bias = din("newv_bias", [128, 2], F32)  # x VS
    edge = din("edge", [128, 2], F32)            # per-core edge-row multipliers

    v_out = dout("v_out", [192, R * W], F8)
    gramT_out = dout("gramT_out", [256, 192])    # rows = qcol space (pb*128+part)
    kss_out = dout("kss_out", [1, 384])          # [sum (KSQS*k0)^2 | sum NKS*k0]
    qstats_out = dout("qstats_out", [128, 2, 2])
    vstats_out = dout("vstats_out", [128, 2, 2])

    # C-phase DoubleRow k-tile pair coordinates in the [128, 3, MC2] msl tile
    # flat free space: region*MC2 + off.  Region 0 = lo channels, 1 = d1
    # (hi | hi<<WP), 2 = d2 (hi | hi<<2).
    def lo(dy, dx):
        return (1 + dy) * WP + dx + 1

    def d1(off):
        return MC2 + off

    def d2(off):
        return 2 * MC2 + off

    # 14 k-tiles -> 7 pairs; weight slot kv8[:, pair, j, :] must match.
    CPAIRS = [
        (lo(-1, -1), lo(-1, 0)), (lo(-1, 1), lo(0, -1)), (lo(0, 0), lo(0, 1)),
        (lo(1, -1), lo(1, 0)), (lo(1, 1), d1(0)), (d1(1), d1(2)),
        (d1(2 * WP + 1), d2(2 * WP)),
    ]

    with tile.TileContext(nc) as tc, ExitStack() as ctx:
        wpool = ctx.enter_context(tc.tile_pool(name="weights", bufs=1))
        xpool = ctx.enter_context(tc.tile_pool(name="xslab", bufs=2))
        bigpool = ctx.enter_context(tc.tile_pool(name="big", bufs=1))
        midpool = ctx.enter_context(tc.tile_pool(name="mid", bufs=2))
        smpool = ctx.enter_context(tc.tile_pool(name="small", bufs=4))
        statpool = ctx.enter_context(tc.tile_pool(name="stats", bufs=1))
        pspool = ctx.enter_context(tc.tile_pool(name="ps", bufs=6, space="PSUM"))
        pspers = ctx.enter_context(tc.tile_pool(name="pspers", bufs=1, space="PSUM"))

        def load1(ap_in, shape, dt=F8, eng=None):
            t = wpool.tile(shape, dt, tag=ap_in.tensor.name)
            (eng or nc.sync).dma_start(out=t[:ap_in.shape[0]], in_=ap_in[:])
            return t

        g = nc.gpsimd
        qw8_s = load1(qw8, [128, 2, 768], eng=g)
        x1b_s = load1(x1_bias, [128, 6], F32, eng=g)
        qdw8_s = load1(qdw8, [128, 6, 5, 2, 128], eng=g)
        kv8_s = load1(kv8, [128, 7, 2, 384], eng=g)
        kvdw8_s = load1(kvdw8, [128, 3, 5, 2, 128], eng=g)
        nkw8_s = load1(nkw8, [128, 2, 2, 192], F8, eng=g)
        nvw8_s = load1(nvw8, [128, 2, 2, 192], F8, eng=g)
        id_s = load1(ident, [128, 128], BF16, eng=g)
        qkvb_s = load1(qkv_bias, [128, 6], F32, eng=g)
        kvb_s = load1(kv_bias, [128, 3], F32, eng=g)
        dwb_s = load1(kvdw_bias, [128, 3], F32, eng=g)
        nvb_s = load1(newv_bias, [128, 2], F32, eng=g)
        edge_s = load1(edge, [128, 2], F32, eng=g)

        # persistent accumulators: one PSUM bank each (zero-region granularity)
        # gramB is 384 wide: cols 192:384 see the ksq half of the moving data;
        # its ONE_ROWS rows (ones in the stationary) yield ksum/kss.
        gramA = pspers.tile([128, 192], F32)
        gramB = pspers.tile([128, 384], F32)

        qstats = statpool.tile([128, 2, NSLAB * (NS // 512), 6], F32)
        vstats = statpool.tile([128, 2, NSLAB * (NS // 512), 6], F32)
        # double-buffered chunk-pair staging for gram/kss (dims: pairbuf, slot).
        # inner dim padded to 400 so the DR pair stride is NOT contiguous with
        # the column dim (contiguous dims get re-flattened in lowering, which
        # breaks the pair interpretation of the moving AP).
        kq8 = statpool.tile([128, 2, 2, 400], F8)    # [0:192]=NKS*k0, [192:384]=ksq
        qT8 = statpool.tile([128, 2, 2, 256], F8)

        n128_total = NSLAB * N128
        CT = [(c0, min(512, XCOLS - c0)) for c0 in range(0, XCOLS, 512)]
        # slabs > 0 reuse the previous slab's last 2 rows of x1/kv1 (halo
        # copy) and only compute the remaining S rows
        CT2 = [(2 * WP + 416 * k, 416) for k in range(5)]
        x1 = bigpool.tile([128, 6, XCOLS], F8, tag="x1")
        kv1 = bigpool.tile([128, 3, XCOLS], F8, tag="kv1")

        for s in range(NSLAB):
            xsl = xpool.tile([128, 2, XCOLS], F8, tag="xsl")
            msl = xpool.tile([128, 3, MC2], F8, tag="msl")
            off = s * S * WP
            # spread the slab input loads over several DMA queues; slab 0's
            # x load is split so the first phase-A matmuls gate on only the
            # first half (sync's queue is otherwise empty at start)
            if s == 0:
                h = 9 * WP
                nc.sync.dma_start(out=xsl[:, :, 0:h], in_=x8[:, :, off:off + h])
                nc.sync.dma_start(out=xsl[:, :, h:XCOLS],
                                  in_=x8[:, :, off + h:off + XCOLS])
            else:
                nc.sync.dma_start(out=xsl[:], in_=x8[:, :, off:off + XCOLS])
            nc.scalar.dma_start(out=msl[:, 0, 1:1 + MCOLS], in_=xm_lo[:, off:off + MCOLS])
            nc.gpsimd.dma_start(out=msl[:, 1, 1:1 + MCOLS], in_=xm_d1[:, off:off + MCOLS])
            nc.scalar.dma_start(out=msl[:, 2, 1:1 + MCOLS], in_=xm_d2[:, off:off + MCOLS])
            for r in range(3):
                nc.vector.memset(msl[:, r, 0:1], 0.0)
                nc.vector.memset(msl[:, r, MC2 - 1:MC2], 0.0)

            def pad_zero(t, npb):
                # zero the 2 pad columns of every row: flat r*WP + {0, 129}
                ap = bass.AP(tensor=t.tensor, offset=t.offset,
                             ap=[list(t.ap[0]), [XCOLS, npb], [WP, S + 2], [WP - 1, 2]])
                nc.vector.memset(ap, 0.0)
                # per-core edge pad row: slab0 row0 (half 0) / slab3 last row
                # (half 1), selected by the `edge` input multipliers
                if s == 0:
                    nc.vector.tensor_scalar(
                        out=t[:, :, 0:WP], in0=t[:, :, 0:WP],
                        scalar1=edge_s[:, 0:1], scalar2=None,
                        op0=mybir.AluOpType.mult)
                if s == NSLAB - 1:
                    nc.vector.tensor_scalar(
                        out=t[:, :, (S + 1) * WP:(S + 2) * WP],
                        in0=t[:, :, (S + 1) * WP:(S + 2) * WP],
                        scalar1=edge_s[:, 1:2], scalar2=None,
                        op0=mybir.AluOpType.mult)

            # ---- Phase A: x1 = 1x1(x); one DR matmul per (pb, col-tile)
            if s > 0:
                nc.sync.dma_start(out=x1[:, :, 0:2 * WP],
                                  in_=x1[:, :, S * WP:(S + 2) * WP])
                nc.gpsimd.dma_start(out=kv1[:, :, 0:2 * WP],
                                    in_=kv1[:, :, S * WP:(S + 2) * WP])
            for pb in range(6):
                for c0, cs in (CT if s == 0 else CT2):
                    ps = pspool.tile([128, 512], F32, tag="ps", name=f"psA{s}_{pb}_{c0}")
                    nc.tensor.matmul(ps[:, :cs], qw8_s[:, :, 128 * pb:128 * pb + 128],
                                     xsl[:, :, c0:c0 + cs], start=True, stop=True,
                                     perf_mode=DR)
                    nc.vector.tensor_scalar(
                        out=x1[:, pb, c0:c0 + cs], in0=ps[:, :cs],
                        scalar1=x1b_s[:, pb:pb + 1], scalar2=1.0 / WS,
                        op0=mybir.AluOpType.add, op1=mybir.AluOpType.mult)
            pad_zero(x1, 6)

            # ---- Phase C: kv1 = 3x3(xm); 7 DR pairs per (pb, col-tile)
            for pb in range(3):
                for cset in ((CT[:3], CT[3:]) if s == 0 else (CT2[:3], CT2[3:])):
                    pss = [pspool.tile([128, 512], F32, tag="ps",
                                       name=f"psC{s}_{pb}_{c0}")
                           for (c0, cs) in cset]
                    for pj, (ca, cb) in enumerate(CPAIRS):
                        lhsT = kv8_s[:, pj, :, 128 * pb:128 * pb + 128]
                        for ti, (c0, cs) in enumerate(cset):
                            rhs = _pair_ap(msl, ca + c0, cb - ca, [[1, cs]])
                            nc.tensor.matmul(pss[ti][:, :cs], lhsT, rhs,
                                             start=(pj == 0), stop=(pj == 6),
                                             perf_mode=DR)
                    for ti, (c0, cs) in enumerate(cset):
                        nc.vector.tensor_scalar(
                            out=kv1[:, pb, c0:c0 + cs], in0=pss[ti][:, :cs],
                            scalar1=kvb_s[:, pb:pb + 1], scalar2=1.0 / WS,
                            op0=mybir.AluOpType.add, op1=mybir.AluOpType.mult)
            pad_zero(kv1, 3)

            # ---- Phase B: qkv = qdw(x1); 2-row groups (N=258), 5 DR tap-pairs
            # per group chained in PSUM; 4-group sets amortize weight loads.
            # route: pb -> list of (dst_tile, slot, part_lo, part_hi)
            def dwconv(src, wsrc, route, npb, bias_s, tag):
                for pb in range(npb):
                    for st in range(2):
                        gset = list(range(st * 4, st * 4 + 4))
                        pss = {gi: pspool.tile([128, 258], F32, tag="ps",
                                               name=f"ps{tag}{s}_{pb}_{gi}")
                               for gi in gset}
                        for pj, (ta, tb) in enumerate(BD_PAIRS):
                            oa = TAP_OFF[ta]
                            delta = (TAP_OFF[tb] - oa) if tb is not None else 1
                            lhsT = wsrc[:, pb, pj, :, :]
                            for gi in gset:
                                base = pb * XCOLS + (2 * gi + 1) * WP + 1 + oa
                                rhs = _pair_ap(src, base, delta, [[1, 258]])
                                nc.tensor.matmul(pss[gi][:, :], lhsT, rhs,
                                                 start=(pj == 0), stop=(pj == 4),
                                                 perf_mode=DR)
                        for gi in gset:
                            ps = pss[gi]
                            in_ap = bass.AP(tensor=ps.tensor, offset=ps.offset,
                                            ap=[list(ps.ap[0]), [WP, 2], [1, 128]])
                            for (dst, slot, plo, phi, chunked) in route(pb):
                                if chunked:
                                    out_ap = dst[plo:phi, 2 * gi:2 * gi + 2, slot, :]
                                else:
                                    out_ap = dst[plo:phi, slot,
                                                 2 * gi * 128:(2 * gi + 2) * 128]
                                nc.scalar.activation(
                                    out=out_ap,
                                    in_=in_ap[plo:phi],
                                    func=mybir.ActivationFunctionType.Identity,
                                    bias=bias_s[plo:phi, pb:pb + 1], scale=1.0 / WS)

            # k8/dwk8 are chunk-major [128, N128, 2, 128] so the kps DR
            # stationary pair is contiguous (pair stride 128 — ISA requires
            # small pair strides for dual-fp8 ldweights).
            q_t = bigpool.tile([128, 2, NS], BF16, tag="q_t")
            k8 = bigpool.tile([128, N128, 2, 128], F8, tag="k8")
            vq = bigpool.tile([128, 2, NS], F8, tag="vq")

            def qkv_route(pb):
                dst = (q_t, k8, vq)[pb // 2]
                return [(dst, pb % 2, 0, 128, pb // 2 == 1)]

            dwconv(x1, qdw8_s, qkv_route, 6, qkvb_s, "B")

            # ---- Phase D: dw = kvdw(kv1); same structure, 3 pblocks
            dwk8 = bigpool.tile([128, N128, 2, 128], F8, tag="dwk8")
            dwv = bigpool.tile([128, 2, NS], F8, tag="dwv")
            nc.vector.memset(dwk8[64:128, :, 1, :], 0.0)
            nc.vector.memset(dwv[0:64, 0, :], 0.0)

            def dw_route(pb):
                if pb == 0:
                    return [(dwk8, 0, 0, 128, True)]
                if pb == 1:
                    return [(dwk8, 1, 0, 64, True), (dwv, 0, 64, 128, False)]
                return [(dwv, 1, 0, 128, False)]

            dwconv(kv1, kvdw8_s, dw_route, 3, dwb_s, "D")

            # ---- Phase E: v = newv(v_cc), fp8 DR (pair over the two source
            # tiles of each pass); psum holds NVS*(v - b); bn_stats on psum.
            vt = midpool.tile([128, 2, NS], F8, tag="vt")
            for mb in range(2):
                msz = 128 if mb == 0 else 64
                for ic in range(NS // 512):
                    sl = slice(ic * 512, (ic + 1) * 512)
                    ps = pspool.tile([128, 512], F32, tag="ps",
                                     name=f"psE{s}_{mb}_{ic}")
                    nc.tensor.matmul(ps[:msz, :],
                                     nvw8_s[:, 0, :, mb * 128:mb * 128 + msz],
                                     vq[:, :, sl], start=True, stop=False,
                                     perf_mode=DR)
                    nc.tensor.matmul(ps[:msz, :],
                                     nvw8_s[:, 1, :, mb * 128:mb * 128 + msz],
                                     dwv[:, :, sl], start=False, stop=True,
                                     perf_mode=DR)
                    si = s * (NS // 512) + ic
                    nc.vector.bn_stats(out=vstats[:msz, mb, si, :], in_=ps[:msz, :])
                    nc.scalar.activation(out=vt[:msz, mb, sl], in_=ps[:msz, :],
                                         func=mybir.ActivationFunctionType.Identity,
                                         bias=nvb_s[:msz, mb:mb + 1], scale=VS / NVS)
            nc.sync.dma_start(out=v_out[0:128, s * NS:(s + 1) * NS], in_=vt[:, 0, :])
            nc.sync.dma_start(out=v_out[128:192, s * NS:(s + 1) * NS], in_=vt[:64, 1, :])
            for sub in range(NS // 512):
                si = s * (NS // 512) + sub
                sl = slice(sub * 512, (sub + 1) * 512)
                nc.vector.bn_stats(out=qstats[:, 0, si, :], in_=q_t[:, 0, sl])
                nc.vector.bn_stats(out=qstats[:, 1, si, :], in_=q_t[:, 1, sl])

            # ---- Phase F: per 128-n chunk: kT (newk, fp8 DR), qT (transpose),
            # then per chunk-pair: gram (fp8 DR) + kss.  Gram for pair P is
            # emitted one chunk late (during chunk 2P+2) so the scalar-engine
            # kq8/qT8 writes are done before the PE needs them.
            def emit_gram(gp):
                pb = gp % 2
                first, last = gp == 0, gp == n128_total // 2 - 1
                nc.tensor.matmul(gramA[:, :], qT8[:, pb, :, 0:128],
                                 kq8[:, pb, :, 0:192], start=first, stop=last,
                                 perf_mode=DR)
                nc.tensor.matmul(gramB[:, :], qT8[:, pb, :, 128:256],
                                 kq8[:, pb, :, 0:384], start=first, stop=last,
                                 perf_mode=DR)

            for ic in range(N128):
                c0 = ic * 128
                gidx = s * N128 + ic
                slot = ic % 2
                pbuf = (ic // 2) % 2
                kps = pspool.tile([128, 192], F32, tag="ps", name=f"kps{s}_{ic}")
                nc.tensor.matmul(kps[:, :], k8[:, ic, :, :], nkw8_s[:, 0, :, :],
                                 start=True, stop=False, perf_mode=DR)
                nc.tensor.matmul(kps[:, :], dwk8[:, ic, :, :], nkw8_s[:, 1, :, :],
                                 start=False, stop=True, perf_mode=DR)
                nc.scalar.copy(out=kq8[:, pbuf, slot, 0:192], in_=kps[:, :])
                nc.scalar.activation(out=kq8[:, pbuf, slot, 192:384], in_=kps[:, :],
                                     func=mybir.ActivationFunctionType.Square,
                                     scale=KSQS / NKS)
                qps = pspool.tile([128, 256], BF16, tag="ps", name=f"qps{s}_{ic}")
                nc.tensor.transpose(qps[:, 0:128], q_t[:, 0, c0:c0 + 128], id_s[:, :])
                nc.tensor.transpose(qps[:, 128:256], q_t[:, 1, c0:c0 + 128], id_s[:, :])
                nc.scalar.mul(out=qT8[:, pbuf, slot, :], in_=qps[:, :], mul=QTS)
                if slot == 0 and gidx >= 2:
                    emit_gram(gidx // 2 - 1)

        emit_gram(n128_total // 2 - 1)

        qmv = statpool.tile([128, 2, 2], F32)
        vmv = statpool.tile([128, 2, 2], F32)
        nc.vector.memset(qmv[:], 0.0)
        nc.vector.memset(vmv[:], 0.0)
        nc.vector.bn_aggr(out=qmv[:, 0, :], in_=qstats[:, 0, :, :])
        nc.vector.bn_aggr(out=qmv[:, 1, :], in_=qstats[:, 1, :, :])
        nc.vector.bn_aggr(out=vmv[:, 0, :], in_=vstats[:, 0, :, :])
        nc.vector.bn_aggr(out=vmv[:64, 1, :], in_=vstats[:64, 1, :, :])
        nc.sync.dma_start(out=qstats_out[:], in_=qmv[:])
        nc.sync.dma_start(out=vstats_out[:], in_=vmv[:])
        gA = statpool.tile([128, 192], F32)
        gB = statpool.tile([128, 192], F32)
        kssb = statpool.tile([128, 384], F32)
        nc.scalar.copy(out=gA[:], in_=gramA[:])
        nc.scalar.copy(out=gB[:], in_=gramB[:, 0:192])
        r0 = ONE_ROWS[0]
        nc.scalar.copy(out=kssb[r0:r0 + 1, :], in_=gramB[r0:r0 + 1, 0:384])
        nc.sync.dma_start(out=gramT_out[0:128, :], in_=gA[:])
        nc.sync.dma_start(out=gramT_out[128:256, :], in_=gB[:])
        nc.sync.dma_start(out=kss_out[:], in_=kssb[r0:r0 + 1, :])

    nc.compile()
    return nc


def build_l2(R=64, W=128):
    NS = R * W
    nc = bacc.Bacc("TRN2", target_bir_lowering=False, debug=False, num_devices=8)
    v_in = nc.dram_tensor("v_in", [256, NS], F8, kind="ExternalInput").ap()
    awT = nc.dram_tensor("awT", [128, 2, 192], F8, kind="ExternalInput").ap()
    pbias = nc.dram_tensor("pbias", [128, 2], F32, kind="ExternalInput").ap()
    out = nc.dram_tensor("out", [192, NS], BF16, kind="ExternalOutput").ap()

    with tile.TileContext(nc) as tc, ExitStack() as ctx:
        wpool = ctx.enter_context(tc.tile_pool(name="w", bufs=1))
        vpool = ctx.enter_context(tc.tile_pool(name="v", bufs=1))
        opool = ctx.enter_context(tc.tile_pool(name="o", bufs=1))
        pspool = ctx.enter_context(tc.tile_pool(name="ps", bufs=8, space="PSUM"))

        aw = wpool.tile([128, 2, 192], F8)
        nc.sync.dma_start(out=aw[:], in_=awT[:])
        pb = wpool.tile([128, 2], F32)
        nc.sync.dma_start(out=pb[:], in_=pbias[:])

        # batched: v lives in SBUF whole (fp8), loaded in chunks on 2 queues;
        # outputs accumulate in SBUF and leave in 2048-col chunks.
        vfull = vpool.tile([128, 2, NS], F8)
        ofull = opool.tile([128, 2, NS], BF16)
        CL = NS // 4
        for j in range(4):
            sl = slice(j * CL, (j + 1) * CL)
            nc.sync.dma_start(out=vfull[:, 0, sl], in_=v_in[0:128, sl])
            nc.gpsimd.dma_start(out=vfull[:, 1, sl], in_=v_in[128:256, sl])

        for c0 in range(0, NS, 512):
            for mb in range(2):
                msz = 128 if mb == 0 else 64
                ps = pspool.tile([128, 512], F32, tag="ps")
                nc.tensor.matmul(ps[:msz, :], aw[:, :, mb * 128:mb * 128 + msz],
                                 vfull[:, :, c0:c0 + 512], start=True, stop=True,
                                 perf_mode=DR)
                if (c0 // 512 + mb) % 4 != 3:
                    nc.scalar.activation(out=ofull[:msz, mb, c0:c0 + 512],
                                         in_=ps[:msz, :],
                                         func=mybir.ActivationFunctionType.Identity,
                                         bias=pb[:msz, mb:mb + 1],
                                         scale=1.0 / (AWS * VS))
                else:
                    nc.vector.tensor_scalar(
                        out=ofull[:msz, mb, c0:c0 + 512], in0=ps[:msz, :],
                        scalar1=1.0 / (AWS * VS), scalar2=pb[:msz, mb:mb + 1],
                        op0=mybir.AluOpType.mult, op1=mybir.AluOpType.add)
            if c0 % 2048 == 2048 - 512:
                b0 = c0 + 512 - 2048
                eng = nc.sync if (b0 // 2048) % 2 == 0 else nc.gpsimd
                eng.dma_start(out=out[0:128, b0:b0 + 2048], in_=ofull[:, 0, b0:b0 + 2048])
                eng.dma_start(out=out[128:192, b0:b0 + 2048], in_=ofull[:64, 1, b0:b0 + 2048])
    nc.compile()
    return nc


# ---------------- host-side prep ----------------

def prep_weights(w):
    """w: dict of reference weights (numpy f32). Returns dict of L1 input arrays."""
    out = {}
    qw = w["q_w"][:, :, 0, 0]          # (576, 192)
    qwT = np.zeros((128, 2, 768), np.float32)
    for (pb, h, x1b, qb, ch0, nch) in qkv_halves():
        win = 128 * pb + 64 * h
        qwT[0:128, 0, win:win + nch] = qw.T[0:128, ch0:ch0 + nch]
        qwT[0:64, 1, win:win + nch] = qw.T[128:192, ch0:ch0 + nch]
    out["qw8"] = (qwT * WS).astype(f8)

    # qdw: grouped 3x3, (tap, pb) 128x128 block; repack into DR tap pairs
    qdw = w["qdw_w"]                   # (576, 3, 3, 3)
    qdwT = np.zeros((9, 6, 128, 128), np.float32)   # (tap_sorted, pb, row, col)
    for ti, (dy, dx) in enumerate(TAPS):
        for (pb, h, x1b, qb, ch0, nch) in qkv_halves():
            for gl in range(nch // 3):
                for i in range(3):
                    for j in range(3):
                        qdwT[ti, pb, x1b + 3 * gl + i, qb + 3 * gl + j] = \
                            qdw[ch0 + 3 * gl + j, i, dy + 1, dx + 1]
    qdw8 = np.zeros((128, 6, 5, 2, 128), np.float32)
    for pj, (ta, tb) in enumerate(BD_PAIRS):
        for pb in range(6):
            qdw8[:, pb, pj, 0, :] = qdwT[ta, pb]
            if tb is not None:
                qdw8[:, pb, pj, 1, :] = qdwT[tb, pb]
    out["qdw8"] = (qdw8 * WS).astype(f8)

    # kv conv: 14 k-tiles -> 7 DR pairs (order must match CPAIRS in build_l1)
    kvw = w["kv_w"]                    # (384, 192, 3, 3)
    tl = lambda dy, dx: kvw[:, 0:128, dy + 1, dx + 1].T       # (128, 384) lo
    th = lambda dy, dx: kvw[:, 128:192, dy + 1, dx + 1].T     # (64, 384) hi
    ktiles = []
    for dy, dx in TAPS:   # 9 lo tiles (sorted tap order == TAPS order)
        ktiles.append(("lo", (dy, dx)))
    ktiles.append(("d1", -1)); ktiles.append(("d1", 0)); ktiles.append(("d1", 1))
    ktiles.append(("single", None)); ktiles.append(("d2", None))
    kv8 = np.zeros((128, 7, 2, 384), np.float32)
    for pj in range(7):
        for j in range(2):
            kind, arg = ktiles[2 * pj + j]
            blk = np.zeros((128, 384), np.float32)
            if kind == "lo":
                blk[0:128] = tl(*arg)
            elif kind == "d1":
                blk[0:64] = th(-1, arg)
                blk[64:128] = th(0, arg)
            elif kind == "d2":
                blk[0:64] = th(1, -1)
                blk[64:128] = th(1, 1)
            elif kind == "single":
                blk[0:64] = th(1, 0)
            kv8[:, pj, j, :] = blk
    out["kv8"] = (kv8 * WS).astype(f8)

    # kvdw depthwise: plain layout, diag blocks per pblock, DR tap pairs
    kvdw = w["kvdw_w"][:, 0]           # (384, 3, 3)
    kvdw8 = np.zeros((128, 3, 5, 2, 128), np.float32)
    for pj, (ta, tb) in enumerate(BD_PAIRS):
        for pb in range(3):
            dya, dxa = TAPS[ta]
            kvdw8[:, pb, pj, 0, :] = np.diag(kvdw[128 * pb:128 * pb + 128, dya + 1, dxa + 1])
            if tb is not None:
                dyb, dxb = TAPS[tb]
                kvdw8[:, pb, pj, 1, :] = np.diag(kvdw[128 * pb:128 * pb + 128, dyb + 1, dxb + 1])
    out["kvdw8"] = (kvdw8 * WS).astype(f8)

    def dw_channel(pb, p):
        return 128 * pb + p   # plain layout

    # newk (no bias on device; kT = NKS * k0): contraction sources
    # pass0 = (k8 slot0 = qkv pb2, k8 slot1 = qkv pb3)
    # pass1 = (dwk8 slot0 = dw pb0, dwk8 slot1 = dw pb1 parts 0..63)
    KCC = [("qkv", 2, 0, 128), ("qkv", 3, 0, 128), ("dw", 0, 0, 128), ("dw", 1, 0, 64)]
    nk = w["newk_w"][:, :, 0, 0]       # (192, 384): in = [k(192) | k_mask(192)]
    nkm = np.zeros((128, 4, 192), np.float32)
    for j, (src, pb, base, sz) in enumerate(KCC):
        for p in range(base, base + sz):
            if src == "qkv":
                ch = qkv_channel_at(pb, p)
                if ch is not None:
                    nkm[p, j, :] = nk[:, ch - 192]      # k part: qkv ch 192-383
            else:
                ch = dw_channel(pb, p)
                if ch < 192:
                    nkm[p, j, :] = nk[:, 192 + ch]      # k_mask: dw ch 0-191
    out["nkw8"] = (nkm.reshape(128, 2, 2, 192) * NKS).astype(f8)

    nv = w["newv_w"][:, :, 0, 0]       # (192, 384): in = [v(192) | v_mask(192)]
    nvT = np.zeros((128, 4, 192), np.float32)
    VCC_P = [("qkv", 4, 0, 128), ("qkv", 5, 0, 128), ("dw", 1, 64, 64), ("dw", 2, 0, 128)]
    for j, (src, pb, base, sz) in enumerate(VCC_P):
        for p in range(base, base + sz):
            if src == "qkv":
                ch = qkv_channel_at(pb, p)
                if ch is not None:
                    nvT[p, j, :] = nv[:, ch - 384]      # v part: qkv ch 384-575
            else:
                ch = dw_channel(pb, p)
                if ch >= 192:
                    nvT[p, j, :] = nv[:, ch]            # v_mask: dw ch 192-383
    out["nvw8"] = (nvT.reshape(128, 2, 2, 192) * NVS).astype(f8)

    out["ident"] = np.eye(128, dtype=bf16)

    x1b = np.zeros((128, 6), np.float32)
    qkvb = np.zeros((128, 6), np.float32)
    for (pb, h, x1b_base, qb, ch0, nch) in qkv_halves():
        x1b[x1b_base:x1b_base + nch, pb] = w["q_b"][ch0:ch0 + nch]
        qkvb[qb:qb + nch, pb] = w["qdw_b"][ch0:ch0 + nch]
    # hijacked ones rows: junk q pb1 partitions become the exact constant
    # 1/QTS, which the qT8 scale turns into 1.0 -> gramB rows = ksum/kss
    for r in ONE_ROWS:
        qkvb[r, 1] = 1.0 / QTS
    out["x1_bias"] = x1b * WS
    out["qkv_bias"] = qkvb

    kvb = np.zeros((128, 3), np.float32)
    dwb = np.zeros((128, 3), np.float32)
    for pb in range(3):
        kvb[:, pb] = w["kv_b"][128 * pb:128 * pb + 128]
        dwb[:, pb] = w["kvdw_b"][128 * pb:128 * pb + 128]
    out["kv_bias"] = kvb * WS
    out["kvdw_bias"] = dwb
    nvb = np.zeros((128, 2), np.float32)
    nvb[:, 0] = w["newv_b"][0:128]
    nvb[0:64, 1] = w["newv_b"][128:192]
    out["newv_bias"] = nvb * VS
    return out


def prep_core(x, xm, b, half, R, H):
    xp = np.zeros((192, R + 2, WP), np.float32)
    mp = np.zeros((192, R + 4, WP), np.float32)
    for r in range(R + 2):
        g = half * R + (r - 1)
        if 0 <= g < H:
            xp[:, r, 1:129] = x[b, :, g, :]
    for r in range(R + 4):
        g = half * R + (r - 2)
        if 0 <= g < H:
            mp[:, r, 1:129] = xm[b, :, g, :]
    xp = xp.reshape(192, -1)
    x8 = np.zeros((128, 2, xp.shape[1]), np.float32)
    x8[:, 0, :] = xp[0:128]
    x8[0:64, 1, :] = xp[128:192]
    x8[64:128, 1, :] = xp[128:192]   # dup (weights zero) to avoid NaN garbage
    mp = mp.reshape(192, -1)
    L = mp.shape[1]
    hi = mp[128:192]
    d1 = np.zeros((128, L), np.float32)
    d2 = np.zeros((128, L), np.float32)
    d1[0:64] = hi
    d1[64:128, :L - WP] = hi[:, WP:]
    d2[0:64] = hi
    d2[64:128, :L - 2] = hi[:, 2:]
    edge = np.ones((128, 2), np.float32)
    if half == 0:
        edge[:, 0] = 0.0     # slab0 row0 = image row -1
    else:
        edge[:, 1] = 0.0     # last slab last row = image row H
    return {
        "x8": x8.astype(f8),
        "xm_lo": mp[0:128].astype(f8),
        "xm_d1": d1.astype(f8), "xm_d2": d2.astype(f8),
        "edge": edge,
    }


# ---------------- host glue (unchanged semantics) ----------------

def _q_maps():
    part = np.zeros(192, np.int64)
    pblk = np.zeros(192, np.int64)
    for (pb, h, x1b, qb, ch0, nch) in qkv_halves():
        if pb >= 2:
            continue
        for i in range(nch):
            pblk[ch0 + i] = pb
            part[ch0 + i] = qb + i
    return pblk, part


def _ss_from_qstats(stats, n_half):
    pblk, part = _q_maps()
    mv = stats.astype(np.float64)
    return (mv[part, pblk, 1] + mv[part, pblk, 0] ** 2) * n_half


def _sum_from_qstats(stats, n_half):
    pblk, part = _q_maps()
    mv = stats.astype(np.float64)
    return mv[part, pblk, 0] * n_half


def _ss_from_vstats(stats, newv_b, n_half):
    # stats are of NVS*(v - b): undo the scale, add back the bias
    m = np.zeros(192, np.float64)
    w = np.zeros(192, np.float64)
    mv = stats.astype(np.float64)
    m[0:128] = mv[0:128, 0, 0]
    w[0:128] = mv[0:128, 0, 1]
    m[128:192] = mv[0:64, 1, 0]
    w[128:192] = mv[0:64, 1, 1]
    m = m / NVS + newv_b.astype(np.float64)
    w = w / (NVS * NVS)
    return (w + m * m) * n_half


def glue(res0, res1, newv_b, n_half):
    """Combine two half-core L1 results into raw sums."""
    GT = res0["gramT_out"].astype(np.float64) + res1["gramT_out"].astype(np.float64)
    pblk, part = _q_maps()
    qrow = pblk * 128 + part
    # GT rows are qT cols (pb*128+part); cols are newk out-ch d. Stored values
    # are sum_n (QTS*q) * (NKS*k0).
    Gq = GT[qrow, :] / (QTS * NKS)             # (c, d): sum_n q[c,n] k0[d,n]
    qss = _ss_from_qstats(res0["qstats_out"], n_half) + _ss_from_qstats(res1["qstats_out"], n_half)
    qsum = _sum_from_qstats(res0["qstats_out"], n_half) + _sum_from_qstats(res1["qstats_out"], n_half)
    vss = _ss_from_vstats(res0["vstats_out"], newv_b, n_half) + \
        _ss_from_vstats(res1["vstats_out"], newv_b, n_half)
    kss_raw = (res0["kss_out"].astype(np.float64) + res1["kss_out"].astype(np.float64))[0]
    k0sum = kss_raw[0:192] / NKS               # sum_n k0 (ones x kT cols)
    k0ss = kss_raw[192:384] / (KSQS * KSQS)    # sum_n k0^2 (ones x ksq cols)
    return Gq, qss, qsum, vss, k0ss, k0sum


def glue_full(res0, res1, temperature, newk_b, newv_b, proj_w, proj_b, n_half):
    Gq, qss, qsum, vss, k0ss, k0sum = glue(res0, res1, newv_b, n_half)
    b = newk_b.astype(np.float64)              # (192,)
    # k = k0 + b: gram/kss bias corrections (sums already cover both halves,
    # total n = 2 * n_half)
    G = Gq + qsum[:, None] * b[None, :]        # (c, d): sum_n q k
    kss = k0ss + 2 * b * k0sum + (2 * n_half) * b * b
    qn = np.maximum(np.sqrt(qss), 1e-12)
    kn = np.maximum(np.sqrt(kss), 1e-12)
    vn = np.maximum(np.sqrt(vss), 1e-12)
    A = G / (qn[:, None] * kn[None, :])        # (c, d)
    M = np.zeros((192, 192), np.float64)
    t = np.asarray(temperature).reshape(-1)
    for h in range(8):
        sl = slice(24 * h, 24 * h + 24)
        a = A[sl, sl] * t[h]
        a = a - a.max(axis=-1, keepdims=True)
        e = np.exp(a)
        sm = e / e.sum(axis=-1, keepdims=True)
        M[sl, sl] = sm / vn[None, sl]
    At = proj_w[:, :, 0, 0].astype(np.float64) @ M   # (out-ch o, d)
    awT = np.zeros((128, 2, 192), np.float32)
    awT[:, 0, :] = At.T[0:128] * AWS
    awT[0:64, 1, :] = At.T[128:192] * AWS
    pbias = np.zeros((128, 2), np.float32)
    pbias[:, 0] = proj_b[0:128]
    pbias[0:64, 1] = proj_b[128:192]
    return {"awT": awT.astype(f8), "pbias": pbias}


# ---------------- driver: kernel(**inputs) ----------------
from concourse.bass_utils import run_bass_kernel_spmd

R_FULL, H_FULL, B_FULL = 64, 128, 4
_NC1 = None
_NC2 = None


def _get_progs():
    global _NC1, _NC2
    if _NC1 is None:
        _NC1 = build_l1(R=R_FULL, S=16)
        _NC2 = build_l2(R=R_FULL)
    return _NC1, _NC2


def kernel(**inputs):
    inputs = {k: np.asarray(v) for k, v in inputs.items()}
    x, xm = inputs["x"], inputs["x_mask"]
    nc1, nc2 = _get_progs()
    wprep = prep_weights(inputs)
    in_maps = []
    for core in range(8):
        b, half = core // 2, core % 2
        m = dict(wprep)
        m.update(prep_core(x, xm, b, half, R_FULL, H_FULL))
        in_maps.append(m)
    res1 = run_bass_kernel_spmd(nc1, in_maps, list(range(8))).results

    n_half = R_FULL * 128
    in_maps2 = []
    for core in range(8):
        b, half = core // 2, core % 2
        if half == 0:
            l2c = glue_full(res1[2 * b], res1[2 * b + 1], inputs["temperature"],
                            inputs["newk_b"], inputs["newv_b"],
                            inputs["proj_w"], inputs["proj_b"], n_half)
        m = dict(l2c)
        vo = res1[core]["v_out"]
        m["v_in"] = np.concatenate([vo, np.zeros((64, vo.shape[1]), vo.dtype)], axis=0)
        in_maps2.append(m)
    res2 = run_bass_kernel_spmd(nc2, in_maps2, list(range(8))).results

    out = np.empty((B_FULL, 192, H_FULL, 128), np.float32)
    for core in range(8):
        b, half = core // 2, core % 2
        out[b, :, half * R_FULL:(half + 1) * R_FULL, :] = \
            res2[core]["out"].reshape(192, R_FULL, 128).astype(np.float32)
    return out
